# revision 22
# baseline (speedup 1.0000x reference)
"""Trainium2 Bass kernel for nn_AttentionNet (axial linear-attention net).

Sharding: cores 0-3 hold batch b=0, cores 4-7 hold b=1. Within a 4-core
group the sequence axis L=512 is split into 4 shards of 128. Every core
holds ALL 780 pairs for its (b, l-shard), so the instruction stream is
identical on all cores (pure SPMD) and only the input data differs.

Residual state per core: 98 SBUF tiles [128, 512] fp16:
  partition = g*64 + n*16 + d   (g = pair-half 0/1, n = head, d = head ch)
  free      = q*128 + l         (q = pair-quad 0..3, l = local seq pos)
  tile t holds pair slots 8t + 4g + q (784 slots = 780 pairs + 4 pads).

v2 design notes:
- LayerNorm affine (gamma/beta) is folded host-side into every consumer
  matmul; the residual stream stores the UN-affined normalized value
  (h-tilde) and residual adds re-apply gamma via the stt scalar slot and
  beta via a 1-partition bias matmul accumulated into the attention/FFN
  output PSUM.
- LN apply uses a DMA partition-broadcast of [rstd | mean*rstd] rows to
  128 partitions, then two 2x-mode f16 TensorTensor ops. No per-tile
  apply matmuls, no PSUM reads on the apply path.
- LN statistics are computed in groups of 32 tiles (one-hot stat matmuls
  into two persistent PSUM banks).
- Row attention uses affine_mul_reduce to fuse (v+bias)*kbar with the
  per-quad KtV reduction; PSUM->SBUF drains ride the Activation engine.
- Engine balance: DVE keeps the PSUM-coupled ops, Act does elu/gelu
  chains + drains, Pool (gpsimd) takes pure-SBUF squares/multiplies,
  the DMA engines do the LN broadcasts.
"""

import contextlib
import sys

import numpy as np

sys.path.insert(0, "/opt/trn_rl_repo")

mybir = None
F32 = F16 = AF = ALU = None


def _lazy_imports():
    global mybir, F32, F16, AF, ALU
    if mybir is None:
        import concourse.mybir as _mybir
        mybir = _mybir
        F32, F16 = mybir.dt.float32, mybir.dt.float16
        AF = mybir.ActivationFunctionType
        ALU = mybir.AluOpType

NB_SEQ = 40
SEQ_LEN = 512
NB_PAIRS = 780
B = 2
N_BLOCKS = 2
CIN = 22

N_CORES = 8
LSH = 128            # l per core
NQ = 4               # quads per tile
NT = 98              # hp tiles per core
GSZ = 32             # LN group size (tiles)
NGROUP = (NT + GSZ - 1) // GSZ       # 4 (32,32,32,2)
OGSZ = 16
NOG = (NT + OGSZ - 1) // OGSZ        # 7, output stage groups
FD = NQ * LSH        # 512, tile free size


def _pair_order():
    order = []
    for d in range(1, NB_SEQ):
        for i in range(NB_SEQ - d):
            order.append((i, i + d))
    return order


PAIRS = _pair_order()


def slot_ij(s):
    return PAIRS[s] if s < NB_PAIRS else PAIRS[0]


# ================================================================ weights
def prep_weights(inp):
    """Pack all constants; LN affine folded into consumer weights."""
    w = {}
    f16 = lambda a: np.ascontiguousarray(a, dtype=np.float16)
    f32 = lambda a: np.ascontiguousarray(a, dtype=np.float32)

    def col(v, n=128):
        v = np.asarray(v, np.float32).reshape(-1)
        if v.size == 64:
            v = np.tile(v, 2)
        if v.size == 1:
            v = np.full(n, v[0], np.float32)
        return f32(v.reshape(n, 1))

    w_in = np.asarray(inp["w_in"])
    w["wconv"] = f16(np.concatenate([w_in.T, w_in.T], axis=1))
    w["bconv"] = col(inp["b_in"])

    # LN instances: 0 = (g0, be0); k+1 = block-k (ln_g[k], ln_b[k]).
    lns = [(np.asarray(inp["g0"], np.float32),
            np.asarray(inp["be0"], np.float32))]
    for k in range(N_BLOCKS):
        lns.append((np.asarray(inp["ln_g"][k], np.float32),
                    np.asarray(inp["ln_b"][k], np.float32)))

    def bd(m, gamma):
        # block-diag lhsT with gamma folded into input columns
        mt = (np.asarray(m, np.float32) * gamma[None, :]).T
        z = np.zeros((128, 128), np.float16)
        z[:64, :64] = mt
        z[64:, 64:] = mt
        return z

    def fold_bias(m, beta, b):
        return np.asarray(m, np.float32) @ beta + np.asarray(b, np.float32)

    # LN instance feeding each phase's input:
    #  rowA/rowB(k): instance k (ln0 for k=0, ffn-LN of k-1 else)
    #  colA/colB(k): instance k+1 (row-LN of block k)
    #  ffn(k):       instance k+1 (col-LN of block k)
    for k in range(N_BLOCKS):
        gr, br_ = lns[k]        # feeds row attention
        gc, bc_ = lns[k + 1]    # feeds col attention and ffn
        for nm, wk, bk, g_, b_ in [
                ("rq", "rqw", "rqb", gr, br_), ("rk", "rkw", "rkb", gr, br_),
                ("rv", "rvw", "rvb", gr, br_), ("cq", "cqw", "cqb", gc, bc_),
                ("ck", "ckw", "ckb", gc, bc_), ("cv", "cvw", "cvb", gc, bc_)]:
            m = np.asarray(inp[wk][k])
            w[f"{nm}{k}"] = f16(bd(m, g_))
            bb = fold_bias(m, b_, inp[bk][k])
            w[f"{nm}b{k}"] = col(bb)
            w[f"{nm}b1{k}"] = col(bb + 1.0)
            w[f"{nm}bn{k}"] = col(-bb)
        # output projections: plain weights; bias handled by bias-row MM
        for nm, wk in [("rp", "rpw"), ("cp", "cpw")]:
            m = np.asarray(inp[wk][k])
            z = np.zeros((128, 128), np.float16)
            z[:64, :64] = m.T
            z[64:, 64:] = m.T
            w[f"{nm}{k}"] = f16(z)
        # residual bias rows: rp-bias + beta of the residual LN instance
        rpb = np.asarray(inp["rpb"][k], np.float32)
        cpb = np.asarray(inp["cpb"][k], np.float32)
        w[f"rpbrow{k}"] = f16(np.tile(rpb + br_, 2).reshape(1, 128))
        w[f"cpbrow{k}"] = f16(np.tile(cpb + bc_, 2).reshape(1, 128))
        # residual gamma cols
        w[f"rgcol{k}"] = col(gr)
        w[f"cgcol{k}"] = col(gc)
        w[f"fgcol{k}"] = col(gc)

        f1w = np.asarray(inp["f1w"][k], np.float32) * gc[None, :]
        f1b = np.asarray(inp["f1w"][k], np.float32) @ bc_ \
            + np.asarray(inp["f1b"][k], np.float32)
        f2w = np.asarray(inp["f2w"][k])
        for j in range(4):
            g, hh = j // 2, (j % 2) * 128
            lt = np.zeros((128, 128), np.float16)
            lt[g * 64:(g + 1) * 64, :] = f1w[hh:hh + 128, :].T
            w[f"f1_{k}_{j}"] = f16(lt)
            lt2 = np.zeros((128, 128), np.float16)
            lt2[:, g * 64:(g + 1) * 64] = f2w[:, hh:hh + 128].T
            w[f"f2_{k}_{j}"] = f16(lt2)
            w[f"f1b_{k}_{j}"] = f32(f1b[hh:hh + 128].reshape(128, 1))
        f2b = np.asarray(inp["f2b"][k], np.float32)
        w[f"f2brow{k}"] = f16(np.tile(f2b + bc_, 2).reshape(1, 128))
        # last block's ffn has no LN after; residual gamma is col instance
        # (gc) because hp there is the col-LN output of block k.

    # conv + gather feed ln0; gather output is raw (pre-LN) so the rowA
    # weights above already fold ln0 -> nothing extra here.

    # stat slabs for GSZ-tile groups: per tau [128, 64]:
    # rows 2*tau+g get ones at partitions g*64..g*64+64
    stat = np.zeros((128, GSZ * 64), np.float16)
    for tau in range(GSZ):
        for g in range(2):
            stat[g * 64:(g + 1) * 64, tau * 64 + 2 * tau + g] = 1.0
    w["stat_lt"] = f16(stat)

    # output stage (16-tile groups)
    outw = np.zeros((128, OGSZ * 32), np.float16)
    wo = np.asarray(inp["wout"], np.float32).reshape(-1)
    for tau in range(OGSZ):
        for g in range(2):
            outw[g * 64:(g + 1) * 64, tau * 32 + 2 * tau + g] = wo
    w["outw_lt"] = f16(outw)
    w["boutc"] = f32(np.full((32, 1), np.asarray(inp["bout"]).reshape(-1)[0],
                             np.float32))
    w["epsc"] = f32(np.full((64, 1), 1e-5, np.float32))
    w["onec"] = f32(np.full((32, 1), 1.0, np.float32))

    p8 = np.zeros((128, 128), np.float16)
    for blk in range(8):
        p8[blk * 16:(blk + 1) * 16, blk * 16:(blk + 1) * 16] = 1.0
    w["P8"] = f16(p8)
    h64 = np.zeros((128, 64), np.float16)
    h64[np.arange(128), np.arange(128) % 64] = 1.0
    w["H64"] = f16(h64)
    hlast = h64.copy()
    hlast[64:, :] = 0.0
    w["H64_last"] = f16(hlast)
    h64t = np.zeros((64, 128), np.float16)
    h64t[np.arange(128) % 64, np.arange(128)] = 1.0
    w["H64T"] = f16(h64t)

    # ---- pack into two tensors ----
    s16, s32 = _pack_layout()
    p16 = np.zeros((128, s16[-1][2] + s16[-1][3]), np.float16)
    for name, rows, off, cols in s16:
        p16[:rows, off:off + cols] = w[name]
    p32 = np.zeros((128, s32[-1][2] + s32[-1][3]), np.float32)
    for name, rows, off, cols in s32:
        p32[:rows, off:off + cols] = w[name]
    w["wpack16"] = p16
    w["wpack32"] = p32
    return w


def _pack_layout():
    e16, e32 = [], []
    o16 = o32 = 0

    def a16(name, rows, cols):
        nonlocal o16
        e16.append((name, rows, o16, cols))
        o16 += cols

    def a32(name, rows, cols):
        nonlocal o32
        e32.append((name, rows, o32, cols))
        o32 += cols

    a16("wconv", CIN, 128)
    a32("bconv", 128, 1)
    for k in range(N_BLOCKS):
        for nm in ["rq", "rk", "rv", "cq", "ck", "cv"]:
            a16(f"{nm}{k}", 128, 128)
            a32(f"{nm}b{k}", 128, 1)
            a32(f"{nm}b1{k}", 128, 1)
            a32(f"{nm}bn{k}", 128, 1)
        for nm in ["rp", "cp"]:
            a16(f"{nm}{k}", 128, 128)
        a16(f"rpbrow{k}", 1, 128)
        a16(f"cpbrow{k}", 1, 128)
        a32(f"rgcol{k}", 128, 1)
        a32(f"cgcol{k}", 128, 1)
        a32(f"fgcol{k}", 128, 1)
        for j in range(4):
            a16(f"f1_{k}_{j}", 128, 128)
            a16(f"f2_{k}_{j}", 128, 128)
            a32(f"f1b_{k}_{j}", 128, 1)
        a16(f"f2brow{k}", 1, 128)
    a16("stat_lt", 128, GSZ * 64)
    a16("outw_lt", 128, OGSZ * 32)
    a32("boutc", 32, 1)
    a32("epsc", 64, 1)
    a32("onec", 32, 1)
    a16("P8", 128, 128)
    a16("H64", 128, 64)
    a16("H64_last", 128, 64)
    a16("H64T", 64, 128)
    return e16, e32


WEIGHT_SPECS = []


def _spec():
    e16, e32 = _pack_layout()
    n16 = e16[-1][2] + e16[-1][3]
    n32 = e32[-1][2] + e32[-1][3]
    return [("wpack16", (128, n16), F16), ("wpack32", (128, n32), F32)]


# ================================================================ views
def _q(ap):
    return ap.rearrange("p (q l) -> p q l", q=NQ)


def _bq(ap_col4):
    """[128, 4] slice -> [128, 4, 128] broadcast over l."""
    a = ap_col4.copy()
    a.ap = a.ap[:-1] + [list(a.ap[-1]), [0, LSH]]
    return a


def _bl(ap_l):
    """[128, 128] -> [128, 4, 128] broadcast over quads."""
    a = ap_l.copy()
    a.ap = a.ap[:-1] + [[0, NQ], list(a.ap[-1])]
    return a


def _brows(ap_2rows):
    """[2, F] rows -> broadcast to [2, 64, F] (DMA source: each row
    repeated 64x so the dest covers 128 partitions)."""
    a = ap_2rows.copy()
    a.ap = a.ap[:1] + [[0, 64], list(a.ap[-1])]
    return a


# ================================================================ kernel IR
def build_kernel():
    _lazy_imports()
    import concourse.bacc as bacc
    import concourse.tile as tile

    global WEIGHT_SPECS
    WEIGHT_SPECS = _spec()

    nc = bacc.Bacc("TRN2", target_bir_lowering=False, debug=False,
                   num_devices=N_CORES)
    xin_d = nc.dram_tensor("xin", [CIN, NB_SEQ, LSH], F16,
                           kind="ExternalInput").ap()
    wd = {}
    for name, shape, dtype in WEIGHT_SPECS:
        wd[name] = nc.dram_tensor(name, list(shape), dtype,
                                  kind="ExternalInput").ap()
    yout_d = nc.dram_tensor("yout", [32, 4 * NOG], F32,
                            kind="ExternalOutput").ap()

    with tile.TileContext(nc) as tc:
        _body(nc, tc, xin_d, wd, yout_d)
    nc.compile()
    return nc


def _body(nc, tc, xin_d, wd, yout_d):
    ctx = contextlib.ExitStack()
    ctx.enter_context(nc.allow_low_precision(
        reason="normalized activations; f16 everywhere is plenty for 2e-2"))
    P = 128

    wpool = ctx.enter_context(tc.tile_pool(name="w", bufs=1))
    hpool = ctx.enter_context(tc.tile_pool(name="hp", bufs=1))
    spool = ctx.enter_context(tc.tile_pool(name="scr", bufs=3))
    xpool = ctx.enter_context(tc.tile_pool(name="xpre", bufs=1))
    gpool = ctx.enter_context(tc.tile_pool(name="grp", bufs=1))
    stpool = ctx.enter_context(tc.tile_pool(name="st2", bufs=2))
    bpool = ctx.enter_context(tc.tile_pool(name="bc", bufs=3))
    ppool = ctx.enter_context(tc.tile_pool(name="ps", bufs=6, space="PSUM"))
    ppers = ctx.enter_context(tc.tile_pool(name="ps2", bufs=1, space="PSUM"))
    dpool = ctx.enter_context(tc.tile_pool(name="dram", bufs=1, space="DRAM"))

    e16, e32 = _pack_layout()
    n16 = e16[-1][2] + e16[-1][3]
    n32 = e32[-1][2] + e32[-1][3]
    pk16 = wpool.tile([128, n16], F16, tag="pk16", name="pk16")
    pk32 = wpool.tile([128, n32], F32, tag="pk32", name="pk32")
    nc.sync.dma_start(pk16[:], wd["wpack16"][:])
    nc.sync.dma_start(pk32[:], wd["wpack32"][:])
    W = {}
    for name, rows, off, cols in e16:
        W[name] = pk16[:rows, off:off + cols]
    for name, rows, off, cols in e32:
        W[name] = pk32[:rows, off:off + cols]

    hp = [hpool.tile([P, FD], F16, tag=f"hp{t}", name=f"hp{t}")
          for t in range(NT)]

    ksum_pr = wpool.tile([P, NQ * NT], F32, tag="ksum_pr")
    ktv_pr = wpool.tile([P, NQ * NT], F32, tag="ktv_pr")
    ksum_h = wpool.tile([P, NQ * NT], F16, tag="ksum_h")
    ktv_h = wpool.tile([P, NQ * NT], F16, tag="ktv_h")
    kc_b = wpool.tile([P, LSH], F16, tag="kc_b")
    tc_b = wpool.tile([P, LSH], F16, tag="tc_b")
    ones_row = wpool.tile([1, FD], F16, tag="ones_row")
    nc.vector.memset(ones_row[:], 1.0)

    # ============================================================ LN
    # producer(t) -> x_pre tile (REAL post-residual values).
    # sq_dve: engine for the square (True=DVE, False=Pool)
    # t1_dve: engine for the first apply multiply
    def ln_phase(producer, sq_dve=False, t1_dve=False):
        stages = producer if isinstance(producer, (list, tuple)) \
            else [producer]
        ns = len(stages)
        pending = []
        for gi in range(NGROUP):
            t0, tend = gi * GSZ, min(NT, gi * GSZ + GSZ)
            ntl = tend - t0
            s_ps = ppers.tile([64, FD], F32, tag="acc1")
            sq_ps = ppers.tile([64, FD], F32, tag="acc2")
            xs = [None] * ntl

            def finish(tau, x_pre, ntl=ntl, s_ps=s_ps, sq_ps=sq_ps, xs=xs,
                       t0=t0):
                xs[tau] = x_pre
                sq = spool.tile([P, FD], F16, tag="sq")
                use_dve = sq_dve if isinstance(sq_dve, bool) \
                    else (tau % 3 == 0)
                if use_dve:
                    nc.vector.tensor_mul(sq[:], x_pre[:], x_pre[:])
                else:
                    nc.gpsimd.tensor_mul(sq[:], x_pre[:], x_pre[:])
                sl = W["stat_lt"][:, tau * 64:tau * 64 + 64]
                nc.tensor.matmul(s_ps[:], sl, x_pre[:],
                                 start=(tau == 0), stop=(tau == ntl - 1))
                nc.tensor.matmul(sq_ps[:], sl, sq[:],
                                 start=(tau == 0), stop=(tau == ntl - 1))

            carry = {}
            for i in range(ntl + ns - 1):
                for si, f in enumerate(stages):
                    tau = i - si
                    if 0 <= tau < ntl:
                        r = f(t0 + tau, carry)
                        if si == ns - 1:
                            finish(tau, r)
                if pending:
                    pending.pop(0)()
                if pending:
                    pending.pop(0)()
            mu = gpool.tile([64, FD], F32, tag="ln_mu")
            e2 = gpool.tile([64, FD], F32, tag="ln_e2")
            msq = gpool.tile([64, FD], F32, tag="ln_msq")
            st2 = stpool.tile([64, 2 * FD], F16, tag="ln_st2")
            nc.scalar.activation(mu[:], s_ps[:], AF.Copy, scale=1.0 / 64)
            nc.scalar.activation(e2[:], sq_ps[:], AF.Copy, scale=1.0 / 64)
            nc.scalar.activation(msq[:], mu[:], AF.Square)
            nc.vector.tensor_sub(e2[:], e2[:], msq[:])
            nc.scalar.activation(msq[:], e2[:], AF.Ln, bias=W["epsc"])
            nc.scalar.activation(st2[:, :FD], msq[:], AF.Exp, scale=-0.5)
            nc.vector.tensor_mul(st2[:, FD:], mu[:], st2[:, :FD])
            while pending:
                pending.pop(0)()

            bds = {}

            def mk_dma(tau, st2=st2, bds=bds):
                def go():
                    bdst = bpool.tile([P, 2 * FD], F16, tag="bdst")
                    nc.sync.dma_start(bdst[:],
                                      _brows(st2[2 * tau:2 * tau + 2, :]))
                    bds[tau] = bdst
                return go

            def mk_cmp(tau, t0=t0, xs=xs, bds=bds):
                def go():
                    t = t0 + tau
                    bdst = bds.pop(tau)
                    t1 = spool.tile([P, FD], F16, tag="t1")
                    use_dve = t1_dve if isinstance(t1_dve, bool) \
                        else (tau % 2 == 0)
                    if use_dve:
                        nc.vector.tensor_mul(t1[:], xs[tau][:], bdst[:, :FD])
                    else:
                        nc.gpsimd.tensor_mul(t1[:], xs[tau][:], bdst[:, :FD])
                    nc.vector.tensor_sub(hp[t][:], t1[:], bdst[:, FD:])
                return go

            # DMA for apply j runs 2 queue slots ahead of its compute
            q = []
            for tau in range(ntl):
                q.append(mk_dma(tau))
            for tau in range(ntl):
                q.insert(2 * tau + 2 if 2 * tau + 2 < len(q) else len(q),
                         mk_cmp(tau))
            # interleave: after position-building above, q has dma j at
            # slot ~2j and cmp j at slot ~2j+2
            pending.extend(q)
        while pending:
            pending.pop(0)()

    # ============================================================ Phase 0
    h2 = wpool.tile([P, NB_SEQ * LSH], F16, tag="h2")
    xin_f = xin_d.rearrange("c s l -> c (s l)")
    for j in range(10):
        xst = spool.tile([CIN, FD], F16, tag="sq")
        nc.sync.dma_start(xst[:], xin_f[:, j * FD:(j + 1) * FD])
        cps = ppool.tile([P, FD], F32, tag="ps")
        nc.tensor.matmul(cps[:], W["wconv"], xst[:])
        nc.scalar.activation(h2[:, j * FD:(j + 1) * FD], cps[:],
                             AF.Relu, bias=W["bconv"])
    h2q = h2[:].rearrange("p (s l) -> p s l", s=NB_SEQ)

    def gather_producer(t, carry=None):
        x_pre = xpool.tile([P, FD], F16, tag=f"xp{t % GSZ}")
        xq = _q(x_pre[:])
        eng = nc.vector
        for g in range(2):
            ij = [slot_ij(8 * t + 4 * g + q) for q in range(NQ)]
            iis = [a for a, _ in ij]
            jjs = [b for _, b in ij]
            rows = slice(g * 64, g * 64 + 64)
            if (all(iis[q] == iis[0] + q for q in range(NQ)) and
                    all(jjs[q] == jjs[0] + q for q in range(NQ))):
                eng.tensor_add(xq[rows, :, :],
                               h2q[rows, iis[0]:iis[0] + NQ, :],
                               h2q[rows, jjs[0]:jjs[0] + NQ, :])
            else:
                for q in range(NQ):
                    eng.tensor_add(xq[rows, q, :],
                                   h2q[rows, iis[q], :],
                                   h2q[rows, jjs[q], :])
        return x_pre

    ln_phase(gather_producer, sq_dve="mix", t1_dve="mix")

    # ============================================================ blocks
    for k in range(N_BLOCKS):

        # ---- row attention phase A: k/v, local partials (3-stage skew) ---
        rc = {}

        def rowA_a(t, k=k):
            k_ps = ppool.tile([P, FD], F32, tag="ps")
            nc.tensor.matmul(k_ps[:], W[f"rk{k}"], hp[t][:])
            mk = spool.tile([P, FD], F16, tag="mk")
            ek = spool.tile([P, FD], F16, tag="ek")
            nc.scalar.activation(mk[:], k_ps[:], AF.Relu, scale=-1.0,
                                 bias=W[f"rkbn{k}"])
            nc.scalar.activation(ek[:], mk[:], AF.Exp, scale=-1.0)
            rc[t] = (k_ps, ek)

        def rowA_b(t, k=k):
            k_ps, ek = rc.pop(t)
            kt = spool.tile([P, FD], F16, tag="ktil")
            nc.vector.scalar_tensor_tensor(
                kt[:], k_ps[:], W[f"rkb1{k}"], ek[:], ALU.add, ALU.max)
            nc.vector.tensor_reduce(ksum_pr[:, NQ * t:NQ * t + NQ],
                                    _q(kt[:]), mybir.AxisListType.X, ALU.add)
            kb_ps = ppool.tile([P, FD], F32, tag="ps")
            nc.tensor.matmul(kb_ps[:], W["P8"], kt[:])
            kb = spool.tile([P, FD], F16, tag="kbsb")
            nc.scalar.activation(kb[:], kb_ps[:], AF.Identity)
            v_ps = ppool.tile([P, FD], F32, tag="ps")
            nc.tensor.matmul(v_ps[:], W[f"rv{k}"], hp[t][:])
            rc[("b", t)] = (kb, v_ps)

        def rowA_c(t, k=k):
            kb, v_ps = rc.pop(("b", t))
            vw = spool.tile([P, FD], F16, tag="vw")
            for q in range(NQ):
                sl = slice(q * LSH, (q + 1) * LSH)
                nc.vector.affine_mul_reduce(
                    vw[:, sl], ktv_pr[:, NQ * t + q:NQ * t + q + 1],
                    v_ps[:, sl], kb[:, sl], 1.0, W[f"rvb{k}"])

        # AllReduce in two halves: first half overlaps rowA's tail.
        TSPLIT = 64

        def ar(lo, hi, half, k=k):
            c0, c1 = NQ * lo, NQ * hi
            n = c1 - c0
            bin_ = dpool.tile([P, 2 * n], F32, tag=f"arin{k}_{half}")
            bout_ = dpool.tile([P, 2 * n], F32, tag=f"arout{k}_{half}")
            nc.sync.dma_start(bin_[:, :n], ksum_pr[:, c0:c1])
            nc.sync.dma_start(bin_[:, n:], ktv_pr[:, c0:c1])
            nc.gpsimd.collective_compute(
                "AllReduce", ALU.add,
                replica_groups=[[0, 1, 2, 3], [4, 5, 6, 7]],
                ins=[bin_.opt()], outs=[bout_.opt()])
            nc.sync.dma_start(ksum_pr[:, c0:c1], bout_[:, :n])
            nc.sync.dma_start(ktv_pr[:, c0:c1], bout_[:, n:])
            nc.vector.tensor_copy(ksum_h[:, c0:c1], ksum_pr[:, c0:c1])
            nc.vector.tensor_copy(ktv_h[:, c0:c1], ktv_pr[:, c0:c1])

        for i in range(NT + 2):
            if i < NT:
                rowA_a(i)
            if 1 <= i <= NT:
                rowA_b(i - 1)
            if i >= 2:
                rowA_c(i - 2)
            if i == TSPLIT + 2:
                ar(0, TSPLIT, 0)
        ar(TSPLIT, NT, 1)

        # ---- row attention phase B (3-stage skew producer) ---------------
        def row_b_a(t, carry, k=k):
            q_ps = ppool.tile([P, FD], F32, tag="ps")
            nc.tensor.matmul(q_ps[:], W[f"rq{k}"], hp[t][:])
            mq = spool.tile([P, FD], F16, tag="mk")
            eq = spool.tile([P, FD], F16, tag="ek")
            qb1 = spool.tile([P, FD], F16, tag="kbsb")
            nc.scalar.activation(mq[:], q_ps[:], AF.Relu, scale=-1.0,
                                 bias=W[f"rqbn{k}"])
            nc.scalar.activation(eq[:], mq[:], AF.Exp, scale=-1.0)
            nc.scalar.activation(qb1[:], q_ps[:], AF.Identity,
                                 bias=W[f"rqb1{k}"])
            carry[t] = (eq, qb1)

        def row_b_b(t, carry, k=k):
            eq, qb1 = carry.pop(t)
            qt = spool.tile([P, FD], F16, tag="ktil")
            nc.vector.tensor_max(qt[:], qb1[:], eq[:])
            prod = spool.tile([P, FD], F16, tag="vw")
            nc.vector.tensor_tensor(_q(prod[:]), _q(qt[:]),
                                    _bq(ksum_h[:, NQ * t:NQ * t + NQ]),
                                    ALU.mult)
            dn_ps = ppool.tile([P, FD], F32, tag="ps")
            nc.tensor.matmul(dn_ps[:], W["P8"], prod[:])
            carry[("b", t)] = dn_ps

        def row_b_c(t, carry, k=k):
            dn_ps = carry.pop(("b", t))
            z = spool.tile([P, FD], F16, tag="z")
            nc.vector.reciprocal(z[:], dn_ps[:])
            V = spool.tile([P, FD], F16, tag="V")
            nc.vector.tensor_tensor(_q(V[:]), _q(z[:]),
                                    _bq(ktv_h[:, NQ * t:NQ * t + NQ]),
                                    ALU.mult)
            att_ps = ppool.tile([P, FD], F32, tag="ps")
            nc.tensor.matmul(att_ps[:], W[f"rpbrow{k}"], ones_row[:],
                             start=True, stop=False)
            nc.tensor.matmul(att_ps[:], W[f"rp{k}"], V[:],
                             start=False, stop=True)
            x_pre = xpool.tile([P, FD], F16, tag=f"xp{t % GSZ}")
            nc.vector.scalar_tensor_tensor(
                x_pre[:], hp[t][:], W[f"rgcol{k}"], att_ps[:],
                ALU.mult, ALU.add)
            return x_pre

        ln_phase([row_b_a, row_b_b, row_b_c], sq_dve="mix", t1_dve="mix")

        # ---- column attention phase A: k/v + local pair reduction --------
        kc_ps = ppers.tile([64, FD], F32, tag="acc1")
        tv_ps = ppers.tile([64, FD], F32, tag="acc2")
        cc = {}

        def colA_a(t, k=k):
            ck_ps = ppool.tile([P, FD], F32, tag="ps")
            nc.tensor.matmul(ck_ps[:], W[f"ck{k}"], hp[t][:])
            mk = spool.tile([P, FD], F16, tag="mk")
            ek = spool.tile([P, FD], F16, tag="ek")
            nc.scalar.activation(mk[:], ck_ps[:], AF.Relu, scale=-1.0,
                                 bias=W[f"ckbn{k}"])
            nc.scalar.activation(ek[:], mk[:], AF.Exp, scale=-1.0)
            cc[t] = (ck_ps, ek)

        def colA_b(t, k=k):
            ck_ps, ek = cc.pop(t)
            h64 = W["H64_last"] if t == NT - 1 else W["H64"]
            kt = spool.tile([P, FD], F16, tag="ktil")
            nc.vector.scalar_tensor_tensor(
                kt[:], ck_ps[:], W[f"ckb1{k}"], ek[:], ALU.add, ALU.max)
            kb_ps = ppool.tile([P, FD], F32, tag="ps")
            nc.tensor.matmul(kb_ps[:], W["P8"], kt[:])
            kb = spool.tile([P, FD], F16, tag="kbsb")
            if t % 2 == 0:
                nc.scalar.activation(kb[:], kb_ps[:], AF.Identity)
            else:
                nc.vector.tensor_copy(kb[:], kb_ps[:])
            nc.tensor.matmul(kc_ps[:], h64[:], kt[:],
                             start=(t == 0), stop=(t == NT - 1))
            cc[("b", t)] = kb

        def colA_c(t, k=k):
            kb = cc.pop(("b", t))
            h64 = W["H64_last"] if t == NT - 1 else W["H64"]
            cv_ps = ppool.tile([P, FD], F32, tag="ps")
            nc.tensor.matmul(cv_ps[:], W[f"cv{k}"], hp[t][:])
            vw = spool.tile([P, FD], F16, tag="vw")
            nc.vector.scalar_tensor_tensor(
                vw[:], cv_ps[:], W[f"cvb{k}"], kb[:], ALU.add, ALU.mult)
            nc.tensor.matmul(tv_ps[:], h64[:], vw[:],
                             start=(t == 0), stop=(t == NT - 1))

        for i in range(NT + 2):
            if i < NT:
                colA_a(i)
            if 1 <= i <= NT:
                colA_b(i - 1)
            if i >= 2:
                colA_c(i - 2)
        kcs_sb = gpool.tile([64, FD], F32, tag="ln_mu")
        tvs_sb = gpool.tile([64, FD], F32, tag="ln_e2")
        nc.vector.tensor_copy(kcs_sb[:], kc_ps[:])
        nc.vector.tensor_copy(tvs_sb[:], tv_ps[:])
        ksc = gpool.tile([64, LSH], F16, tag="ksc")
        tvc = gpool.tile([64, LSH], F16, tag="tvc")
        fo1 = gpool.tile([64, LSH], F16, tag="fold1")
        fo2 = gpool.tile([64, LSH], F16, tag="fold2")
        kq, tq = _q(kcs_sb[:]), _q(tvs_sb[:])
        nc.vector.tensor_add(fo1[:], kq[:, 0, :], kq[:, 1, :])
        nc.vector.tensor_add(ksc[:], kq[:, 2, :], kq[:, 3, :])
        nc.vector.tensor_add(ksc[:], fo1[:], ksc[:])
        nc.gpsimd.tensor_add(fo2[:], tq[:, 0, :], tq[:, 1, :])
        nc.gpsimd.tensor_add(tvc[:], tq[:, 2, :], tq[:, 3, :])
        nc.gpsimd.tensor_add(tvc[:], fo2[:], tvc[:])
        kcb_ps = ppool.tile([P, FD], F32, tag="ps")
        nc.tensor.matmul(kcb_ps[:, :LSH], W["H64T"], ksc[:])
        nc.vector.tensor_copy(kc_b[:], kcb_ps[:, :LSH])
        tcb_ps = ppool.tile([P, FD], F32, tag="ps")
        nc.tensor.matmul(tcb_ps[:, :LSH], W["H64T"], tvc[:])
        nc.vector.tensor_copy(tc_b[:], tcb_ps[:, :LSH])

        # ---- column attention phase B (3-stage skew producer) ------------
        def col_b_a(t, carry, k=k):
            q_ps = ppool.tile([P, FD], F32, tag="ps")
            nc.tensor.matmul(q_ps[:], W[f"cq{k}"], hp[t][:])
            mq = spool.tile([P, FD], F16, tag="mk")
            eq = spool.tile([P, FD], F16, tag="ek")
            qb1 = spool.tile([P, FD], F16, tag="kbsb")
            nc.scalar.activation(mq[:], q_ps[:], AF.Relu, scale=-1.0,
                                 bias=W[f"cqbn{k}"])
            nc.scalar.activation(eq[:], mq[:], AF.Exp, scale=-1.0)
            nc.scalar.activation(qb1[:], q_ps[:], AF.Identity,
                                 bias=W[f"cqb1{k}"])
            carry[t] = (eq, qb1)

        def col_b_b(t, carry, k=k):
            eq, qb1 = carry.pop(t)
            qt = spool.tile([P, FD], F16, tag="ktil")
            nc.vector.tensor_max(qt[:], qb1[:], eq[:])
            prod = spool.tile([P, FD], F16, tag="vw")
            nc.vector.tensor_tensor(_q(prod[:]), _q(qt[:]), _bl(kc_b[:]),
                                    ALU.mult)
            dn_ps = ppool.tile([P, FD], F32, tag="ps")
            nc.tensor.matmul(dn_ps[:], W["P8"], prod[:])
            carry[("b", t)] = dn_ps

        def col_b_c(t, carry, k=k):
            dn_ps = carry.pop(("b", t))
            z = spool.tile([P, FD], F16, tag="z")
            nc.vector.reciprocal(z[:], dn_ps[:])
            V = spool.tile([P, FD], F16, tag="V")
            nc.vector.tensor_tensor(_q(V[:]), _q(z[:]), _bl(tc_b[:]),
                                    ALU.mult)
            att_ps = ppool.tile([P, FD], F32, tag="ps")
            nc.tensor.matmul(att_ps[:], W[f"cpbrow{k}"], ones_row[:],
                             start=True, stop=False)
            nc.tensor.matmul(att_ps[:], W[f"cp{k}"], V[:],
                             start=False, stop=True)
            x_pre = xpool.tile([P, FD], F16, tag=f"xp{t % GSZ}")
            nc.vector.scalar_tensor_tensor(
                x_pre[:], hp[t][:], W[f"cgcol{k}"], att_ps[:],
                ALU.mult, ALU.add)
            return x_pre

        ln_phase([col_b_a, col_b_b, col_b_c], sq_dve=False, t1_dve=False)

        # ---- FFN ----------------------------------------------------------
        def ffn(t, carry=None, k=k, to_hp=False):
            o_ps = ppool.tile([P, FD], F32, tag="ps")
            nc.tensor.matmul(o_ps[:], W[f"f2brow{k}"], ones_row[:],
                             start=True, stop=False)
            for j in range(4):
                h_ps = ppool.tile([P, FD], F32, tag="ps")
                nc.tensor.matmul(h_ps[:], W[f"f1_{k}_{j}"], hp[t][:])
                hid = spool.tile([P, FD], F16, tag="V")
                nc.scalar.activation(hid[:], h_ps[:], AF.Gelu,
                                     bias=W[f"f1b_{k}_{j}"])
                nc.tensor.matmul(o_ps[:], W[f"f2_{k}_{j}"], hid[:],
                                 start=False, stop=(j == 3))
            if to_hp:
                nc.vector.scalar_tensor_tensor(
                    hp[t][:], hp[t][:], W[f"fgcol{k}"], o_ps[:],
                    ALU.mult, ALU.add)
                return None
            x_pre = xpool.tile([P, FD], F16, tag=f"xp{t % GSZ}")
            nc.vector.scalar_tensor_tensor(
                x_pre[:], hp[t][:], W[f"fgcol{k}"], o_ps[:],
                ALU.mult, ALU.add)
            return x_pre

        if k != N_BLOCKS - 1:
            ln_phase(ffn, sq_dve=True, t1_dve=True)
        else:
            for t in range(NT):
                ffn(t, to_hp=True)

    # ============================================================ output
    ystage = wpool.tile([32, 4 * NOG], F32, tag="ystage")
    for gi in range(NOG):
        t0, tend = gi * OGSZ, min(NT, gi * OGSZ + OGSZ)
        ntl = tend - t0
        o_ps = ppers.tile([64, FD], F32, tag="acc1")
        for tau in range(ntl):
            nc.tensor.matmul(o_ps[:32, :],
                             W["outw_lt"][:, tau * 32:(tau + 1) * 32],
                             hp[t0 + tau][:],
                             start=(tau == 0), stop=(tau == ntl - 1))
        ab = gpool.tile([64, FD], F32, tag="ln_mu")
        l1 = gpool.tile([64, FD], F32, tag="ln_e2")
        rl = gpool.tile([64, FD], F32, tag="ln_msq")
        nc.scalar.activation(ab[:32, :], o_ps[:32, :], AF.Abs, bias=W["boutc"])
        nc.scalar.activation(ab[:32, :], ab[:32, :], AF.Exp, scale=-1.0)
        nc.scalar.activation(l1[:32, :], ab[:32, :], AF.Ln, bias=W["onec"])
        nc.scalar.activation(rl[:32, :], o_ps[:32, :], AF.Relu, bias=W["boutc"])
        nc.vector.tensor_add(l1[:32, :], l1[:32, :], rl[:32, :])
        nc.vector.tensor_reduce(
            ystage[:, 4 * gi:4 * gi + 4],
            l1[:32, :].rearrange("p (q l) -> p q l", q=NQ),
            mybir.AxisListType.X, ALU.add)
    nc.sync.dma_start(yout_d[:], ystage[:])
    ctx.close()


# ================================================================ host API
_NC_CACHE = {}


def _get_nc():
    if "nc" not in _NC_CACHE:
        _NC_CACHE["nc"] = build_kernel()
    return _NC_CACHE["nc"]


def kernel(**inputs):
    from concourse.bass_utils import run_bass_kernel_spmd

    nc = _get_nc()
    w = prep_weights(inputs)

    x = np.asarray(inputs["x"])
    in_maps = []
    for core in range(N_CORES):
        b, lq = core // 4, core % 4
        xs = x[b, :, lq * LSH:(lq + 1) * LSH, :]
        xs = np.ascontiguousarray(np.transpose(xs, (0, 2, 1)),
                                  dtype=np.float16)
        m = {"xin": xs, "wpack16": w["wpack16"], "wpack32": w["wpack32"]}
        in_maps.append(m)

    res = run_bass_kernel_spmd(nc, in_maps, core_ids=list(range(N_CORES)))
    outs = [r["yout"] for r in res.results]

    y = np.zeros((B, NB_PAIRS), np.float64)
    for core in range(N_CORES):
        b = core // 4
        st = outs[core].astype(np.float64)
        for gi in range(NOG):
            for tau in range(min(OGSZ, NT - gi * OGSZ)):
                t = gi * OGSZ + tau
                for g in range(2):
                    for q in range(NQ):
                        s = 8 * t + 4 * g + q
                        if s < NB_PAIRS:
                            y[b, s] += st[2 * tau + g, 4 * gi + q]
    y /= SEQ_LEN

    out = np.zeros((B, NB_PAIRS), np.float32)
    ii, jj = np.triu_indices(NB_SEQ, 1)
    tri = {(a, c): p for p, (a, c) in enumerate(zip(ii, jj))}
    for s, (a, c) in enumerate(PAIRS):
        out[:, tri[(a, c)]] = y[:, s]
    return out


# revision 23
# speedup vs baseline: 1.0089x; 1.0089x over previous
"""Trainium2 Bass kernel for nn_AttentionNet (axial linear-attention net).

Sharding: cores 0-3 hold batch b=0, cores 4-7 hold b=1. Within a 4-core
group the sequence axis L=512 is split into 4 shards of 128. Every core
holds ALL 780 pairs for its (b, l-shard), so the instruction stream is
identical on all cores (pure SPMD) and only the input data differs.

Residual state per core: 98 SBUF tiles [128, 512] fp16:
  partition = g*64 + n*16 + d   (g = pair-half 0/1, n = head, d = head ch)
  free      = q*128 + l         (q = pair-quad 0..3, l = local seq pos)
  tile t holds pair slots 8t + 4g + q (784 slots = 780 pairs + 4 pads).

v2 design notes:
- LayerNorm affine (gamma/beta) is folded host-side into every consumer
  matmul; the residual stream stores the UN-affined normalized value
  (h-tilde) and residual adds re-apply gamma via the stt scalar slot and
  beta via a 1-partition bias matmul accumulated into the attention/FFN
  output PSUM.
- LN apply uses a DMA partition-broadcast of [rstd | mean*rstd] rows to
  128 partitions, then two 2x-mode f16 TensorTensor ops. No per-tile
  apply matmuls, no PSUM reads on the apply path.
- LN statistics are computed in groups of 32 tiles (one-hot stat matmuls
  into two persistent PSUM banks).
- Row attention uses affine_mul_reduce to fuse (v+bias)*kbar with the
  per-quad KtV reduction; PSUM->SBUF drains ride the Activation engine.
- Engine balance: DVE keeps the PSUM-coupled ops, Act does elu/gelu
  chains + drains, Pool (gpsimd) takes pure-SBUF squares/multiplies,
  the DMA engines do the LN broadcasts.
"""

import contextlib
import sys

import numpy as np

sys.path.insert(0, "/opt/trn_rl_repo")

mybir = None
F32 = F16 = AF = ALU = None


def _lazy_imports():
    global mybir, F32, F16, AF, ALU
    if mybir is None:
        import concourse.mybir as _mybir
        mybir = _mybir
        F32, F16 = mybir.dt.float32, mybir.dt.float16
        AF = mybir.ActivationFunctionType
        ALU = mybir.AluOpType

NB_SEQ = 40
SEQ_LEN = 512
NB_PAIRS = 780
B = 2
N_BLOCKS = 2
CIN = 22

N_CORES = 8
LSH = 128            # l per core
NQ = 4               # quads per tile
NT = 98              # hp tiles per core
GSZ = 32             # LN group size (tiles)
NGROUP = (NT + GSZ - 1) // GSZ       # 4 (32,32,32,2)
OGSZ = 16
NOG = (NT + OGSZ - 1) // OGSZ        # 7, output stage groups
FD = NQ * LSH        # 512, tile free size


def _pair_order():
    order = []
    for d in range(1, NB_SEQ):
        for i in range(NB_SEQ - d):
            order.append((i, i + d))
    return order


PAIRS = _pair_order()


def slot_ij(s):
    return PAIRS[s] if s < NB_PAIRS else PAIRS[0]


# ================================================================ weights
def prep_weights(inp):
    """Pack all constants; LN affine folded into consumer weights."""
    w = {}
    f16 = lambda a: np.ascontiguousarray(a, dtype=np.float16)
    f32 = lambda a: np.ascontiguousarray(a, dtype=np.float32)

    def col(v, n=128):
        v = np.asarray(v, np.float32).reshape(-1)
        if v.size == 64:
            v = np.tile(v, 2)
        if v.size == 1:
            v = np.full(n, v[0], np.float32)
        return f32(v.reshape(n, 1))

    w_in = np.asarray(inp["w_in"])
    w["wconv"] = f16(np.concatenate([w_in.T, w_in.T], axis=1))
    w["bconv"] = col(inp["b_in"])

    # LN instances: 0 = (g0, be0); k+1 = block-k (ln_g[k], ln_b[k]).
    lns = [(np.asarray(inp["g0"], np.float32),
            np.asarray(inp["be0"], np.float32))]
    for k in range(N_BLOCKS):
        lns.append((np.asarray(inp["ln_g"][k], np.float32),
                    np.asarray(inp["ln_b"][k], np.float32)))

    def bd(m, gamma):
        # block-diag lhsT with gamma folded into input columns
        mt = (np.asarray(m, np.float32) * gamma[None, :]).T
        z = np.zeros((128, 128), np.float16)
        z[:64, :64] = mt
        z[64:, 64:] = mt
        return z

    def fold_bias(m, beta, b):
        return np.asarray(m, np.float32) @ beta + np.asarray(b, np.float32)

    # LN instance feeding each phase's input:
    #  rowA/rowB(k): instance k (ln0 for k=0, ffn-LN of k-1 else)
    #  colA/colB(k): instance k+1 (row-LN of block k)
    #  ffn(k):       instance k+1 (col-LN of block k)
    for k in range(N_BLOCKS):
        gr, br_ = lns[k]        # feeds row attention
        gc, bc_ = lns[k + 1]    # feeds col attention and ffn
        for nm, wk, bk, g_, b_ in [
                ("rq", "rqw", "rqb", gr, br_), ("rk", "rkw", "rkb", gr, br_),
                ("rv", "rvw", "rvb", gr, br_), ("cq", "cqw", "cqb", gc, bc_),
                ("ck", "ckw", "ckb", gc, bc_), ("cv", "cvw", "cvb", gc, bc_)]:
            m = np.asarray(inp[wk][k])
            w[f"{nm}{k}"] = f16(bd(m, g_))
            bb = fold_bias(m, b_, inp[bk][k])
            w[f"{nm}b{k}"] = col(bb)
            w[f"{nm}b1{k}"] = col(bb + 1.0)
            w[f"{nm}bn{k}"] = col(-bb)
        # output projections: plain weights; bias handled by bias-row MM
        for nm, wk in [("rp", "rpw"), ("cp", "cpw")]:
            m = np.asarray(inp[wk][k])
            z = np.zeros((128, 128), np.float16)
            z[:64, :64] = m.T
            z[64:, 64:] = m.T
            w[f"{nm}{k}"] = f16(z)
        # residual bias rows: rp-bias + beta of the residual LN instance
        rpb = np.asarray(inp["rpb"][k], np.float32)
        cpb = np.asarray(inp["cpb"][k], np.float32)
        w[f"rpbrow{k}"] = f16(np.tile(rpb + br_, 2).reshape(1, 128))
        w[f"cpbrow{k}"] = f16(np.tile(cpb + bc_, 2).reshape(1, 128))
        # residual gamma cols
        w[f"rgcol{k}"] = col(gr)
        w[f"cgcol{k}"] = col(gc)
        w[f"fgcol{k}"] = col(gc)

        f1w = np.asarray(inp["f1w"][k], np.float32) * gc[None, :]
        f1b = np.asarray(inp["f1w"][k], np.float32) @ bc_ \
            + np.asarray(inp["f1b"][k], np.float32)
        f2w = np.asarray(inp["f2w"][k])
        for j in range(4):
            g, hh = j // 2, (j % 2) * 128
            lt = np.zeros((128, 128), np.float16)
            lt[g * 64:(g + 1) * 64, :] = f1w[hh:hh + 128, :].T
            w[f"f1_{k}_{j}"] = f16(lt)
            lt2 = np.zeros((128, 128), np.float16)
            lt2[:, g * 64:(g + 1) * 64] = f2w[:, hh:hh + 128].T
            w[f"f2_{k}_{j}"] = f16(lt2)
            w[f"f1b_{k}_{j}"] = f32(f1b[hh:hh + 128].reshape(128, 1))
        f2b = np.asarray(inp["f2b"][k], np.float32)
        w[f"f2brow{k}"] = f16(np.tile(f2b + bc_, 2).reshape(1, 128))
        # last block's ffn has no LN after; residual gamma is col instance
        # (gc) because hp there is the col-LN output of block k.

    # conv + gather feed ln0; gather output is raw (pre-LN) so the rowA
    # weights above already fold ln0 -> nothing extra here.

    # stat slabs for GSZ-tile groups: per tau [128, 64]:
    # rows 2*tau+g get ones at partitions g*64..g*64+64
    stat = np.zeros((128, GSZ * 64), np.float16)
    for tau in range(GSZ):
        for g in range(2):
            stat[g * 64:(g + 1) * 64, tau * 64 + 2 * tau + g] = 1.0
    w["stat_lt"] = f16(stat)

    # output stage (16-tile groups)
    outw = np.zeros((128, OGSZ * 32), np.float16)
    wo = np.asarray(inp["wout"], np.float32).reshape(-1)
    for tau in range(OGSZ):
        for g in range(2):
            outw[g * 64:(g + 1) * 64, tau * 32 + 2 * tau + g] = wo
    w["outw_lt"] = f16(outw)
    w["boutc"] = f32(np.full((32, 1), np.asarray(inp["bout"]).reshape(-1)[0],
                             np.float32))
    w["epsc"] = f32(np.full((64, 1), 1e-5, np.float32))
    w["onec"] = f32(np.full((32, 1), 1.0, np.float32))

    p8 = np.zeros((128, 128), np.float16)
    for blk in range(8):
        p8[blk * 16:(blk + 1) * 16, blk * 16:(blk + 1) * 16] = 1.0
    w["P8"] = f16(p8)
    h64 = np.zeros((128, 64), np.float16)
    h64[np.arange(128), np.arange(128) % 64] = 1.0
    w["H64"] = f16(h64)
    hlast = h64.copy()
    hlast[64:, :] = 0.0
    w["H64_last"] = f16(hlast)
    h64t = np.zeros((64, 128), np.float16)
    h64t[np.arange(128) % 64, np.arange(128)] = 1.0
    w["H64T"] = f16(h64t)

    # ---- pack into two tensors ----
    s16, s32 = _pack_layout()
    p16 = np.zeros((128, s16[-1][2] + s16[-1][3]), np.float16)
    for name, rows, off, cols in s16:
        p16[:rows, off:off + cols] = w[name]
    p32 = np.zeros((128, s32[-1][2] + s32[-1][3]), np.float32)
    for name, rows, off, cols in s32:
        p32[:rows, off:off + cols] = w[name]
    w["wpack16"] = p16
    w["wpack32"] = p32
    return w


def _pack_layout():
    e16, e32 = [], []
    o16 = o32 = 0

    def a16(name, rows, cols):
        nonlocal o16
        e16.append((name, rows, o16, cols))
        o16 += cols

    def a32(name, rows, cols):
        nonlocal o32
        e32.append((name, rows, o32, cols))
        o32 += cols

    a16("wconv", CIN, 128)
    a32("bconv", 128, 1)
    for k in range(N_BLOCKS):
        for nm in ["rq", "rk", "rv", "cq", "ck", "cv"]:
            a16(f"{nm}{k}", 128, 128)
            a32(f"{nm}b{k}", 128, 1)
            a32(f"{nm}b1{k}", 128, 1)
            a32(f"{nm}bn{k}", 128, 1)
        for nm in ["rp", "cp"]:
            a16(f"{nm}{k}", 128, 128)
        a16(f"rpbrow{k}", 1, 128)
        a16(f"cpbrow{k}", 1, 128)
        a32(f"rgcol{k}", 128, 1)
        a32(f"cgcol{k}", 128, 1)
        a32(f"fgcol{k}", 128, 1)
        for j in range(4):
            a16(f"f1_{k}_{j}", 128, 128)
            a16(f"f2_{k}_{j}", 128, 128)
            a32(f"f1b_{k}_{j}", 128, 1)
        a16(f"f2brow{k}", 1, 128)
    a16("stat_lt", 128, GSZ * 64)
    a16("outw_lt", 128, OGSZ * 32)
    a32("boutc", 32, 1)
    a32("epsc", 64, 1)
    a32("onec", 32, 1)
    a16("P8", 128, 128)
    a16("H64", 128, 64)
    a16("H64_last", 128, 64)
    a16("H64T", 64, 128)
    return e16, e32


WEIGHT_SPECS = []


def _spec():
    e16, e32 = _pack_layout()
    n16 = e16[-1][2] + e16[-1][3]
    n32 = e32[-1][2] + e32[-1][3]
    return [("wpack16", (128, n16), F16), ("wpack32", (128, n32), F32)]


# ================================================================ views
def _q(ap):
    return ap.rearrange("p (q l) -> p q l", q=NQ)


def _bq(ap_col4):
    """[128, 4] slice -> [128, 4, 128] broadcast over l."""
    a = ap_col4.copy()
    a.ap = a.ap[:-1] + [list(a.ap[-1]), [0, LSH]]
    return a


def _bl(ap_l):
    """[128, 128] -> [128, 4, 128] broadcast over quads."""
    a = ap_l.copy()
    a.ap = a.ap[:-1] + [[0, NQ], list(a.ap[-1])]
    return a


def _brows(ap_2rows):
    """[2, F] rows -> broadcast to [2, 64, F] (DMA source: each row
    repeated 64x so the dest covers 128 partitions)."""
    a = ap_2rows.copy()
    a.ap = a.ap[:1] + [[0, 64], list(a.ap[-1])]
    return a


# ================================================================ kernel IR
def build_kernel():
    _lazy_imports()
    import concourse.bacc as bacc
    import concourse.tile as tile

    global WEIGHT_SPECS
    WEIGHT_SPECS = _spec()

    nc = bacc.Bacc("TRN2", target_bir_lowering=False, debug=False,
                   num_devices=N_CORES)
    xin_d = nc.dram_tensor("xin", [CIN, NB_SEQ, LSH], F16,
                           kind="ExternalInput").ap()
    wd = {}
    for name, shape, dtype in WEIGHT_SPECS:
        wd[name] = nc.dram_tensor(name, list(shape), dtype,
                                  kind="ExternalInput").ap()
    yout_d = nc.dram_tensor("yout", [32, 4 * NOG], F32,
                            kind="ExternalOutput").ap()

    with tile.TileContext(nc) as tc:
        _body(nc, tc, xin_d, wd, yout_d)
    nc.compile()
    return nc


def _body(nc, tc, xin_d, wd, yout_d):
    ctx = contextlib.ExitStack()
    ctx.enter_context(nc.allow_low_precision(
        reason="normalized activations; f16 everywhere is plenty for 2e-2"))
    P = 128

    wpool = ctx.enter_context(tc.tile_pool(name="w", bufs=1))
    hpool = ctx.enter_context(tc.tile_pool(name="hp", bufs=1))
    spool = ctx.enter_context(tc.tile_pool(name="scr", bufs=3))
    xpool = ctx.enter_context(tc.tile_pool(name="xpre", bufs=1))
    gpool = ctx.enter_context(tc.tile_pool(name="grp", bufs=1))
    stpool = ctx.enter_context(tc.tile_pool(name="st2", bufs=2))
    bpool = ctx.enter_context(tc.tile_pool(name="bc", bufs=3))
    ppool = ctx.enter_context(tc.tile_pool(name="ps", bufs=6, space="PSUM"))
    ppers = ctx.enter_context(tc.tile_pool(name="ps2", bufs=1, space="PSUM"))
    dpool = ctx.enter_context(tc.tile_pool(name="dram", bufs=1, space="DRAM"))

    e16, e32 = _pack_layout()
    n16 = e16[-1][2] + e16[-1][3]
    n32 = e32[-1][2] + e32[-1][3]
    pk16 = wpool.tile([128, n16], F16, tag="pk16", name="pk16")
    pk32 = wpool.tile([128, n32], F32, tag="pk32", name="pk32")
    nc.sync.dma_start(pk16[:], wd["wpack16"][:])
    nc.sync.dma_start(pk32[:], wd["wpack32"][:])
    W = {}
    for name, rows, off, cols in e16:
        W[name] = pk16[:rows, off:off + cols]
    for name, rows, off, cols in e32:
        W[name] = pk32[:rows, off:off + cols]

    hp = [hpool.tile([P, FD], F16, tag=f"hp{t}", name=f"hp{t}")
          for t in range(NT)]

    ksum_pr = wpool.tile([P, NQ * NT], F32, tag="ksum_pr")
    ktv_pr = wpool.tile([P, NQ * NT], F32, tag="ktv_pr")
    ksum_h = wpool.tile([P, NQ * NT], F16, tag="ksum_h")
    ktv_h = wpool.tile([P, NQ * NT], F16, tag="ktv_h")
    kc_b = wpool.tile([P, LSH], F16, tag="kc_b")
    tc_b = wpool.tile([P, LSH], F16, tag="tc_b")
    ones_row = wpool.tile([1, FD], F16, tag="ones_row")
    nc.vector.memset(ones_row[:], 1.0)

    # ============================================================ LN
    # producer(t) -> x_pre tile (REAL post-residual values).
    # sq_dve: engine for the square (True=DVE, False=Pool)
    # t1_dve: engine for the first apply multiply
    def ln_phase(producer, sq_dve=False, t1_dve=False):
        stages = producer if isinstance(producer, (list, tuple)) \
            else [producer]
        ns = len(stages)
        pending = []
        for gi in range(NGROUP):
            t0, tend = gi * GSZ, min(NT, gi * GSZ + GSZ)
            ntl = tend - t0
            s_ps = ppers.tile([64, FD], F32, tag="acc1")
            sq_ps = ppers.tile([64, FD], F32, tag="acc2")
            xs = [None] * ntl

            def finish(tau, x_pre, ntl=ntl, s_ps=s_ps, sq_ps=sq_ps, xs=xs,
                       t0=t0):
                xs[tau] = x_pre
                sq = spool.tile([P, FD], F16, tag="sq")
                use_dve = sq_dve if isinstance(sq_dve, bool) \
                    else (tau % 3 == 0)
                if use_dve:
                    nc.vector.tensor_mul(sq[:], x_pre[:], x_pre[:])
                else:
                    nc.gpsimd.tensor_mul(sq[:], x_pre[:], x_pre[:])
                sl = W["stat_lt"][:, tau * 64:tau * 64 + 64]
                nc.tensor.matmul(s_ps[:], sl, x_pre[:],
                                 start=(tau == 0), stop=(tau == ntl - 1))
                nc.tensor.matmul(sq_ps[:], sl, sq[:],
                                 start=(tau == 0), stop=(tau == ntl - 1))

            carry = {}
            for i in range(ntl + ns - 1):
                for si, f in enumerate(stages):
                    tau = i - si
                    if 0 <= tau < ntl:
                        r = f(t0 + tau, carry)
                        if si == ns - 1:
                            finish(tau, r)
                if pending:
                    pending.pop(0)()
                if pending:
                    pending.pop(0)()
            mu = gpool.tile([64, FD], F32, tag="ln_mu")
            e2 = gpool.tile([64, FD], F32, tag="ln_e2")
            msq = gpool.tile([64, FD], F32, tag="ln_msq")
            st2 = stpool.tile([64, 2 * FD], F16, tag="ln_st2")
            nc.scalar.activation(mu[:], s_ps[:], AF.Copy, scale=1.0 / 64)
            nc.scalar.activation(e2[:], sq_ps[:], AF.Copy, scale=1.0 / 64)
            nc.scalar.activation(msq[:], mu[:], AF.Square)
            nc.vector.tensor_sub(e2[:], e2[:], msq[:])
            nc.scalar.activation(msq[:], e2[:], AF.Ln, bias=W["epsc"])
            nc.scalar.activation(st2[:, :FD], msq[:], AF.Exp, scale=-0.5)
            nc.vector.tensor_mul(st2[:, FD:], mu[:], st2[:, :FD])
            while pending:
                pending.pop(0)()

            bds = {}

            def mk_dma(tau, st2=st2, bds=bds):
                def go():
                    bdst = bpool.tile([P, 2 * FD], F16, tag="bdst")
                    nc.sync.dma_start(bdst[:],
                                      _brows(st2[2 * tau:2 * tau + 2, :]))
                    bds[tau] = bdst
                return go

            def mk_cmp(tau, t0=t0, xs=xs, bds=bds):
                def go():
                    t = t0 + tau
                    bdst = bds.pop(tau)
                    t1 = spool.tile([P, FD], F16, tag="t1")
                    use_dve = t1_dve if isinstance(t1_dve, bool) \
                        else (tau % 2 == 0)
                    if use_dve:
                        nc.vector.tensor_mul(t1[:], xs[tau][:], bdst[:, :FD])
                    else:
                        nc.gpsimd.tensor_mul(t1[:], xs[tau][:], bdst[:, :FD])
                    nc.vector.tensor_sub(hp[t][:], t1[:], bdst[:, FD:])
                return go

            # DMA for apply j runs 2 queue slots ahead of its compute
            q = []
            for tau in range(ntl):
                q.append(mk_dma(tau))
            for tau in range(ntl):
                q.insert(2 * tau + 2 if 2 * tau + 2 < len(q) else len(q),
                         mk_cmp(tau))
            # interleave: after position-building above, q has dma j at
            # slot ~2j and cmp j at slot ~2j+2
            pending.extend(q)
        while pending:
            pending.pop(0)()

    # ============================================================ Phase 0
    h2 = wpool.tile([P, NB_SEQ * LSH], F16, tag="h2")
    xin_f = xin_d.rearrange("c s l -> c (s l)")
    for j in range(10):
        xst = spool.tile([CIN, FD], F16, tag="sq")
        nc.sync.dma_start(xst[:], xin_f[:, j * FD:(j + 1) * FD])
        cps = ppool.tile([P, FD], F32, tag="ps")
        nc.tensor.matmul(cps[:], W["wconv"], xst[:])
        nc.scalar.activation(h2[:, j * FD:(j + 1) * FD], cps[:],
                             AF.Relu, bias=W["bconv"])
    h2q = h2[:].rearrange("p (s l) -> p s l", s=NB_SEQ)

    def gather_producer(t, carry=None):
        x_pre = xpool.tile([P, FD], F16, tag=f"xp{t % GSZ}")
        xq = _q(x_pre[:])
        eng = nc.vector
        for g in range(2):
            ij = [slot_ij(8 * t + 4 * g + q) for q in range(NQ)]
            iis = [a for a, _ in ij]
            jjs = [b for _, b in ij]
            rows = slice(g * 64, g * 64 + 64)
            if (all(iis[q] == iis[0] + q for q in range(NQ)) and
                    all(jjs[q] == jjs[0] + q for q in range(NQ))):
                eng.tensor_add(xq[rows, :, :],
                               h2q[rows, iis[0]:iis[0] + NQ, :],
                               h2q[rows, jjs[0]:jjs[0] + NQ, :])
            else:
                for q in range(NQ):
                    eng.tensor_add(xq[rows, q, :],
                                   h2q[rows, iis[q], :],
                                   h2q[rows, jjs[q], :])
        return x_pre

    ln_phase(gather_producer, sq_dve="mix", t1_dve="mix")

    # ============================================================ blocks
    for k in range(N_BLOCKS):

        # ---- row attention phase A: k/v, local partials (3-stage skew) ---
        rc = {}

        def rowA_a(t, k=k):
            k_ps = ppool.tile([P, FD], F32, tag="ps")
            nc.tensor.matmul(k_ps[:], W[f"rk{k}"], hp[t][:])
            mk = spool.tile([P, FD], F16, tag="mk")
            ek = spool.tile([P, FD], F16, tag="ek")
            nc.scalar.activation(mk[:], k_ps[:], AF.Relu, scale=-1.0,
                                 bias=W[f"rkbn{k}"])
            nc.scalar.activation(ek[:], mk[:], AF.Exp, scale=-1.0)
            rc[t] = (k_ps, ek)

        def rowA_b(t, k=k):
            k_ps, ek = rc.pop(t)
            kt = spool.tile([P, FD], F16, tag="ktil")
            nc.vector.scalar_tensor_tensor(
                kt[:], k_ps[:], W[f"rkb1{k}"], ek[:], ALU.add, ALU.max)
            nc.vector.tensor_reduce(ksum_pr[:, NQ * t:NQ * t + NQ],
                                    _q(kt[:]), mybir.AxisListType.X, ALU.add)
            kb_ps = ppool.tile([P, FD], F32, tag="ps")
            nc.tensor.matmul(kb_ps[:], W["P8"], kt[:])
            kb = spool.tile([P, FD], F16, tag="kbsb")
            nc.scalar.activation(kb[:], kb_ps[:], AF.Identity)
            v_ps = ppool.tile([P, FD], F32, tag="ps")
            nc.tensor.matmul(v_ps[:], W[f"rv{k}"], hp[t][:])
            rc[("b", t)] = (kb, v_ps)

        def rowA_c(t, k=k):
            kb, v_ps = rc.pop(("b", t))
            vw = spool.tile([P, FD], F16, tag="vw")
            for q in range(NQ):
                sl = slice(q * LSH, (q + 1) * LSH)
                nc.vector.affine_mul_reduce(
                    vw[:, sl], ktv_pr[:, NQ * t + q:NQ * t + q + 1],
                    v_ps[:, sl], kb[:, sl], 1.0, W[f"rvb{k}"])

        # AllReduce in two halves: first half overlaps rowA's tail.
        TSPLIT = 64

        def ar(lo, hi, half, k=k):
            c0, c1 = NQ * lo, NQ * hi
            n = c1 - c0
            bin_ = dpool.tile([P, 2 * n], F32, tag=f"arin{k}_{half}")
            bout_ = dpool.tile([P, 2 * n], F32, tag=f"arout{k}_{half}")
            nc.sync.dma_start(bin_[:, :n], ksum_pr[:, c0:c1])
            nc.sync.dma_start(bin_[:, n:], ktv_pr[:, c0:c1])
            nc.gpsimd.collective_compute(
                "AllReduce", ALU.add,
                replica_groups=[[0, 1, 2, 3], [4, 5, 6, 7]],
                ins=[bin_.opt()], outs=[bout_.opt()])
            nc.sync.dma_start(ksum_pr[:, c0:c1], bout_[:, :n])
            nc.sync.dma_start(ktv_pr[:, c0:c1], bout_[:, n:])
            nc.vector.tensor_copy(ksum_h[:, c0:c1], ksum_pr[:, c0:c1])
            nc.vector.tensor_copy(ktv_h[:, c0:c1], ktv_pr[:, c0:c1])

        for i in range(NT + 2):
            if i < NT:
                rowA_a(i)
            if 1 <= i <= NT:
                rowA_b(i - 1)
            if i >= 2:
                rowA_c(i - 2)
            if i == TSPLIT + 2:
                ar(0, TSPLIT, 0)
        ar(TSPLIT, NT, 1)

        # ---- row attention phase B (3-stage skew producer) ---------------
        def row_b_a(t, carry, k=k):
            q_ps = ppool.tile([P, FD], F32, tag="ps")
            nc.tensor.matmul(q_ps[:], W[f"rq{k}"], hp[t][:])
            mq = spool.tile([P, FD], F16, tag="mk")
            eq = spool.tile([P, FD], F16, tag="ek")
            qb1 = spool.tile([P, FD], F16, tag="kbsb")
            nc.scalar.activation(mq[:], q_ps[:], AF.Relu, scale=-1.0,
                                 bias=W[f"rqbn{k}"])
            nc.scalar.activation(eq[:], mq[:], AF.Exp, scale=-1.0)
            nc.scalar.activation(qb1[:], q_ps[:], AF.Identity,
                                 bias=W[f"rqb1{k}"])
            carry[t] = (eq, qb1)

        def row_b_b(t, carry, k=k):
            eq, qb1 = carry.pop(t)
            qt = spool.tile([P, FD], F16, tag="ktil")
            nc.vector.tensor_max(qt[:], qb1[:], eq[:])
            prod = spool.tile([P, FD], F16, tag="vw")
            nc.vector.tensor_tensor(_q(prod[:]), _q(qt[:]),
                                    _bq(ksum_h[:, NQ * t:NQ * t + NQ]),
                                    ALU.mult)
            dn_ps = ppool.tile([P, FD], F32, tag="ps")
            nc.tensor.matmul(dn_ps[:], W["P8"], prod[:])
            carry[("b", t)] = dn_ps

        def row_b_c(t, carry, k=k):
            dn_ps = carry.pop(("b", t))
            z = spool.tile([P, FD], F16, tag="z")
            nc.vector.reciprocal(z[:], dn_ps[:])
            V = spool.tile([P, FD], F16, tag="V")
            nc.vector.tensor_tensor(_q(V[:]), _q(z[:]),
                                    _bq(ktv_h[:, NQ * t:NQ * t + NQ]),
                                    ALU.mult)
            att_ps = ppool.tile([P, FD], F32, tag="ps")
            nc.tensor.matmul(att_ps[:], W[f"rpbrow{k}"], ones_row[:],
                             start=True, stop=False)
            nc.tensor.matmul(att_ps[:], W[f"rp{k}"], V[:],
                             start=False, stop=True)
            carry[("c", t)] = att_ps

        def row_b_d(t, carry, k=k):
            att_ps = carry.pop(("c", t))
            x_pre = xpool.tile([P, FD], F16, tag=f"xp{t % GSZ}")
            nc.vector.scalar_tensor_tensor(
                x_pre[:], hp[t][:], W[f"rgcol{k}"], att_ps[:],
                ALU.mult, ALU.add)
            return x_pre

        ln_phase([row_b_a, row_b_b, row_b_c, row_b_d],
                 sq_dve="mix", t1_dve="mix")

        # ---- column attention phase A: k/v + local pair reduction --------
        kc_ps = ppers.tile([64, FD], F32, tag="acc1")
        tv_ps = ppers.tile([64, FD], F32, tag="acc2")
        cc = {}

        def colA_a(t, k=k):
            ck_ps = ppool.tile([P, FD], F32, tag="ps")
            nc.tensor.matmul(ck_ps[:], W[f"ck{k}"], hp[t][:])
            mk = spool.tile([P, FD], F16, tag="mk")
            ek = spool.tile([P, FD], F16, tag="ek")
            nc.scalar.activation(mk[:], ck_ps[:], AF.Relu, scale=-1.0,
                                 bias=W[f"ckbn{k}"])
            nc.scalar.activation(ek[:], mk[:], AF.Exp, scale=-1.0)
            cc[t] = (ck_ps, ek)

        def colA_b(t, k=k):
            ck_ps, ek = cc.pop(t)
            h64 = W["H64_last"] if t == NT - 1 else W["H64"]
            kt = spool.tile([P, FD], F16, tag="ktil")
            nc.vector.scalar_tensor_tensor(
                kt[:], ck_ps[:], W[f"ckb1{k}"], ek[:], ALU.add, ALU.max)
            kb_ps = ppool.tile([P, FD], F32, tag="ps")
            nc.tensor.matmul(kb_ps[:], W["P8"], kt[:])
            kb = spool.tile([P, FD], F16, tag="kbsb")
            if t % 2 == 0:
                nc.scalar.activation(kb[:], kb_ps[:], AF.Identity)
            else:
                nc.vector.tensor_copy(kb[:], kb_ps[:])
            nc.tensor.matmul(kc_ps[:], h64[:], kt[:],
                             start=(t == 0), stop=(t == NT - 1))
            cc[("b", t)] = kb

        def colA_c(t, k=k):
            kb = cc.pop(("b", t))
            h64 = W["H64_last"] if t == NT - 1 else W["H64"]
            cv_ps = ppool.tile([P, FD], F32, tag="ps")
            nc.tensor.matmul(cv_ps[:], W[f"cv{k}"], hp[t][:])
            vw = spool.tile([P, FD], F16, tag="vw")
            nc.vector.scalar_tensor_tensor(
                vw[:], cv_ps[:], W[f"cvb{k}"], kb[:], ALU.add, ALU.mult)
            nc.tensor.matmul(tv_ps[:], h64[:], vw[:],
                             start=(t == 0), stop=(t == NT - 1))

        for i in range(NT + 2):
            if i < NT:
                colA_a(i)
            if 1 <= i <= NT:
                colA_b(i - 1)
            if i >= 2:
                colA_c(i - 2)
        kcs_sb = gpool.tile([64, FD], F32, tag="ln_mu")
        tvs_sb = gpool.tile([64, FD], F32, tag="ln_e2")
        nc.vector.tensor_copy(kcs_sb[:], kc_ps[:])
        nc.vector.tensor_copy(tvs_sb[:], tv_ps[:])
        ksc = gpool.tile([64, LSH], F16, tag="ksc")
        tvc = gpool.tile([64, LSH], F16, tag="tvc")
        fo1 = gpool.tile([64, LSH], F16, tag="fold1")
        fo2 = gpool.tile([64, LSH], F16, tag="fold2")
        kq, tq = _q(kcs_sb[:]), _q(tvs_sb[:])
        nc.vector.tensor_add(fo1[:], kq[:, 0, :], kq[:, 1, :])
        nc.vector.tensor_add(ksc[:], kq[:, 2, :], kq[:, 3, :])
        nc.vector.tensor_add(ksc[:], fo1[:], ksc[:])
        nc.gpsimd.tensor_add(fo2[:], tq[:, 0, :], tq[:, 1, :])
        nc.gpsimd.tensor_add(tvc[:], tq[:, 2, :], tq[:, 3, :])
        nc.gpsimd.tensor_add(tvc[:], fo2[:], tvc[:])
        kcb_ps = ppool.tile([P, FD], F32, tag="ps")
        nc.tensor.matmul(kcb_ps[:, :LSH], W["H64T"], ksc[:])
        nc.vector.tensor_copy(kc_b[:], kcb_ps[:, :LSH])
        tcb_ps = ppool.tile([P, FD], F32, tag="ps")
        nc.tensor.matmul(tcb_ps[:, :LSH], W["H64T"], tvc[:])
        nc.vector.tensor_copy(tc_b[:], tcb_ps[:, :LSH])

        # ---- column attention phase B (3-stage skew producer) ------------
        def col_b_a(t, carry, k=k):
            q_ps = ppool.tile([P, FD], F32, tag="ps")
            nc.tensor.matmul(q_ps[:], W[f"cq{k}"], hp[t][:])
            mq = spool.tile([P, FD], F16, tag="mk")
            eq = spool.tile([P, FD], F16, tag="ek")
            qb1 = spool.tile([P, FD], F16, tag="kbsb")
            nc.scalar.activation(mq[:], q_ps[:], AF.Relu, scale=-1.0,
                                 bias=W[f"cqbn{k}"])
            nc.scalar.activation(eq[:], mq[:], AF.Exp, scale=-1.0)
            nc.scalar.activation(qb1[:], q_ps[:], AF.Identity,
                                 bias=W[f"cqb1{k}"])
            carry[t] = (eq, qb1)

        def col_b_b(t, carry, k=k):
            eq, qb1 = carry.pop(t)
            qt = spool.tile([P, FD], F16, tag="ktil")
            nc.vector.tensor_max(qt[:], qb1[:], eq[:])
            prod = spool.tile([P, FD], F16, tag="vw")
            nc.vector.tensor_tensor(_q(prod[:]), _q(qt[:]), _bl(kc_b[:]),
                                    ALU.mult)
            dn_ps = ppool.tile([P, FD], F32, tag="ps")
            nc.tensor.matmul(dn_ps[:], W["P8"], prod[:])
            carry[("b", t)] = dn_ps

        def col_b_c(t, carry, k=k):
            dn_ps = carry.pop(("b", t))
            z = spool.tile([P, FD], F16, tag="z")
            nc.vector.reciprocal(z[:], dn_ps[:])
            V = spool.tile([P, FD], F16, tag="V")
            nc.vector.tensor_tensor(_q(V[:]), _q(z[:]), _bl(tc_b[:]),
                                    ALU.mult)
            att_ps = ppool.tile([P, FD], F32, tag="ps")
            nc.tensor.matmul(att_ps[:], W[f"cpbrow{k}"], ones_row[:],
                             start=True, stop=False)
            nc.tensor.matmul(att_ps[:], W[f"cp{k}"], V[:],
                             start=False, stop=True)
            x_pre = xpool.tile([P, FD], F16, tag=f"xp{t % GSZ}")
            nc.vector.scalar_tensor_tensor(
                x_pre[:], hp[t][:], W[f"cgcol{k}"], att_ps[:],
                ALU.mult, ALU.add)
            return x_pre

        ln_phase([col_b_a, col_b_b, col_b_c], sq_dve="mix", t1_dve="mix")

        # ---- FFN ----------------------------------------------------------
        def ffn(t, carry=None, k=k, to_hp=False):
            o_ps = ppool.tile([P, FD], F32, tag="ps")
            nc.tensor.matmul(o_ps[:], W[f"f2brow{k}"], ones_row[:],
                             start=True, stop=False)
            for j in range(4):
                h_ps = ppool.tile([P, FD], F32, tag="ps")
                nc.tensor.matmul(h_ps[:], W[f"f1_{k}_{j}"], hp[t][:])
                hid = spool.tile([P, FD], F16, tag="V")
                nc.scalar.activation(hid[:], h_ps[:], AF.Gelu,
                                     bias=W[f"f1b_{k}_{j}"])
                nc.tensor.matmul(o_ps[:], W[f"f2_{k}_{j}"], hid[:],
                                 start=False, stop=(j == 3))
            if to_hp:
                nc.vector.scalar_tensor_tensor(
                    hp[t][:], hp[t][:], W[f"fgcol{k}"], o_ps[:],
                    ALU.mult, ALU.add)
                return None
            x_pre = xpool.tile([P, FD], F16, tag=f"xp{t % GSZ}")
            nc.vector.scalar_tensor_tensor(
                x_pre[:], hp[t][:], W[f"fgcol{k}"], o_ps[:],
                ALU.mult, ALU.add)
            return x_pre

        if k != N_BLOCKS - 1:
            ln_phase(ffn, sq_dve=True, t1_dve=True)
        else:
            for t in range(NT):
                ffn(t, to_hp=True)

    # ============================================================ output
    ystage = wpool.tile([32, 4 * NOG], F32, tag="ystage")
    for gi in range(NOG):
        t0, tend = gi * OGSZ, min(NT, gi * OGSZ + OGSZ)
        ntl = tend - t0
        o_ps = ppers.tile([64, FD], F32, tag="acc1")
        for tau in range(ntl):
            nc.tensor.matmul(o_ps[:32, :],
                             W["outw_lt"][:, tau * 32:(tau + 1) * 32],
                             hp[t0 + tau][:],
                             start=(tau == 0), stop=(tau == ntl - 1))
        ab = gpool.tile([64, FD], F32, tag="ln_mu")
        l1 = gpool.tile([64, FD], F32, tag="ln_e2")
        rl = gpool.tile([64, FD], F32, tag="ln_msq")
        nc.scalar.activation(ab[:32, :], o_ps[:32, :], AF.Abs, bias=W["boutc"])
        nc.scalar.activation(ab[:32, :], ab[:32, :], AF.Exp, scale=-1.0)
        nc.scalar.activation(l1[:32, :], ab[:32, :], AF.Ln, bias=W["onec"])
        nc.scalar.activation(rl[:32, :], o_ps[:32, :], AF.Relu, bias=W["boutc"])
        nc.vector.tensor_add(l1[:32, :], l1[:32, :], rl[:32, :])
        nc.vector.tensor_reduce(
            ystage[:, 4 * gi:4 * gi + 4],
            l1[:32, :].rearrange("p (q l) -> p q l", q=NQ),
            mybir.AxisListType.X, ALU.add)
    nc.sync.dma_start(yout_d[:], ystage[:])
    ctx.close()


# ================================================================ host API
_NC_CACHE = {}


def _get_nc():
    if "nc" not in _NC_CACHE:
        _NC_CACHE["nc"] = build_kernel()
    return _NC_CACHE["nc"]


def kernel(**inputs):
    from concourse.bass_utils import run_bass_kernel_spmd

    nc = _get_nc()
    w = prep_weights(inputs)

    x = np.asarray(inputs["x"])
    in_maps = []
    for core in range(N_CORES):
        b, lq = core // 4, core % 4
        xs = x[b, :, lq * LSH:(lq + 1) * LSH, :]
        xs = np.ascontiguousarray(np.transpose(xs, (0, 2, 1)),
                                  dtype=np.float16)
        m = {"xin": xs, "wpack16": w["wpack16"], "wpack32": w["wpack32"]}
        in_maps.append(m)

    res = run_bass_kernel_spmd(nc, in_maps, core_ids=list(range(N_CORES)))
    outs = [r["yout"] for r in res.results]

    y = np.zeros((B, NB_PAIRS), np.float64)
    for core in range(N_CORES):
        b = core // 4
        st = outs[core].astype(np.float64)
        for gi in range(NOG):
            for tau in range(min(OGSZ, NT - gi * OGSZ)):
                t = gi * OGSZ + tau
                for g in range(2):
                    for q in range(NQ):
                        s = 8 * t + 4 * g + q
                        if s < NB_PAIRS:
                            y[b, s] += st[2 * tau + g, 4 * gi + q]
    y /= SEQ_LEN

    out = np.zeros((B, NB_PAIRS), np.float32)
    ii, jj = np.triu_indices(NB_SEQ, 1)
    tri = {(a, c): p for p, (a, c) in enumerate(zip(ii, jj))}
    for s, (a, c) in enumerate(PAIRS):
        out[:, tri[(a, c)]] = y[:, s]
    return out


# revision 24
# speedup vs baseline: 1.0266x; 1.0176x over previous
"""Trainium2 Bass kernel for nn_AttentionNet (axial linear-attention net).

Sharding: cores 0-3 hold batch b=0, cores 4-7 hold b=1. Within a 4-core
group the sequence axis L=512 is split into 4 shards of 128. Every core
holds ALL 780 pairs for its (b, l-shard), so the instruction stream is
identical on all cores (pure SPMD) and only the input data differs.

Residual state per core: 98 SBUF tiles [128, 512] fp16:
  partition = g*64 + n*16 + d   (g = pair-half 0/1, n = head, d = head ch)
  free      = q*128 + l         (q = pair-quad 0..3, l = local seq pos)
  tile t holds pair slots 8t + 4g + q (784 slots = 780 pairs + 4 pads).

v2 design notes:
- LayerNorm affine (gamma/beta) is folded host-side into every consumer
  matmul; the residual stream stores the UN-affined normalized value
  (h-tilde) and residual adds re-apply gamma via the stt scalar slot and
  beta via a 1-partition bias matmul accumulated into the attention/FFN
  output PSUM.
- LN apply uses a DMA partition-broadcast of [rstd | mean*rstd] rows to
  128 partitions, then two 2x-mode f16 TensorTensor ops. No per-tile
  apply matmuls, no PSUM reads on the apply path.
- LN statistics are computed in groups of 32 tiles (one-hot stat matmuls
  into two persistent PSUM banks).
- Row attention uses affine_mul_reduce to fuse (v+bias)*kbar with the
  per-quad KtV reduction; PSUM->SBUF drains ride the Activation engine.
- Engine balance: DVE keeps the PSUM-coupled ops, Act does elu/gelu
  chains + drains, Pool (gpsimd) takes pure-SBUF squares/multiplies,
  the DMA engines do the LN broadcasts.
"""

import contextlib
import sys

import numpy as np

sys.path.insert(0, "/opt/trn_rl_repo")

mybir = None
F32 = F16 = AF = ALU = None


def _lazy_imports():
    global mybir, F32, F16, AF, ALU
    if mybir is None:
        import concourse.mybir as _mybir
        mybir = _mybir
        F32, F16 = mybir.dt.float32, mybir.dt.float16
        AF = mybir.ActivationFunctionType
        ALU = mybir.AluOpType

NB_SEQ = 40
SEQ_LEN = 512
NB_PAIRS = 780
B = 2
N_BLOCKS = 2
CIN = 22

N_CORES = 8
LSH = 128            # l per core
NQ = 4               # quads per tile
NT = 98              # hp tiles per core
GSZ = 32             # LN group size (tiles)
NGROUP = (NT + GSZ - 1) // GSZ       # 4 (32,32,32,2)
OGSZ = 16
NOG = (NT + OGSZ - 1) // OGSZ        # 7, output stage groups
FD = NQ * LSH        # 512, tile free size


def _pair_order():
    order = []
    for d in range(1, NB_SEQ):
        for i in range(NB_SEQ - d):
            order.append((i, i + d))
    return order


PAIRS = _pair_order()


def slot_ij(s):
    return PAIRS[s] if s < NB_PAIRS else PAIRS[0]


# ================================================================ weights
def prep_weights(inp):
    """Pack all constants; LN affine folded into consumer weights."""
    w = {}
    f16 = lambda a: np.ascontiguousarray(a, dtype=np.float16)
    f32 = lambda a: np.ascontiguousarray(a, dtype=np.float32)

    def col(v, n=128):
        v = np.asarray(v, np.float32).reshape(-1)
        if v.size == 64:
            v = np.tile(v, 2)
        if v.size == 1:
            v = np.full(n, v[0], np.float32)
        return f32(v.reshape(n, 1))

    w_in = np.asarray(inp["w_in"])
    w["wconv"] = f16(np.concatenate([w_in.T, w_in.T], axis=1))
    w["bconv"] = col(inp["b_in"])

    # LN instances: 0 = (g0, be0); k+1 = block-k (ln_g[k], ln_b[k]).
    lns = [(np.asarray(inp["g0"], np.float32),
            np.asarray(inp["be0"], np.float32))]
    for k in range(N_BLOCKS):
        lns.append((np.asarray(inp["ln_g"][k], np.float32),
                    np.asarray(inp["ln_b"][k], np.float32)))

    def bd(m, gamma):
        # block-diag lhsT with gamma folded into input columns
        mt = (np.asarray(m, np.float32) * gamma[None, :]).T
        z = np.zeros((128, 128), np.float16)
        z[:64, :64] = mt
        z[64:, 64:] = mt
        return z

    def fold_bias(m, beta, b):
        return np.asarray(m, np.float32) @ beta + np.asarray(b, np.float32)

    # LN instance feeding each phase's input:
    #  rowA/rowB(k): instance k (ln0 for k=0, ffn-LN of k-1 else)
    #  colA/colB(k): instance k+1 (row-LN of block k)
    #  ffn(k):       instance k+1 (col-LN of block k)
    for k in range(N_BLOCKS):
        gr, br_ = lns[k]        # feeds row attention
        gc, bc_ = lns[k + 1]    # feeds col attention and ffn
        for nm, wk, bk, g_, b_ in [
                ("rq", "rqw", "rqb", gr, br_), ("rk", "rkw", "rkb", gr, br_),
                ("rv", "rvw", "rvb", gr, br_), ("cq", "cqw", "cqb", gc, bc_),
                ("ck", "ckw", "ckb", gc, bc_), ("cv", "cvw", "cvb", gc, bc_)]:
            m = np.asarray(inp[wk][k])
            w[f"{nm}{k}"] = f16(bd(m, g_))
            bb = fold_bias(m, b_, inp[bk][k])
            w[f"{nm}b{k}"] = col(bb)
            w[f"{nm}b1{k}"] = col(bb + 1.0)
            w[f"{nm}bn{k}"] = col(-bb)
        # output projections: plain weights; bias handled by bias-row MM
        for nm, wk in [("rp", "rpw"), ("cp", "cpw")]:
            m = np.asarray(inp[wk][k])
            z = np.zeros((128, 128), np.float16)
            z[:64, :64] = m.T
            z[64:, 64:] = m.T
            w[f"{nm}{k}"] = f16(z)
        # residual bias rows: rp-bias + beta of the residual LN instance
        rpb = np.asarray(inp["rpb"][k], np.float32)
        cpb = np.asarray(inp["cpb"][k], np.float32)
        w[f"rpbrow{k}"] = f16(np.tile(rpb + br_, 2).reshape(1, 128))
        w[f"cpbrow{k}"] = f16(np.tile(cpb + bc_, 2).reshape(1, 128))
        # residual gamma cols
        w[f"rgcol{k}"] = col(gr)
        w[f"cgcol{k}"] = col(gc)
        w[f"fgcol{k}"] = col(gc)

        f1w = np.asarray(inp["f1w"][k], np.float32) * gc[None, :]
        f1b = np.asarray(inp["f1w"][k], np.float32) @ bc_ \
            + np.asarray(inp["f1b"][k], np.float32)
        f2w = np.asarray(inp["f2w"][k])
        for j in range(4):
            g, hh = j // 2, (j % 2) * 128
            lt = np.zeros((128, 128), np.float16)
            lt[g * 64:(g + 1) * 64, :] = f1w[hh:hh + 128, :].T
            w[f"f1_{k}_{j}"] = f16(lt)
            lt2 = np.zeros((128, 128), np.float16)
            lt2[:, g * 64:(g + 1) * 64] = f2w[:, hh:hh + 128].T
            w[f"f2_{k}_{j}"] = f16(lt2)
            w[f"f1b_{k}_{j}"] = f32(f1b[hh:hh + 128].reshape(128, 1))
        f2b = np.asarray(inp["f2b"][k], np.float32)
        w[f"f2brow{k}"] = f16(np.tile(f2b + bc_, 2).reshape(1, 128))
        # last block's ffn has no LN after; residual gamma is col instance
        # (gc) because hp there is the col-LN output of block k.

    # conv + gather feed ln0; gather output is raw (pre-LN) so the rowA
    # weights above already fold ln0 -> nothing extra here.

    # stat slabs for GSZ-tile groups: per tau [128, 64]:
    # rows 2*tau+g get ones at partitions g*64..g*64+64
    stat = np.zeros((128, GSZ * 64), np.float16)
    for tau in range(GSZ):
        for g in range(2):
            stat[g * 64:(g + 1) * 64, tau * 64 + 2 * tau + g] = 1.0
    w["stat_lt"] = f16(stat)

    # output stage (16-tile groups)
    outw = np.zeros((128, OGSZ * 32), np.float16)
    wo = np.asarray(inp["wout"], np.float32).reshape(-1)
    for tau in range(OGSZ):
        for g in range(2):
            outw[g * 64:(g + 1) * 64, tau * 32 + 2 * tau + g] = wo
    w["outw_lt"] = f16(outw)
    w["boutc"] = f32(np.full((32, 1), np.asarray(inp["bout"]).reshape(-1)[0],
                             np.float32))
    w["epsc"] = f32(np.full((64, 1), 1e-5, np.float32))
    w["onec"] = f32(np.full((32, 1), 1.0, np.float32))

    p8 = np.zeros((128, 128), np.float16)
    for blk in range(8):
        p8[blk * 16:(blk + 1) * 16, blk * 16:(blk + 1) * 16] = 1.0
    w["P8"] = f16(p8)
    h64 = np.zeros((128, 64), np.float16)
    h64[np.arange(128), np.arange(128) % 64] = 1.0
    w["H64"] = f16(h64)
    hlast = h64.copy()
    hlast[64:, :] = 0.0
    w["H64_last"] = f16(hlast)
    h64t = np.zeros((64, 128), np.float16)
    h64t[np.arange(128) % 64, np.arange(128)] = 1.0
    w["H64T"] = f16(h64t)

    # ---- pack into two tensors ----
    s16, s32 = _pack_layout()
    p16 = np.zeros((128, s16[-1][2] + s16[-1][3]), np.float16)
    for name, rows, off, cols in s16:
        p16[:rows, off:off + cols] = w[name]
    p32 = np.zeros((128, s32[-1][2] + s32[-1][3]), np.float32)
    for name, rows, off, cols in s32:
        p32[:rows, off:off + cols] = w[name]
    w["wpack16"] = p16
    w["wpack32"] = p32
    return w


def _pack_layout():
    e16, e32 = [], []
    o16 = o32 = 0

    def a16(name, rows, cols):
        nonlocal o16
        e16.append((name, rows, o16, cols))
        o16 += cols

    def a32(name, rows, cols):
        nonlocal o32
        e32.append((name, rows, o32, cols))
        o32 += cols

    a16("wconv", CIN, 128)
    a32("bconv", 128, 1)
    for k in range(N_BLOCKS):
        for nm in ["rq", "rk", "rv", "cq", "ck", "cv"]:
            a16(f"{nm}{k}", 128, 128)
            a32(f"{nm}b{k}", 128, 1)
            a32(f"{nm}b1{k}", 128, 1)
            a32(f"{nm}bn{k}", 128, 1)
        for nm in ["rp", "cp"]:
            a16(f"{nm}{k}", 128, 128)
        a16(f"rpbrow{k}", 1, 128)
        a16(f"cpbrow{k}", 1, 128)
        a32(f"rgcol{k}", 128, 1)
        a32(f"cgcol{k}", 128, 1)
        a32(f"fgcol{k}", 128, 1)
        for j in range(4):
            a16(f"f1_{k}_{j}", 128, 128)
            a16(f"f2_{k}_{j}", 128, 128)
            a32(f"f1b_{k}_{j}", 128, 1)
        a16(f"f2brow{k}", 1, 128)
    a16("stat_lt", 128, GSZ * 64)
    a16("outw_lt", 128, OGSZ * 32)
    a32("boutc", 32, 1)
    a32("epsc", 64, 1)
    a32("onec", 32, 1)
    a16("P8", 128, 128)
    a16("H64", 128, 64)
    a16("H64_last", 128, 64)
    a16("H64T", 64, 128)
    return e16, e32


WEIGHT_SPECS = []


def _spec():
    e16, e32 = _pack_layout()
    n16 = e16[-1][2] + e16[-1][3]
    n32 = e32[-1][2] + e32[-1][3]
    return [("wpack16", (128, n16), F16), ("wpack32", (128, n32), F32)]


# ================================================================ views
def _q(ap):
    return ap.rearrange("p (q l) -> p q l", q=NQ)


def _bq(ap_col4):
    """[128, 4] slice -> [128, 4, 128] broadcast over l."""
    a = ap_col4.copy()
    a.ap = a.ap[:-1] + [list(a.ap[-1]), [0, LSH]]
    return a


def _bl(ap_l):
    """[128, 128] -> [128, 4, 128] broadcast over quads."""
    a = ap_l.copy()
    a.ap = a.ap[:-1] + [[0, NQ], list(a.ap[-1])]
    return a


def _brows(ap_2rows):
    """[2, F] rows -> broadcast to [2, 64, F] (DMA source: each row
    repeated 64x so the dest covers 128 partitions)."""
    a = ap_2rows.copy()
    a.ap = a.ap[:1] + [[0, 64], list(a.ap[-1])]
    return a


# ================================================================ kernel IR
def build_kernel():
    _lazy_imports()
    import concourse.bacc as bacc
    import concourse.tile as tile

    global WEIGHT_SPECS
    WEIGHT_SPECS = _spec()

    nc = bacc.Bacc("TRN2", target_bir_lowering=False, debug=False,
                   num_devices=N_CORES)
    xin_d = nc.dram_tensor("xin", [CIN, NB_SEQ, LSH], F16,
                           kind="ExternalInput").ap()
    wd = {}
    for name, shape, dtype in WEIGHT_SPECS:
        wd[name] = nc.dram_tensor(name, list(shape), dtype,
                                  kind="ExternalInput").ap()
    yout_d = nc.dram_tensor("yout", [32, 4 * NOG], F32,
                            kind="ExternalOutput").ap()

    with tile.TileContext(nc) as tc:
        _body(nc, tc, xin_d, wd, yout_d)
    nc.compile()
    return nc


def _body(nc, tc, xin_d, wd, yout_d):
    ctx = contextlib.ExitStack()
    ctx.enter_context(nc.allow_low_precision(
        reason="normalized activations; f16 everywhere is plenty for 2e-2"))
    P = 128

    wpool = ctx.enter_context(tc.tile_pool(name="w", bufs=1))
    hpool = ctx.enter_context(tc.tile_pool(name="hp", bufs=1))
    spool = ctx.enter_context(tc.tile_pool(name="scr", bufs=3))
    xpool = ctx.enter_context(tc.tile_pool(name="xpre", bufs=1))
    gpool = ctx.enter_context(tc.tile_pool(name="grp", bufs=1))
    stpool = ctx.enter_context(tc.tile_pool(name="st2", bufs=2))
    bpool = ctx.enter_context(tc.tile_pool(name="bc", bufs=3))
    ppool = ctx.enter_context(tc.tile_pool(name="ps", bufs=6, space="PSUM"))
    ppers = ctx.enter_context(tc.tile_pool(name="ps2", bufs=1, space="PSUM"))
    dpool = ctx.enter_context(tc.tile_pool(name="dram", bufs=1, space="DRAM"))

    e16, e32 = _pack_layout()
    n16 = e16[-1][2] + e16[-1][3]
    n32 = e32[-1][2] + e32[-1][3]
    pk16 = wpool.tile([128, n16], F16, tag="pk16", name="pk16")
    pk32 = wpool.tile([128, n32], F32, tag="pk32", name="pk32")
    nc.sync.dma_start(pk16[:], wd["wpack16"][:])
    nc.sync.dma_start(pk32[:], wd["wpack32"][:])
    W = {}
    for name, rows, off, cols in e16:
        W[name] = pk16[:rows, off:off + cols]
    for name, rows, off, cols in e32:
        W[name] = pk32[:rows, off:off + cols]

    hp = [hpool.tile([P, FD], F16, tag=f"hp{t}", name=f"hp{t}")
          for t in range(NT)]

    ksum_pr = wpool.tile([P, NQ * NT], F32, tag="ksum_pr")
    ktv_pr = wpool.tile([P, NQ * NT], F32, tag="ktv_pr")
    ksum_h = wpool.tile([P, NQ * NT], F16, tag="ksum_h")
    ktv_h = wpool.tile([P, NQ * NT], F16, tag="ktv_h")
    kc_b = wpool.tile([P, LSH], F16, tag="kc_b")
    tc_b = wpool.tile([P, LSH], F16, tag="tc_b")
    ones_row = wpool.tile([1, FD], F16, tag="ones_row")
    nc.vector.memset(ones_row[:], 1.0)

    # ============================================================ LN
    # producer(t) -> x_pre tile (REAL post-residual values).
    # sq_dve: engine for the square (True=DVE, False=Pool)
    # t1_dve: engine for the first apply multiply
    def ln_phase(producer, sq_dve=False, t1_dve=False):
        stages = producer if isinstance(producer, (list, tuple)) \
            else [producer]
        ns = len(stages)
        pending = []
        for gi in range(NGROUP):
            t0, tend = gi * GSZ, min(NT, gi * GSZ + GSZ)
            ntl = tend - t0
            s_ps = ppers.tile([64, FD], F32, tag="acc1")
            sq_ps = ppers.tile([64, FD], F32, tag="acc2")
            xs = [None] * ntl

            def finish(tau, x_pre, ntl=ntl, s_ps=s_ps, sq_ps=sq_ps, xs=xs,
                       t0=t0):
                xs[tau] = x_pre
                sq = spool.tile([P, FD], F16, tag="sq")
                use_dve = sq_dve if isinstance(sq_dve, bool) \
                    else (tau % 3 == 0)
                if use_dve:
                    nc.vector.tensor_mul(sq[:], x_pre[:], x_pre[:])
                else:
                    nc.gpsimd.tensor_mul(sq[:], x_pre[:], x_pre[:])
                sl = W["stat_lt"][:, tau * 64:tau * 64 + 64]
                nc.tensor.matmul(s_ps[:], sl, x_pre[:],
                                 start=(tau == 0), stop=(tau == ntl - 1))
                nc.tensor.matmul(sq_ps[:], sl, sq[:],
                                 start=(tau == 0), stop=(tau == ntl - 1))

            carry = {}
            for i in range(ntl + ns - 1):
                for si, f in enumerate(stages):
                    tau = i - si
                    if 0 <= tau < ntl:
                        r = f(t0 + tau, carry)
                        if si == ns - 1:
                            finish(tau, r)
                if pending:
                    pending.pop(0)()
                if pending:
                    pending.pop(0)()
            mu = gpool.tile([64, FD], F32, tag="ln_mu")
            e2 = gpool.tile([64, FD], F32, tag="ln_e2")
            msq = gpool.tile([64, FD], F32, tag="ln_msq")
            st2 = stpool.tile([64, 2 * FD], F16, tag="ln_st2")
            nc.scalar.activation(mu[:], s_ps[:], AF.Copy, scale=1.0 / 64)
            nc.scalar.activation(e2[:], sq_ps[:], AF.Copy, scale=1.0 / 64)
            nc.scalar.activation(msq[:], mu[:], AF.Square)
            nc.vector.tensor_sub(e2[:], e2[:], msq[:])
            nc.scalar.activation(msq[:], e2[:], AF.Ln, bias=W["epsc"])
            nc.scalar.activation(st2[:, :FD], msq[:], AF.Exp, scale=-0.5)
            nc.vector.tensor_mul(st2[:, FD:], mu[:], st2[:, :FD])
            while pending:
                pending.pop(0)()

            bds = {}

            def mk_dma(tau, st2=st2, bds=bds):
                def go():
                    bdst = bpool.tile([P, 2 * FD], F16, tag="bdst")
                    nc.sync.dma_start(bdst[:],
                                      _brows(st2[2 * tau:2 * tau + 2, :]))
                    bds[tau] = bdst
                return go

            def mk_cmp(tau, t0=t0, xs=xs, bds=bds):
                def go():
                    t = t0 + tau
                    bdst = bds.pop(tau)
                    t1 = spool.tile([P, FD], F16, tag="t1")
                    use_dve = t1_dve if isinstance(t1_dve, bool) \
                        else (tau % 2 == 0)
                    if use_dve:
                        nc.vector.tensor_mul(t1[:], xs[tau][:], bdst[:, :FD])
                    else:
                        nc.gpsimd.tensor_mul(t1[:], xs[tau][:], bdst[:, :FD])
                    nc.vector.tensor_sub(hp[t][:], t1[:], bdst[:, FD:])
                return go

            # DMA for apply j runs 2 queue slots ahead of its compute
            q = []
            for tau in range(ntl):
                q.append(mk_dma(tau))
            for tau in range(ntl):
                q.insert(2 * tau + 2 if 2 * tau + 2 < len(q) else len(q),
                         mk_cmp(tau))
            # interleave: after position-building above, q has dma j at
            # slot ~2j and cmp j at slot ~2j+2
            pending.extend(q)
        while pending:
            pending.pop(0)()

    # ============================================================ Phase 0
    h2 = wpool.tile([P, NB_SEQ * LSH], F16, tag="h2")
    xin_f = xin_d.rearrange("c s l -> c (s l)")
    for j in range(10):
        xst = spool.tile([CIN, FD], F16, tag="sq")
        nc.sync.dma_start(xst[:], xin_f[:, j * FD:(j + 1) * FD])
        cps = ppool.tile([P, FD], F32, tag="ps")
        nc.tensor.matmul(cps[:], W["wconv"], xst[:])
        nc.scalar.activation(h2[:, j * FD:(j + 1) * FD], cps[:],
                             AF.Relu, bias=W["bconv"])
    h2q = h2[:].rearrange("p (s l) -> p s l", s=NB_SEQ)

    def gather_producer(t, carry=None):
        x_pre = xpool.tile([P, FD], F16, tag=f"xp{t % GSZ}")
        xq = _q(x_pre[:])
        eng = nc.vector
        for g in range(2):
            ij = [slot_ij(8 * t + 4 * g + q) for q in range(NQ)]
            iis = [a for a, _ in ij]
            jjs = [b for _, b in ij]
            rows = slice(g * 64, g * 64 + 64)
            if (all(iis[q] == iis[0] + q for q in range(NQ)) and
                    all(jjs[q] == jjs[0] + q for q in range(NQ))):
                eng.tensor_add(xq[rows, :, :],
                               h2q[rows, iis[0]:iis[0] + NQ, :],
                               h2q[rows, jjs[0]:jjs[0] + NQ, :])
            else:
                for q in range(NQ):
                    eng.tensor_add(xq[rows, q, :],
                                   h2q[rows, iis[q], :],
                                   h2q[rows, jjs[q], :])
        return x_pre

    ln_phase(gather_producer, sq_dve="mix", t1_dve="mix")

    # ============================================================ blocks
    for k in range(N_BLOCKS):

        # ---- row attention phase A: k/v, local partials (3-stage skew) ---
        rc = {}

        def rowA_a(t, k=k):
            k_ps = ppool.tile([P, FD], F32, tag="ps")
            nc.tensor.matmul(k_ps[:], W[f"rk{k}"], hp[t][:])
            mk = spool.tile([P, FD], F16, tag="mk")
            ek = spool.tile([P, FD], F16, tag="ek")
            nc.scalar.activation(mk[:], k_ps[:], AF.Relu, scale=-1.0,
                                 bias=W[f"rkbn{k}"])
            nc.scalar.activation(ek[:], mk[:], AF.Exp, scale=-1.0)
            rc[t] = (k_ps, ek)

        def rowA_b(t, k=k):
            k_ps, ek = rc.pop(t)
            kt = spool.tile([P, FD], F16, tag="ktil")
            nc.vector.scalar_tensor_tensor(
                kt[:], k_ps[:], W[f"rkb1{k}"], ek[:], ALU.add, ALU.max)
            nc.vector.tensor_reduce(ksum_pr[:, NQ * t:NQ * t + NQ],
                                    _q(kt[:]), mybir.AxisListType.X, ALU.add)
            kb_ps = ppool.tile([P, FD], F32, tag="ps")
            nc.tensor.matmul(kb_ps[:], W["P8"], kt[:])
            kb = spool.tile([P, FD], F16, tag="kbsb")
            nc.scalar.activation(kb[:], kb_ps[:], AF.Identity)
            v_ps = ppool.tile([P, FD], F32, tag="ps")
            nc.tensor.matmul(v_ps[:], W[f"rv{k}"], hp[t][:])
            rc[("b", t)] = (kb, v_ps)

        def rowA_c(t, k=k):
            kb, v_ps = rc.pop(("b", t))
            vw = spool.tile([P, FD], F16, tag="vw")
            for q in range(NQ):
                sl = slice(q * LSH, (q + 1) * LSH)
                nc.vector.affine_mul_reduce(
                    vw[:, sl], ktv_pr[:, NQ * t + q:NQ * t + q + 1],
                    v_ps[:, sl], kb[:, sl], 1.0, W[f"rvb{k}"])

        # AllReduce in two halves: first half overlaps rowA's tail.
        TSPLIT = 64

        def ar(lo, hi, half, k=k):
            c0, c1 = NQ * lo, NQ * hi
            n = c1 - c0
            bin_ = dpool.tile([P, 2 * n], F32, tag=f"arin{k}_{half}")
            bout_ = dpool.tile([P, 2 * n], F32, tag=f"arout{k}_{half}")
            nc.sync.dma_start(bin_[:, :n], ksum_pr[:, c0:c1])
            nc.sync.dma_start(bin_[:, n:], ktv_pr[:, c0:c1])
            nc.gpsimd.collective_compute(
                "AllReduce", ALU.add,
                replica_groups=[[0, 1, 2, 3], [4, 5, 6, 7]],
                ins=[bin_.opt()], outs=[bout_.opt()])
            nc.sync.dma_start(ksum_pr[:, c0:c1], bout_[:, :n])
            nc.sync.dma_start(ktv_pr[:, c0:c1], bout_[:, n:])
            nc.vector.tensor_copy(ksum_h[:, c0:c1], ksum_pr[:, c0:c1])
            nc.vector.tensor_copy(ktv_h[:, c0:c1], ktv_pr[:, c0:c1])

        for i in range(NT + 2):
            if i < NT:
                rowA_a(i)
            if 1 <= i <= NT:
                rowA_b(i - 1)
            if i >= 2:
                rowA_c(i - 2)
            if i == TSPLIT + 2:
                ar(0, TSPLIT, 0)
        ar(TSPLIT, NT, 1)

        # ---- row attention phase B (3-stage skew producer) ---------------
        def row_b_a(t, carry, k=k):
            q_ps = ppool.tile([P, FD], F32, tag="ps")
            nc.tensor.matmul(q_ps[:], W[f"rq{k}"], hp[t][:])
            mq = spool.tile([P, FD], F16, tag="mk")
            eq = spool.tile([P, FD], F16, tag="ek")
            qb1 = spool.tile([P, FD], F16, tag="kbsb")
            nc.scalar.activation(mq[:], q_ps[:], AF.Relu, scale=-1.0,
                                 bias=W[f"rqbn{k}"])
            nc.scalar.activation(eq[:], mq[:], AF.Exp, scale=-1.0)
            nc.scalar.activation(qb1[:], q_ps[:], AF.Identity,
                                 bias=W[f"rqb1{k}"])
            carry[t] = (eq, qb1)

        def row_b_b(t, carry, k=k):
            eq, qb1 = carry.pop(t)
            qt = spool.tile([P, FD], F16, tag="ktil")
            nc.vector.tensor_max(qt[:], qb1[:], eq[:])
            prod = spool.tile([P, FD], F16, tag="vw")
            nc.vector.tensor_tensor(_q(prod[:]), _q(qt[:]),
                                    _bq(ksum_h[:, NQ * t:NQ * t + NQ]),
                                    ALU.mult)
            dn_ps = ppool.tile([P, FD], F32, tag="ps")
            nc.tensor.matmul(dn_ps[:], W["P8"], prod[:])
            carry[("b", t)] = dn_ps

        def row_b_c(t, carry, k=k):
            dn_ps = carry.pop(("b", t))
            z = spool.tile([P, FD], F16, tag="z")
            nc.vector.reciprocal(z[:], dn_ps[:])
            V = spool.tile([P, FD], F16, tag="V")
            nc.vector.tensor_tensor(_q(V[:]), _q(z[:]),
                                    _bq(ktv_h[:, NQ * t:NQ * t + NQ]),
                                    ALU.mult)
            att_ps = ppool.tile([P, FD], F32, tag="ps")
            nc.tensor.matmul(att_ps[:], W[f"rpbrow{k}"], ones_row[:],
                             start=True, stop=False)
            nc.tensor.matmul(att_ps[:], W[f"rp{k}"], V[:],
                             start=False, stop=True)
            carry[("c", t)] = att_ps

        def row_b_d(t, carry, k=k):
            att_ps = carry.pop(("c", t))
            x_pre = xpool.tile([P, FD], F16, tag=f"xp{t % GSZ}")
            nc.vector.scalar_tensor_tensor(
                x_pre[:], hp[t][:], W[f"rgcol{k}"], att_ps[:],
                ALU.mult, ALU.add)
            return x_pre

        ln_phase([row_b_a, row_b_b, row_b_c, row_b_d],
                 sq_dve="mix", t1_dve="mix")

        # ---- column attention phase A: k/v + local pair reduction --------
        kc_ps = ppers.tile([64, FD], F32, tag="acc1")
        tv_ps = ppers.tile([64, FD], F32, tag="acc2")
        cc = {}

        def colA_a(t, k=k):
            ck_ps = ppool.tile([P, FD], F32, tag="ps")
            nc.tensor.matmul(ck_ps[:], W[f"ck{k}"], hp[t][:])
            mk = spool.tile([P, FD], F16, tag="mk")
            ek = spool.tile([P, FD], F16, tag="ek")
            nc.scalar.activation(mk[:], ck_ps[:], AF.Relu, scale=-1.0,
                                 bias=W[f"ckbn{k}"])
            nc.scalar.activation(ek[:], mk[:], AF.Exp, scale=-1.0)
            cc[t] = (ck_ps, ek)

        def colA_b(t, k=k):
            ck_ps, ek = cc.pop(t)
            h64 = W["H64_last"] if t == NT - 1 else W["H64"]
            kt = spool.tile([P, FD], F16, tag="ktil")
            nc.vector.scalar_tensor_tensor(
                kt[:], ck_ps[:], W[f"ckb1{k}"], ek[:], ALU.add, ALU.max)
            kb_ps = ppool.tile([P, FD], F32, tag="ps")
            nc.tensor.matmul(kb_ps[:], W["P8"], kt[:])
            kb = spool.tile([P, FD], F16, tag="kbsb")
            if t % 2 == 0:
                nc.scalar.activation(kb[:], kb_ps[:], AF.Identity)
            else:
                nc.vector.tensor_copy(kb[:], kb_ps[:])
            nc.tensor.matmul(kc_ps[:], h64[:], kt[:],
                             start=(t == 0), stop=(t == NT - 1))
            cc[("b", t)] = kb

        def colA_c(t, k=k):
            kb = cc.pop(("b", t))
            h64 = W["H64_last"] if t == NT - 1 else W["H64"]
            cv_ps = ppool.tile([P, FD], F32, tag="ps")
            nc.tensor.matmul(cv_ps[:], W[f"cv{k}"], hp[t][:])
            vw = spool.tile([P, FD], F16, tag="vw")
            nc.vector.scalar_tensor_tensor(
                vw[:], cv_ps[:], W[f"cvb{k}"], kb[:], ALU.add, ALU.mult)
            nc.tensor.matmul(tv_ps[:], h64[:], vw[:],
                             start=(t == 0), stop=(t == NT - 1))

        for i in range(NT + 2):
            if i < NT:
                colA_a(i)
            if 1 <= i <= NT:
                colA_b(i - 1)
            if i >= 2:
                colA_c(i - 2)
        kcs_sb = gpool.tile([64, FD], F32, tag="ln_mu")
        tvs_sb = gpool.tile([64, FD], F32, tag="ln_e2")
        nc.vector.tensor_copy(kcs_sb[:], kc_ps[:])
        nc.vector.tensor_copy(tvs_sb[:], tv_ps[:])
        ksc = gpool.tile([64, LSH], F16, tag="ksc")
        tvc = gpool.tile([64, LSH], F16, tag="tvc")
        fo1 = gpool.tile([64, LSH], F16, tag="fold1")
        fo2 = gpool.tile([64, LSH], F16, tag="fold2")
        kq, tq = _q(kcs_sb[:]), _q(tvs_sb[:])
        nc.vector.tensor_add(fo1[:], kq[:, 0, :], kq[:, 1, :])
        nc.vector.tensor_add(ksc[:], kq[:, 2, :], kq[:, 3, :])
        nc.vector.tensor_add(ksc[:], fo1[:], ksc[:])
        nc.gpsimd.tensor_add(fo2[:], tq[:, 0, :], tq[:, 1, :])
        nc.gpsimd.tensor_add(tvc[:], tq[:, 2, :], tq[:, 3, :])
        nc.gpsimd.tensor_add(tvc[:], fo2[:], tvc[:])
        kcb_ps = ppool.tile([P, FD], F32, tag="ps")
        nc.tensor.matmul(kcb_ps[:, :LSH], W["H64T"], ksc[:])
        nc.vector.tensor_copy(kc_b[:], kcb_ps[:, :LSH])
        tcb_ps = ppool.tile([P, FD], F32, tag="ps")
        nc.tensor.matmul(tcb_ps[:, :LSH], W["H64T"], tvc[:])
        nc.vector.tensor_copy(tc_b[:], tcb_ps[:, :LSH])

        # ---- column attention phase B (3-stage skew producer) ------------
        def col_b_a(t, carry, k=k):
            q_ps = ppool.tile([P, FD], F32, tag="ps")
            nc.tensor.matmul(q_ps[:], W[f"cq{k}"], hp[t][:])
            mq = spool.tile([P, FD], F16, tag="mk")
            eq = spool.tile([P, FD], F16, tag="ek")
            qb1 = spool.tile([P, FD], F16, tag="kbsb")
            nc.scalar.activation(mq[:], q_ps[:], AF.Relu, scale=-1.0,
                                 bias=W[f"cqbn{k}"])
            nc.scalar.activation(eq[:], mq[:], AF.Exp, scale=-1.0)
            nc.scalar.activation(qb1[:], q_ps[:], AF.Identity,
                                 bias=W[f"cqb1{k}"])
            carry[t] = (eq, qb1)

        def col_b_b(t, carry, k=k):
            eq, qb1 = carry.pop(t)
            qt = spool.tile([P, FD], F16, tag="ktil")
            nc.vector.tensor_max(qt[:], qb1[:], eq[:])
            prod = spool.tile([P, FD], F16, tag="vw")
            nc.vector.tensor_tensor(_q(prod[:]), _q(qt[:]), _bl(kc_b[:]),
                                    ALU.mult)
            dn_ps = ppool.tile([P, FD], F32, tag="ps")
            nc.tensor.matmul(dn_ps[:], W["P8"], prod[:])
            carry[("b", t)] = dn_ps

        def col_b_c(t, carry, k=k):
            dn_ps = carry.pop(("b", t))
            z = spool.tile([P, FD], F16, tag="z")
            nc.vector.reciprocal(z[:], dn_ps[:])
            V = spool.tile([P, FD], F16, tag="V")
            nc.vector.tensor_tensor(_q(V[:]), _q(z[:]), _bl(tc_b[:]),
                                    ALU.mult)
            att_ps = ppool.tile([P, FD], F32, tag="ps")
            nc.tensor.matmul(att_ps[:], W[f"cpbrow{k}"], ones_row[:],
                             start=True, stop=False)
            nc.tensor.matmul(att_ps[:], W[f"cp{k}"], V[:],
                             start=False, stop=True)
            carry[("c", t)] = att_ps

        def col_b_d(t, carry, k=k):
            att_ps = carry.pop(("c", t))
            x_pre = xpool.tile([P, FD], F16, tag=f"xp{t % GSZ}")
            nc.vector.scalar_tensor_tensor(
                x_pre[:], hp[t][:], W[f"cgcol{k}"], att_ps[:],
                ALU.mult, ALU.add)
            return x_pre

        ln_phase([col_b_a, col_b_b, col_b_c, col_b_d],
                 sq_dve="mix", t1_dve="mix")

        # ---- FFN ----------------------------------------------------------
        def ffn(t, carry=None, k=k, to_hp=False):
            o_ps = ppool.tile([P, FD], F32, tag="ps")
            nc.tensor.matmul(o_ps[:], W[f"f2brow{k}"], ones_row[:],
                             start=True, stop=False)
            for j in range(4):
                h_ps = ppool.tile([P, FD], F32, tag="ps")
                nc.tensor.matmul(h_ps[:], W[f"f1_{k}_{j}"], hp[t][:])
                hid = spool.tile([P, FD], F16, tag="V")
                nc.scalar.activation(hid[:], h_ps[:], AF.Gelu,
                                     bias=W[f"f1b_{k}_{j}"])
                nc.tensor.matmul(o_ps[:], W[f"f2_{k}_{j}"], hid[:],
                                 start=False, stop=(j == 3))
            if to_hp:
                nc.vector.scalar_tensor_tensor(
                    hp[t][:], hp[t][:], W[f"fgcol{k}"], o_ps[:],
                    ALU.mult, ALU.add)
                return None
            x_pre = xpool.tile([P, FD], F16, tag=f"xp{t % GSZ}")
            nc.vector.scalar_tensor_tensor(
                x_pre[:], hp[t][:], W[f"fgcol{k}"], o_ps[:],
                ALU.mult, ALU.add)
            return x_pre

        if k != N_BLOCKS - 1:
            ln_phase(ffn, sq_dve=True, t1_dve=True)
        else:
            for t in range(NT):
                ffn(t, to_hp=True)

    # ============================================================ output
    ystage = wpool.tile([32, 4 * NOG], F32, tag="ystage")
    for gi in range(NOG):
        t0, tend = gi * OGSZ, min(NT, gi * OGSZ + OGSZ)
        ntl = tend - t0
        o_ps = ppers.tile([64, FD], F32, tag="acc1")
        for tau in range(ntl):
            nc.tensor.matmul(o_ps[:32, :],
                             W["outw_lt"][:, tau * 32:(tau + 1) * 32],
                             hp[t0 + tau][:],
                             start=(tau == 0), stop=(tau == ntl - 1))
        ab = gpool.tile([64, FD], F32, tag="ln_mu")
        l1 = gpool.tile([64, FD], F32, tag="ln_e2")
        rl = gpool.tile([64, FD], F32, tag="ln_msq")
        nc.scalar.activation(ab[:32, :], o_ps[:32, :], AF.Abs, bias=W["boutc"])
        nc.scalar.activation(ab[:32, :], ab[:32, :], AF.Exp, scale=-1.0)
        nc.scalar.activation(l1[:32, :], ab[:32, :], AF.Ln, bias=W["onec"])
        nc.scalar.activation(rl[:32, :], o_ps[:32, :], AF.Relu, bias=W["boutc"])
        nc.vector.tensor_add(l1[:32, :], l1[:32, :], rl[:32, :])
        nc.vector.tensor_reduce(
            ystage[:, 4 * gi:4 * gi + 4],
            l1[:32, :].rearrange("p (q l) -> p q l", q=NQ),
            mybir.AxisListType.X, ALU.add)
    nc.sync.dma_start(yout_d[:], ystage[:])
    ctx.close()


# ================================================================ host API
_NC_CACHE = {}


def _get_nc():
    if "nc" not in _NC_CACHE:
        _NC_CACHE["nc"] = build_kernel()
    return _NC_CACHE["nc"]


def kernel(**inputs):
    from concourse.bass_utils import run_bass_kernel_spmd

    nc = _get_nc()
    w = prep_weights(inputs)

    x = np.asarray(inputs["x"])
    in_maps = []
    for core in range(N_CORES):
        b, lq = core // 4, core % 4
        xs = x[b, :, lq * LSH:(lq + 1) * LSH, :]
        xs = np.ascontiguousarray(np.transpose(xs, (0, 2, 1)),
                                  dtype=np.float16)
        m = {"xin": xs, "wpack16": w["wpack16"], "wpack32": w["wpack32"]}
        in_maps.append(m)

    res = run_bass_kernel_spmd(nc, in_maps, core_ids=list(range(N_CORES)))
    outs = [r["yout"] for r in res.results]

    y = np.zeros((B, NB_PAIRS), np.float64)
    for core in range(N_CORES):
        b = core // 4
        st = outs[core].astype(np.float64)
        for gi in range(NOG):
            for tau in range(min(OGSZ, NT - gi * OGSZ)):
                t = gi * OGSZ + tau
                for g in range(2):
                    for q in range(NQ):
                        s = 8 * t + 4 * g + q
                        if s < NB_PAIRS:
                            y[b, s] += st[2 * tau + g, 4 * gi + q]
    y /= SEQ_LEN

    out = np.zeros((B, NB_PAIRS), np.float32)
    ii, jj = np.triu_indices(NB_SEQ, 1)
    tri = {(a, c): p for p, (a, c) in enumerate(zip(ii, jj))}
    for s, (a, c) in enumerate(PAIRS):
        out[:, tri[(a, c)]] = y[:, s]
    return out


# revision 25
# speedup vs baseline: 1.0403x; 1.0133x over previous
"""Trainium2 Bass kernel for nn_AttentionNet (axial linear-attention net).

Sharding: cores 0-3 hold batch b=0, cores 4-7 hold b=1. Within a 4-core
group the sequence axis L=512 is split into 4 shards of 128. Every core
holds ALL 780 pairs for its (b, l-shard), so the instruction stream is
identical on all cores (pure SPMD) and only the input data differs.

Residual state per core: 98 SBUF tiles [128, 512] fp16:
  partition = g*64 + n*16 + d   (g = pair-half 0/1, n = head, d = head ch)
  free      = q*128 + l         (q = pair-quad 0..3, l = local seq pos)
  tile t holds pair slots 8t + 4g + q (784 slots = 780 pairs + 4 pads).

v2 design notes:
- LayerNorm affine (gamma/beta) is folded host-side into every consumer
  matmul; the residual stream stores the UN-affined normalized value
  (h-tilde) and residual adds re-apply gamma via the stt scalar slot and
  beta via a 1-partition bias matmul accumulated into the attention/FFN
  output PSUM.
- LN apply uses a DMA partition-broadcast of [rstd | mean*rstd] rows to
  128 partitions, then two 2x-mode f16 TensorTensor ops. No per-tile
  apply matmuls, no PSUM reads on the apply path.
- LN statistics are computed in groups of 32 tiles (one-hot stat matmuls
  into two persistent PSUM banks).
- Row attention uses affine_mul_reduce to fuse (v+bias)*kbar with the
  per-quad KtV reduction; PSUM->SBUF drains ride the Activation engine.
- Engine balance: DVE keeps the PSUM-coupled ops, Act does elu/gelu
  chains + drains, Pool (gpsimd) takes pure-SBUF squares/multiplies,
  the DMA engines do the LN broadcasts.
"""

import contextlib
import sys

import numpy as np

sys.path.insert(0, "/opt/trn_rl_repo")

mybir = None
F32 = F16 = AF = ALU = None


def _lazy_imports():
    global mybir, F32, F16, AF, ALU
    if mybir is None:
        import concourse.mybir as _mybir
        mybir = _mybir
        F32, F16 = mybir.dt.float32, mybir.dt.float16
        AF = mybir.ActivationFunctionType
        ALU = mybir.AluOpType

NB_SEQ = 40
SEQ_LEN = 512
NB_PAIRS = 780
B = 2
N_BLOCKS = 2
CIN = 22

N_CORES = 8
LSH = 128            # l per core
NQ = 4               # quads per tile
NT = 98              # hp tiles per core
GSZ = 32             # LN group size (tiles)
NGROUP = (NT + GSZ - 1) // GSZ       # 4 (32,32,32,2)
OGSZ = 16
NOG = (NT + OGSZ - 1) // OGSZ        # 7, output stage groups
FD = NQ * LSH        # 512, tile free size


def _pair_order():
    order = []
    for d in range(1, NB_SEQ):
        for i in range(NB_SEQ - d):
            order.append((i, i + d))
    return order


PAIRS = _pair_order()


def slot_ij(s):
    return PAIRS[s] if s < NB_PAIRS else PAIRS[0]


# ================================================================ weights
def prep_weights(inp):
    """Pack all constants; LN affine folded into consumer weights."""
    w = {}
    f16 = lambda a: np.ascontiguousarray(a, dtype=np.float16)
    f32 = lambda a: np.ascontiguousarray(a, dtype=np.float32)

    def col(v, n=128):
        v = np.asarray(v, np.float32).reshape(-1)
        if v.size == 64:
            v = np.tile(v, 2)
        if v.size == 1:
            v = np.full(n, v[0], np.float32)
        return f32(v.reshape(n, 1))

    w_in = np.asarray(inp["w_in"])
    w["wconv"] = f16(np.concatenate([w_in.T, w_in.T], axis=1))
    w["bconv"] = col(inp["b_in"])

    # LN instances: 0 = (g0, be0); k+1 = block-k (ln_g[k], ln_b[k]).
    lns = [(np.asarray(inp["g0"], np.float32),
            np.asarray(inp["be0"], np.float32))]
    for k in range(N_BLOCKS):
        lns.append((np.asarray(inp["ln_g"][k], np.float32),
                    np.asarray(inp["ln_b"][k], np.float32)))

    def bd(m, gamma):
        # block-diag lhsT with gamma folded into input columns
        mt = (np.asarray(m, np.float32) * gamma[None, :]).T
        z = np.zeros((128, 128), np.float16)
        z[:64, :64] = mt
        z[64:, 64:] = mt
        return z

    def fold_bias(m, beta, b):
        return np.asarray(m, np.float32) @ beta + np.asarray(b, np.float32)

    # LN instance feeding each phase's input:
    #  rowA/rowB(k): instance k (ln0 for k=0, ffn-LN of k-1 else)
    #  colA/colB(k): instance k+1 (row-LN of block k)
    #  ffn(k):       instance k+1 (col-LN of block k)
    for k in range(N_BLOCKS):
        gr, br_ = lns[k]        # feeds row attention
        gc, bc_ = lns[k + 1]    # feeds col attention and ffn
        for nm, wk, bk, g_, b_ in [
                ("rq", "rqw", "rqb", gr, br_), ("rk", "rkw", "rkb", gr, br_),
                ("rv", "rvw", "rvb", gr, br_), ("cq", "cqw", "cqb", gc, bc_),
                ("ck", "ckw", "ckb", gc, bc_), ("cv", "cvw", "cvb", gc, bc_)]:
            m = np.asarray(inp[wk][k])
            w[f"{nm}{k}"] = f16(bd(m, g_))
            bb = fold_bias(m, b_, inp[bk][k])
            w[f"{nm}b{k}"] = col(bb)
            w[f"{nm}b1{k}"] = col(bb + 1.0)
            w[f"{nm}bn{k}"] = col(-bb)
        # output projections: plain weights; bias handled by bias-row MM
        for nm, wk in [("rp", "rpw"), ("cp", "cpw")]:
            m = np.asarray(inp[wk][k])
            z = np.zeros((128, 128), np.float16)
            z[:64, :64] = m.T
            z[64:, 64:] = m.T
            w[f"{nm}{k}"] = f16(z)
        # residual bias rows: rp-bias + beta of the residual LN instance
        rpb = np.asarray(inp["rpb"][k], np.float32)
        cpb = np.asarray(inp["cpb"][k], np.float32)
        w[f"rpbrow{k}"] = f16(np.tile(rpb + br_, 2).reshape(1, 128))
        w[f"cpbrow{k}"] = f16(np.tile(cpb + bc_, 2).reshape(1, 128))
        # residual gamma cols
        w[f"rgcol{k}"] = col(gr)
        w[f"cgcol{k}"] = col(gc)
        w[f"fgcol{k}"] = col(gc)

        f1w = np.asarray(inp["f1w"][k], np.float32) * gc[None, :]
        f1b = np.asarray(inp["f1w"][k], np.float32) @ bc_ \
            + np.asarray(inp["f1b"][k], np.float32)
        f2w = np.asarray(inp["f2w"][k])
        for j in range(4):
            g, hh = j // 2, (j % 2) * 128
            lt = np.zeros((128, 128), np.float16)
            lt[g * 64:(g + 1) * 64, :] = f1w[hh:hh + 128, :].T
            w[f"f1_{k}_{j}"] = f16(lt)
            lt2 = np.zeros((128, 128), np.float16)
            lt2[:, g * 64:(g + 1) * 64] = f2w[:, hh:hh + 128].T
            w[f"f2_{k}_{j}"] = f16(lt2)
            w[f"f1b_{k}_{j}"] = f32(f1b[hh:hh + 128].reshape(128, 1))
        f2b = np.asarray(inp["f2b"][k], np.float32)
        w[f"f2brow{k}"] = f16(np.tile(f2b + bc_, 2).reshape(1, 128))
        # last block's ffn has no LN after; residual gamma is col instance
        # (gc) because hp there is the col-LN output of block k.

    # conv + gather feed ln0; gather output is raw (pre-LN) so the rowA
    # weights above already fold ln0 -> nothing extra here.

    # stat slabs for GSZ-tile groups: per tau [128, 64]:
    # rows 2*tau+g get ones at partitions g*64..g*64+64
    stat = np.zeros((128, GSZ * 64), np.float16)
    for tau in range(GSZ):
        for g in range(2):
            stat[g * 64:(g + 1) * 64, tau * 64 + 2 * tau + g] = 1.0
    w["stat_lt"] = f16(stat)

    # output stage (16-tile groups)
    outw = np.zeros((128, OGSZ * 32), np.float16)
    wo = np.asarray(inp["wout"], np.float32).reshape(-1)
    for tau in range(OGSZ):
        for g in range(2):
            outw[g * 64:(g + 1) * 64, tau * 32 + 2 * tau + g] = wo
    w["outw_lt"] = f16(outw)
    w["boutc"] = f32(np.full((32, 1), np.asarray(inp["bout"]).reshape(-1)[0],
                             np.float32))
    w["epsc"] = f32(np.full((64, 1), 1e-5, np.float32))
    w["c64"] = f32(np.full((64, 1), 1.0 / 64, np.float32))
    w["onec"] = f32(np.full((32, 1), 1.0, np.float32))

    p8 = np.zeros((128, 128), np.float16)
    for blk in range(8):
        p8[blk * 16:(blk + 1) * 16, blk * 16:(blk + 1) * 16] = 1.0
    w["P8"] = f16(p8)
    h64 = np.zeros((128, 64), np.float16)
    h64[np.arange(128), np.arange(128) % 64] = 1.0
    w["H64"] = f16(h64)
    hlast = h64.copy()
    hlast[64:, :] = 0.0
    w["H64_last"] = f16(hlast)
    h64t = np.zeros((64, 128), np.float16)
    h64t[np.arange(128) % 64, np.arange(128)] = 1.0
    w["H64T"] = f16(h64t)

    # ---- pack into two tensors ----
    s16, s32 = _pack_layout()
    p16 = np.zeros((128, s16[-1][2] + s16[-1][3]), np.float16)
    for name, rows, off, cols in s16:
        p16[:rows, off:off + cols] = w[name]
    p32 = np.zeros((128, s32[-1][2] + s32[-1][3]), np.float32)
    for name, rows, off, cols in s32:
        p32[:rows, off:off + cols] = w[name]
    w["wpack16"] = p16
    w["wpack32"] = p32
    return w


def _pack_layout():
    e16, e32 = [], []
    o16 = o32 = 0

    def a16(name, rows, cols):
        nonlocal o16
        e16.append((name, rows, o16, cols))
        o16 += cols

    def a32(name, rows, cols):
        nonlocal o32
        e32.append((name, rows, o32, cols))
        o32 += cols

    a16("wconv", CIN, 128)
    a32("bconv", 128, 1)
    for k in range(N_BLOCKS):
        for nm in ["rq", "rk", "rv", "cq", "ck", "cv"]:
            a16(f"{nm}{k}", 128, 128)
            a32(f"{nm}b{k}", 128, 1)
            a32(f"{nm}b1{k}", 128, 1)
            a32(f"{nm}bn{k}", 128, 1)
        for nm in ["rp", "cp"]:
            a16(f"{nm}{k}", 128, 128)
        a16(f"rpbrow{k}", 1, 128)
        a16(f"cpbrow{k}", 1, 128)
        a32(f"rgcol{k}", 128, 1)
        a32(f"cgcol{k}", 128, 1)
        a32(f"fgcol{k}", 128, 1)
        for j in range(4):
            a16(f"f1_{k}_{j}", 128, 128)
            a16(f"f2_{k}_{j}", 128, 128)
            a32(f"f1b_{k}_{j}", 128, 1)
        a16(f"f2brow{k}", 1, 128)
    a16("stat_lt", 128, GSZ * 64)
    a16("outw_lt", 128, OGSZ * 32)
    a32("boutc", 32, 1)
    a32("epsc", 64, 1)
    a32("c64", 64, 1)
    a32("onec", 32, 1)
    a16("P8", 128, 128)
    a16("H64", 128, 64)
    a16("H64_last", 128, 64)
    a16("H64T", 64, 128)
    return e16, e32


WEIGHT_SPECS = []


def _spec():
    e16, e32 = _pack_layout()
    n16 = e16[-1][2] + e16[-1][3]
    n32 = e32[-1][2] + e32[-1][3]
    return [("wpack16", (128, n16), F16), ("wpack32", (128, n32), F32)]


# ================================================================ views
def _q(ap):
    return ap.rearrange("p (q l) -> p q l", q=NQ)


def _bq(ap_col4):
    """[128, 4] slice -> [128, 4, 128] broadcast over l."""
    a = ap_col4.copy()
    a.ap = a.ap[:-1] + [list(a.ap[-1]), [0, LSH]]
    return a


def _bl(ap_l):
    """[128, 128] -> [128, 4, 128] broadcast over quads."""
    a = ap_l.copy()
    a.ap = a.ap[:-1] + [[0, NQ], list(a.ap[-1])]
    return a


def _brows(ap_2rows):
    """[2, F] rows -> broadcast to [2, 64, F] (DMA source: each row
    repeated 64x so the dest covers 128 partitions)."""
    a = ap_2rows.copy()
    a.ap = a.ap[:1] + [[0, 64], list(a.ap[-1])]
    return a


# ================================================================ kernel IR
def build_kernel():
    _lazy_imports()
    import concourse.bacc as bacc
    import concourse.tile as tile

    global WEIGHT_SPECS
    WEIGHT_SPECS = _spec()

    nc = bacc.Bacc("TRN2", target_bir_lowering=False, debug=False,
                   num_devices=N_CORES)
    xin_d = nc.dram_tensor("xin", [CIN, NB_SEQ, LSH], F16,
                           kind="ExternalInput").ap()
    wd = {}
    for name, shape, dtype in WEIGHT_SPECS:
        wd[name] = nc.dram_tensor(name, list(shape), dtype,
                                  kind="ExternalInput").ap()
    yout_d = nc.dram_tensor("yout", [32, 4 * NOG], F32,
                            kind="ExternalOutput").ap()

    with tile.TileContext(nc) as tc:
        _body(nc, tc, xin_d, wd, yout_d)
    nc.compile()
    return nc


def _body(nc, tc, xin_d, wd, yout_d):
    ctx = contextlib.ExitStack()
    ctx.enter_context(nc.allow_low_precision(
        reason="normalized activations; f16 everywhere is plenty for 2e-2"))
    P = 128

    wpool = ctx.enter_context(tc.tile_pool(name="w", bufs=1))
    hpool = ctx.enter_context(tc.tile_pool(name="hp", bufs=1))
    spool = ctx.enter_context(tc.tile_pool(name="scr", bufs=3))
    xpool = ctx.enter_context(tc.tile_pool(name="xpre", bufs=1))
    gpool = ctx.enter_context(tc.tile_pool(name="grp", bufs=1))
    stpool = ctx.enter_context(tc.tile_pool(name="st2", bufs=2))
    bpool = ctx.enter_context(tc.tile_pool(name="bc", bufs=3))
    ppool = ctx.enter_context(tc.tile_pool(name="ps", bufs=6, space="PSUM"))
    ppers = ctx.enter_context(tc.tile_pool(name="ps2", bufs=1, space="PSUM"))
    dpool = ctx.enter_context(tc.tile_pool(name="dram", bufs=1, space="DRAM"))

    e16, e32 = _pack_layout()
    n16 = e16[-1][2] + e16[-1][3]
    n32 = e32[-1][2] + e32[-1][3]
    pk16 = wpool.tile([128, n16], F16, tag="pk16", name="pk16")
    pk32 = wpool.tile([128, n32], F32, tag="pk32", name="pk32")
    nc.sync.dma_start(pk16[:], wd["wpack16"][:])
    nc.sync.dma_start(pk32[:], wd["wpack32"][:])
    W = {}
    for name, rows, off, cols in e16:
        W[name] = pk16[:rows, off:off + cols]
    for name, rows, off, cols in e32:
        W[name] = pk32[:rows, off:off + cols]

    hp = [hpool.tile([P, FD], F16, tag=f"hp{t}", name=f"hp{t}")
          for t in range(NT)]

    ksum_pr = wpool.tile([P, NQ * NT], F32, tag="ksum_pr")
    ktv_pr = wpool.tile([P, NQ * NT], F32, tag="ktv_pr")
    ksum_h = wpool.tile([P, NQ * NT], F16, tag="ksum_h")
    ktv_h = wpool.tile([P, NQ * NT], F16, tag="ktv_h")
    kc_b = wpool.tile([P, LSH], F16, tag="kc_b")
    tc_b = wpool.tile([P, LSH], F16, tag="tc_b")
    ones_row = wpool.tile([1, FD], F16, tag="ones_row")
    nc.vector.memset(ones_row[:], 1.0)

    # ============================================================ LN
    # producer(t) -> x_pre tile (REAL post-residual values).
    # sq_dve: engine for the square (True=DVE, False=Pool)
    # t1_dve: engine for the first apply multiply
    def ln_phase(producer, sq_dve=False, t1_dve=False):
        stages = producer if isinstance(producer, (list, tuple)) \
            else [producer]
        ns = len(stages)
        pending = []
        for gi in range(NGROUP):
            t0, tend = gi * GSZ, min(NT, gi * GSZ + GSZ)
            ntl = tend - t0
            s_ps = ppers.tile([64, FD], F32, tag="acc1")
            sq_ps = ppers.tile([64, FD], F32, tag="acc2")
            xs = [None] * ntl

            def finish(tau, x_pre, ntl=ntl, s_ps=s_ps, sq_ps=sq_ps, xs=xs,
                       t0=t0):
                xs[tau] = x_pre
                sq = spool.tile([P, FD], F16, tag="sq")
                use_dve = sq_dve if isinstance(sq_dve, bool) \
                    else (tau % 3 == 0)
                if use_dve:
                    nc.vector.tensor_mul(sq[:], x_pre[:], x_pre[:])
                else:
                    nc.gpsimd.tensor_mul(sq[:], x_pre[:], x_pre[:])
                sl = W["stat_lt"][:, tau * 64:tau * 64 + 64]
                nc.tensor.matmul(s_ps[:], sl, x_pre[:],
                                 start=(tau == 0), stop=(tau == ntl - 1))
                nc.tensor.matmul(sq_ps[:], sl, sq[:],
                                 start=(tau == 0), stop=(tau == ntl - 1))

            carry = {}
            for i in range(ntl + ns - 1):
                for si, f in enumerate(stages):
                    tau = i - si
                    if 0 <= tau < ntl:
                        r = f(t0 + tau, carry)
                        if si == ns - 1:
                            finish(tau, r)
                if pending:
                    pending.pop(0)()
                if pending:
                    pending.pop(0)()
            mu = gpool.tile([64, FD], F32, tag="ln_mu")
            e2 = gpool.tile([64, FD], F32, tag="ln_e2")
            msq = gpool.tile([64, FD], F32, tag="ln_msq")
            st2 = stpool.tile([64, 2 * FD], F16, tag="ln_st2")
            nc.scalar.activation(mu[:], s_ps[:], AF.Copy, scale=1.0 / 64)
            nc.scalar.activation(msq[:], mu[:], AF.Square)
            nc.vector.scalar_tensor_tensor(
                e2[:], sq_ps[:], W["c64"], msq[:], ALU.mult, ALU.subtract)
            nc.scalar.activation(msq[:], e2[:], AF.Ln, bias=W["epsc"])
            nc.scalar.activation(st2[:, :FD], msq[:], AF.Exp, scale=-0.5)
            nc.vector.tensor_mul(st2[:, FD:], mu[:], st2[:, :FD])
            while pending:
                pending.pop(0)()

            bds = {}

            def mk_dma(tau, st2=st2, bds=bds):
                def go():
                    bdst = bpool.tile([P, 2 * FD], F16, tag="bdst")
                    nc.sync.dma_start(bdst[:],
                                      _brows(st2[2 * tau:2 * tau + 2, :]))
                    bds[tau] = bdst
                return go

            def mk_cmp(tau, t0=t0, xs=xs, bds=bds):
                def go():
                    t = t0 + tau
                    bdst = bds.pop(tau)
                    t1 = spool.tile([P, FD], F16, tag="t1")
                    use_dve = t1_dve if isinstance(t1_dve, bool) \
                        else (tau % 2 == 0)
                    if use_dve:
                        nc.vector.tensor_mul(t1[:], xs[tau][:], bdst[:, :FD])
                    else:
                        nc.gpsimd.tensor_mul(t1[:], xs[tau][:], bdst[:, :FD])
                    nc.vector.tensor_sub(hp[t][:], t1[:], bdst[:, FD:])
                return go

            # DMA for apply j runs 2 queue slots ahead of its compute
            q = []
            for tau in range(ntl):
                q.append(mk_dma(tau))
            for tau in range(ntl):
                q.insert(2 * tau + 2 if 2 * tau + 2 < len(q) else len(q),
                         mk_cmp(tau))
            # interleave: after position-building above, q has dma j at
            # slot ~2j and cmp j at slot ~2j+2
            pending.extend(q)
        while pending:
            pending.pop(0)()

    # ============================================================ Phase 0
    h2 = wpool.tile([P, NB_SEQ * LSH], F16, tag="h2")
    xin_f = xin_d.rearrange("c s l -> c (s l)")
    for j in range(10):
        xst = spool.tile([CIN, FD], F16, tag="sq")
        nc.sync.dma_start(xst[:], xin_f[:, j * FD:(j + 1) * FD])
        cps = ppool.tile([P, FD], F32, tag="ps")
        nc.tensor.matmul(cps[:], W["wconv"], xst[:])
        nc.scalar.activation(h2[:, j * FD:(j + 1) * FD], cps[:],
                             AF.Relu, bias=W["bconv"])
    h2q = h2[:].rearrange("p (s l) -> p s l", s=NB_SEQ)

    def gather_producer(t, carry=None):
        x_pre = xpool.tile([P, FD], F16, tag=f"xp{t % GSZ}")
        xq = _q(x_pre[:])
        eng = nc.vector
        for g in range(2):
            ij = [slot_ij(8 * t + 4 * g + q) for q in range(NQ)]
            iis = [a for a, _ in ij]
            jjs = [b for _, b in ij]
            rows = slice(g * 64, g * 64 + 64)
            if (all(iis[q] == iis[0] + q for q in range(NQ)) and
                    all(jjs[q] == jjs[0] + q for q in range(NQ))):
                eng.tensor_add(xq[rows, :, :],
                               h2q[rows, iis[0]:iis[0] + NQ, :],
                               h2q[rows, jjs[0]:jjs[0] + NQ, :])
            else:
                for q in range(NQ):
                    eng.tensor_add(xq[rows, q, :],
                                   h2q[rows, iis[q], :],
                                   h2q[rows, jjs[q], :])
        return x_pre

    ln_phase(gather_producer, sq_dve="mix", t1_dve="mix")

    # ============================================================ blocks
    for k in range(N_BLOCKS):

        # ---- row attention phase A: k/v, local partials (3-stage skew) ---
        rc = {}

        def rowA_a(t, k=k):
            k_ps = ppool.tile([P, FD], F32, tag="ps")
            nc.tensor.matmul(k_ps[:], W[f"rk{k}"], hp[t][:])
            mk = spool.tile([P, FD], F16, tag="mk")
            ek = spool.tile([P, FD], F16, tag="ek")
            nc.scalar.activation(mk[:], k_ps[:], AF.Relu, scale=-1.0,
                                 bias=W[f"rkbn{k}"])
            nc.scalar.activation(ek[:], mk[:], AF.Exp, scale=-1.0)
            rc[t] = (k_ps, ek)

        def rowA_b(t, k=k):
            k_ps, ek = rc.pop(t)
            kt = spool.tile([P, FD], F16, tag="ktil")
            nc.vector.scalar_tensor_tensor(
                kt[:], k_ps[:], W[f"rkb1{k}"], ek[:], ALU.add, ALU.max)
            nc.vector.tensor_reduce(ksum_pr[:, NQ * t:NQ * t + NQ],
                                    _q(kt[:]), mybir.AxisListType.X, ALU.add)
            kb_ps = ppool.tile([P, FD], F32, tag="ps")
            nc.tensor.matmul(kb_ps[:], W["P8"], kt[:])
            kb = spool.tile([P, FD], F16, tag="kbsb")
            nc.scalar.activation(kb[:], kb_ps[:], AF.Identity)
            v_ps = ppool.tile([P, FD], F32, tag="ps")
            nc.tensor.matmul(v_ps[:], W[f"rv{k}"], hp[t][:])
            rc[("b", t)] = (kb, v_ps)

        def rowA_c(t, k=k):
            kb, v_ps = rc.pop(("b", t))
            vw = spool.tile([P, FD], F16, tag="vw")
            for q in range(NQ):
                sl = slice(q * LSH, (q + 1) * LSH)
                nc.vector.affine_mul_reduce(
                    vw[:, sl], ktv_pr[:, NQ * t + q:NQ * t + q + 1],
                    v_ps[:, sl], kb[:, sl], 1.0, W[f"rvb{k}"])

        # AllReduce in two halves: first half overlaps rowA's tail.
        TSPLIT = 64

        def ar(lo, hi, half, k=k):
            c0, c1 = NQ * lo, NQ * hi
            n = c1 - c0
            bin_ = dpool.tile([P, 2 * n], F32, tag=f"arin{k}_{half}")
            bout_ = dpool.tile([P, 2 * n], F32, tag=f"arout{k}_{half}")
            nc.sync.dma_start(bin_[:, :n], ksum_pr[:, c0:c1])
            nc.sync.dma_start(bin_[:, n:], ktv_pr[:, c0:c1])
            nc.gpsimd.collective_compute(
                "AllReduce", ALU.add,
                replica_groups=[[0, 1, 2, 3], [4, 5, 6, 7]],
                ins=[bin_.opt()], outs=[bout_.opt()])
            nc.sync.dma_start(ksum_pr[:, c0:c1], bout_[:, :n])
            nc.sync.dma_start(ktv_pr[:, c0:c1], bout_[:, n:])
            nc.scalar.activation(ksum_h[:, c0:c1], ksum_pr[:, c0:c1],
                                 AF.Identity)
            nc.scalar.activation(ktv_h[:, c0:c1], ktv_pr[:, c0:c1],
                                 AF.Identity)

        for i in range(NT + 2):
            if i < NT:
                rowA_a(i)
            if 1 <= i <= NT:
                rowA_b(i - 1)
            if i >= 2:
                rowA_c(i - 2)
            if i == TSPLIT + 2:
                ar(0, TSPLIT, 0)
        ar(TSPLIT, NT, 1)

        # ---- row attention phase B (3-stage skew producer) ---------------
        def row_b_a(t, carry, k=k):
            q_ps = ppool.tile([P, FD], F32, tag="ps")
            nc.tensor.matmul(q_ps[:], W[f"rq{k}"], hp[t][:])
            mq = spool.tile([P, FD], F16, tag="mk")
            eq = spool.tile([P, FD], F16, tag="ek")
            qb1 = spool.tile([P, FD], F16, tag="kbsb")
            nc.scalar.activation(mq[:], q_ps[:], AF.Relu, scale=-1.0,
                                 bias=W[f"rqbn{k}"])
            nc.scalar.activation(eq[:], mq[:], AF.Exp, scale=-1.0)
            nc.scalar.activation(qb1[:], q_ps[:], AF.Identity,
                                 bias=W[f"rqb1{k}"])
            carry[t] = (eq, qb1)

        def row_b_b(t, carry, k=k):
            eq, qb1 = carry.pop(t)
            qt = spool.tile([P, FD], F16, tag="ktil")
            nc.vector.tensor_max(qt[:], qb1[:], eq[:])
            prod = spool.tile([P, FD], F16, tag="vw")
            nc.vector.tensor_tensor(_q(prod[:]), _q(qt[:]),
                                    _bq(ksum_h[:, NQ * t:NQ * t + NQ]),
                                    ALU.mult)
            dn_ps = ppool.tile([P, FD], F32, tag="ps")
            nc.tensor.matmul(dn_ps[:], W["P8"], prod[:])
            carry[("b", t)] = dn_ps

        def row_b_c(t, carry, k=k):
            dn_ps = carry.pop(("b", t))
            z = spool.tile([P, FD], F16, tag="z")
            nc.vector.reciprocal(z[:], dn_ps[:])
            V = spool.tile([P, FD], F16, tag="V")
            nc.vector.tensor_tensor(_q(V[:]), _q(z[:]),
                                    _bq(ktv_h[:, NQ * t:NQ * t + NQ]),
                                    ALU.mult)
            att_ps = ppool.tile([P, FD], F32, tag="ps")
            nc.tensor.matmul(att_ps[:], W[f"rpbrow{k}"], ones_row[:],
                             start=True, stop=False)
            nc.tensor.matmul(att_ps[:], W[f"rp{k}"], V[:],
                             start=False, stop=True)
            carry[("c", t)] = att_ps

        def row_b_d(t, carry, k=k):
            att_ps = carry.pop(("c", t))
            x_pre = xpool.tile([P, FD], F16, tag=f"xp{t % GSZ}")
            nc.vector.scalar_tensor_tensor(
                x_pre[:], hp[t][:], W[f"rgcol{k}"], att_ps[:],
                ALU.mult, ALU.add)
            return x_pre

        ln_phase([row_b_a, row_b_b, row_b_c, row_b_d],
                 sq_dve="mix", t1_dve="mix")

        # ---- column attention phase A: k/v + local pair reduction --------
        kc_ps = ppers.tile([64, FD], F32, tag="acc1")
        tv_ps = ppers.tile([64, FD], F32, tag="acc2")
        cc = {}

        def colA_a(t, k=k):
            ck_ps = ppool.tile([P, FD], F32, tag="ps")
            nc.tensor.matmul(ck_ps[:], W[f"ck{k}"], hp[t][:])
            mk = spool.tile([P, FD], F16, tag="mk")
            ek = spool.tile([P, FD], F16, tag="ek")
            nc.scalar.activation(mk[:], ck_ps[:], AF.Relu, scale=-1.0,
                                 bias=W[f"ckbn{k}"])
            nc.scalar.activation(ek[:], mk[:], AF.Exp, scale=-1.0)
            cc[t] = (ck_ps, ek)

        def colA_b(t, k=k):
            ck_ps, ek = cc.pop(t)
            h64 = W["H64_last"] if t == NT - 1 else W["H64"]
            kt = spool.tile([P, FD], F16, tag="ktil")
            nc.vector.scalar_tensor_tensor(
                kt[:], ck_ps[:], W[f"ckb1{k}"], ek[:], ALU.add, ALU.max)
            kb_ps = ppool.tile([P, FD], F32, tag="ps")
            nc.tensor.matmul(kb_ps[:], W["P8"], kt[:])
            kb = spool.tile([P, FD], F16, tag="kbsb")
            if t % 2 == 0:
                nc.scalar.activation(kb[:], kb_ps[:], AF.Identity)
            else:
                nc.vector.tensor_copy(kb[:], kb_ps[:])
            nc.tensor.matmul(kc_ps[:], h64[:], kt[:],
                             start=(t == 0), stop=(t == NT - 1))
            cc[("b", t)] = kb

        def colA_c(t, k=k):
            kb = cc.pop(("b", t))
            cv_ps = ppool.tile([P, FD], F32, tag="ps")
            nc.tensor.matmul(cv_ps[:], W[f"cv{k}"], hp[t][:])
            cc[("c", t)] = (kb, cv_ps)

        def colA_d(t, k=k):
            kb, cv_ps = cc.pop(("c", t))
            h64 = W["H64_last"] if t == NT - 1 else W["H64"]
            vw = spool.tile([P, FD], F16, tag="vw")
            nc.vector.scalar_tensor_tensor(
                vw[:], cv_ps[:], W[f"cvb{k}"], kb[:], ALU.add, ALU.mult)
            nc.tensor.matmul(tv_ps[:], h64[:], vw[:],
                             start=(t == 0), stop=(t == NT - 1))

        for i in range(NT + 3):
            if i < NT:
                colA_a(i)
            if 1 <= i <= NT:
                colA_b(i - 1)
            if 2 <= i <= NT + 1:
                colA_c(i - 2)
            if i >= 3:
                colA_d(i - 3)
        kcs_sb = gpool.tile([64, FD], F32, tag="ln_mu")
        tvs_sb = gpool.tile([64, FD], F32, tag="ln_e2")
        nc.vector.tensor_copy(kcs_sb[:], kc_ps[:])
        nc.vector.tensor_copy(tvs_sb[:], tv_ps[:])
        ksc = gpool.tile([64, LSH], F16, tag="ksc")
        tvc = gpool.tile([64, LSH], F16, tag="tvc")
        fo1 = gpool.tile([64, LSH], F16, tag="fold1")
        fo2 = gpool.tile([64, LSH], F16, tag="fold2")
        kq, tq = _q(kcs_sb[:]), _q(tvs_sb[:])
        nc.vector.tensor_add(fo1[:], kq[:, 0, :], kq[:, 1, :])
        nc.vector.tensor_add(ksc[:], kq[:, 2, :], kq[:, 3, :])
        nc.vector.tensor_add(ksc[:], fo1[:], ksc[:])
        nc.gpsimd.tensor_add(fo2[:], tq[:, 0, :], tq[:, 1, :])
        nc.gpsimd.tensor_add(tvc[:], tq[:, 2, :], tq[:, 3, :])
        nc.gpsimd.tensor_add(tvc[:], fo2[:], tvc[:])
        kcb_ps = ppool.tile([P, FD], F32, tag="ps")
        nc.tensor.matmul(kcb_ps[:, :LSH], W["H64T"], ksc[:])
        nc.vector.tensor_copy(kc_b[:], kcb_ps[:, :LSH])
        tcb_ps = ppool.tile([P, FD], F32, tag="ps")
        nc.tensor.matmul(tcb_ps[:, :LSH], W["H64T"], tvc[:])
        nc.vector.tensor_copy(tc_b[:], tcb_ps[:, :LSH])

        # ---- column attention phase B (3-stage skew producer) ------------
        def col_b_a(t, carry, k=k):
            q_ps = ppool.tile([P, FD], F32, tag="ps")
            nc.tensor.matmul(q_ps[:], W[f"cq{k}"], hp[t][:])
            mq = spool.tile([P, FD], F16, tag="mk")
            eq = spool.tile([P, FD], F16, tag="ek")
            qb1 = spool.tile([P, FD], F16, tag="kbsb")
            nc.scalar.activation(mq[:], q_ps[:], AF.Relu, scale=-1.0,
                                 bias=W[f"cqbn{k}"])
            nc.scalar.activation(eq[:], mq[:], AF.Exp, scale=-1.0)
            nc.scalar.activation(qb1[:], q_ps[:], AF.Identity,
                                 bias=W[f"cqb1{k}"])
            carry[t] = (eq, qb1)

        def col_b_b(t, carry, k=k):
            eq, qb1 = carry.pop(t)
            qt = spool.tile([P, FD], F16, tag="ktil")
            nc.vector.tensor_max(qt[:], qb1[:], eq[:])
            prod = spool.tile([P, FD], F16, tag="vw")
            nc.vector.tensor_tensor(_q(prod[:]), _q(qt[:]), _bl(kc_b[:]),
                                    ALU.mult)
            dn_ps = ppool.tile([P, FD], F32, tag="ps")
            nc.tensor.matmul(dn_ps[:], W["P8"], prod[:])
            carry[("b", t)] = dn_ps

        def col_b_c(t, carry, k=k):
            dn_ps = carry.pop(("b", t))
            z = spool.tile([P, FD], F16, tag="z")
            nc.vector.reciprocal(z[:], dn_ps[:])
            V = spool.tile([P, FD], F16, tag="V")
            nc.vector.tensor_tensor(_q(V[:]), _q(z[:]), _bl(tc_b[:]),
                                    ALU.mult)
            att_ps = ppool.tile([P, FD], F32, tag="ps")
            nc.tensor.matmul(att_ps[:], W[f"cpbrow{k}"], ones_row[:],
                             start=True, stop=False)
            nc.tensor.matmul(att_ps[:], W[f"cp{k}"], V[:],
                             start=False, stop=True)
            carry[("c", t)] = att_ps

        def col_b_d(t, carry, k=k):
            att_ps = carry.pop(("c", t))
            x_pre = xpool.tile([P, FD], F16, tag=f"xp{t % GSZ}")
            nc.vector.scalar_tensor_tensor(
                x_pre[:], hp[t][:], W[f"cgcol{k}"], att_ps[:],
                ALU.mult, ALU.add)
            return x_pre

        ln_phase([col_b_a, col_b_b, col_b_c, col_b_d],
                 sq_dve="mix", t1_dve="mix")

        # ---- FFN ----------------------------------------------------------
        def ffn(t, carry=None, k=k, to_hp=False):
            o_ps = ppool.tile([P, FD], F32, tag="ps")
            nc.tensor.matmul(o_ps[:], W[f"f2brow{k}"], ones_row[:],
                             start=True, stop=False)
            for j in range(4):
                h_ps = ppool.tile([P, FD], F32, tag="ps")
                nc.tensor.matmul(h_ps[:], W[f"f1_{k}_{j}"], hp[t][:])
                hid = spool.tile([P, FD], F16, tag="V")
                nc.scalar.activation(hid[:], h_ps[:], AF.Gelu,
                                     bias=W[f"f1b_{k}_{j}"])
                nc.tensor.matmul(o_ps[:], W[f"f2_{k}_{j}"], hid[:],
                                 start=False, stop=(j == 3))
            if to_hp:
                nc.vector.scalar_tensor_tensor(
                    hp[t][:], hp[t][:], W[f"fgcol{k}"], o_ps[:],
                    ALU.mult, ALU.add)
                return None
            x_pre = xpool.tile([P, FD], F16, tag=f"xp{t % GSZ}")
            nc.vector.scalar_tensor_tensor(
                x_pre[:], hp[t][:], W[f"fgcol{k}"], o_ps[:],
                ALU.mult, ALU.add)
            return x_pre

        if k != N_BLOCKS - 1:
            ln_phase(ffn, sq_dve=True, t1_dve=True)
        else:
            for t in range(NT):
                ffn(t, to_hp=True)

    # ============================================================ output
    ystage = wpool.tile([32, 4 * NOG], F32, tag="ystage")
    for gi in range(NOG):
        t0, tend = gi * OGSZ, min(NT, gi * OGSZ + OGSZ)
        ntl = tend - t0
        o_ps = ppers.tile([64, FD], F32, tag="acc1")
        for tau in range(ntl):
            nc.tensor.matmul(o_ps[:32, :],
                             W["outw_lt"][:, tau * 32:(tau + 1) * 32],
                             hp[t0 + tau][:],
                             start=(tau == 0), stop=(tau == ntl - 1))
        ab = gpool.tile([64, FD], F32, tag="ln_mu")
        l1 = gpool.tile([64, FD], F32, tag="ln_e2")
        rl = gpool.tile([64, FD], F32, tag="ln_msq")
        nc.scalar.activation(ab[:32, :], o_ps[:32, :], AF.Abs, bias=W["boutc"])
        nc.scalar.activation(ab[:32, :], ab[:32, :], AF.Exp, scale=-1.0)
        nc.scalar.activation(l1[:32, :], ab[:32, :], AF.Ln, bias=W["onec"])
        nc.scalar.activation(rl[:32, :], o_ps[:32, :], AF.Relu, bias=W["boutc"])
        nc.vector.tensor_add(l1[:32, :], l1[:32, :], rl[:32, :])
        nc.vector.tensor_reduce(
            ystage[:, 4 * gi:4 * gi + 4],
            l1[:32, :].rearrange("p (q l) -> p q l", q=NQ),
            mybir.AxisListType.X, ALU.add)
    nc.sync.dma_start(yout_d[:], ystage[:])
    ctx.close()


# ================================================================ host API
_NC_CACHE = {}


def _get_nc():
    if "nc" not in _NC_CACHE:
        _NC_CACHE["nc"] = build_kernel()
    return _NC_CACHE["nc"]


def kernel(**inputs):
    from concourse.bass_utils import run_bass_kernel_spmd

    nc = _get_nc()
    w = prep_weights(inputs)

    x = np.asarray(inputs["x"])
    in_maps = []
    for core in range(N_CORES):
        b, lq = core // 4, core % 4
        xs = x[b, :, lq * LSH:(lq + 1) * LSH, :]
        xs = np.ascontiguousarray(np.transpose(xs, (0, 2, 1)),
                                  dtype=np.float16)
        m = {"xin": xs, "wpack16": w["wpack16"], "wpack32": w["wpack32"]}
        in_maps.append(m)

    res = run_bass_kernel_spmd(nc, in_maps, core_ids=list(range(N_CORES)))
    outs = [r["yout"] for r in res.results]

    y = np.zeros((B, NB_PAIRS), np.float64)
    for core in range(N_CORES):
        b = core // 4
        st = outs[core].astype(np.float64)
        for gi in range(NOG):
            for tau in range(min(OGSZ, NT - gi * OGSZ)):
                t = gi * OGSZ + tau
                for g in range(2):
                    for q in range(NQ):
                        s = 8 * t + 4 * g + q
                        if s < NB_PAIRS:
                            y[b, s] += st[2 * tau + g, 4 * gi + q]
    y /= SEQ_LEN

    out = np.zeros((B, NB_PAIRS), np.float32)
    ii, jj = np.triu_indices(NB_SEQ, 1)
    tri = {(a, c): p for p, (a, c) in enumerate(zip(ii, jj))}
    for s, (a, c) in enumerate(PAIRS):
        out[:, tri[(a, c)]] = y[:, s]
    return out


# revision 26
# speedup vs baseline: 1.0463x; 1.0057x over previous
"""Trainium2 Bass kernel for nn_AttentionNet (axial linear-attention net).

Sharding: cores 0-3 hold batch b=0, cores 4-7 hold b=1. Within a 4-core
group the sequence axis L=512 is split into 4 shards of 128. Every core
holds ALL 780 pairs for its (b, l-shard), so the instruction stream is
identical on all cores (pure SPMD) and only the input data differs.

Residual state per core: 98 SBUF tiles [128, 512] fp16:
  partition = g*64 + n*16 + d   (g = pair-half 0/1, n = head, d = head ch)
  free      = q*128 + l         (q = pair-quad 0..3, l = local seq pos)
  tile t holds pair slots 8t + 4g + q (784 slots = 780 pairs + 4 pads).

v2 design notes:
- LayerNorm affine (gamma/beta) is folded host-side into every consumer
  matmul; the residual stream stores the UN-affined normalized value
  (h-tilde) and residual adds re-apply gamma via the stt scalar slot and
  beta via a 1-partition bias matmul accumulated into the attention/FFN
  output PSUM.
- LN apply uses a DMA partition-broadcast of [rstd | mean*rstd] rows to
  128 partitions, then two 2x-mode f16 TensorTensor ops. No per-tile
  apply matmuls, no PSUM reads on the apply path.
- LN statistics are computed in groups of 32 tiles (one-hot stat matmuls
  into two persistent PSUM banks).
- Row attention uses affine_mul_reduce to fuse (v+bias)*kbar with the
  per-quad KtV reduction; PSUM->SBUF drains ride the Activation engine.
- Engine balance: DVE keeps the PSUM-coupled ops, Act does elu/gelu
  chains + drains, Pool (gpsimd) takes pure-SBUF squares/multiplies,
  the DMA engines do the LN broadcasts.
"""

import contextlib
import sys

import numpy as np

sys.path.insert(0, "/opt/trn_rl_repo")

mybir = None
F32 = F16 = AF = ALU = None


def _lazy_imports():
    global mybir, F32, F16, AF, ALU
    if mybir is None:
        import concourse.mybir as _mybir
        mybir = _mybir
        F32, F16 = mybir.dt.float32, mybir.dt.float16
        AF = mybir.ActivationFunctionType
        ALU = mybir.AluOpType

NB_SEQ = 40
SEQ_LEN = 512
NB_PAIRS = 780
B = 2
N_BLOCKS = 2
CIN = 22

N_CORES = 8
LSH = 128            # l per core
NQ = 4               # quads per tile
NT = 98              # hp tiles per core
GSZ = 32             # LN group size (tiles)
NGROUP = (NT + GSZ - 1) // GSZ       # 4 (32,32,32,2)
OGSZ = 16
NOG = (NT + OGSZ - 1) // OGSZ        # 7, output stage groups
FD = NQ * LSH        # 512, tile free size


def _pair_order():
    order = []
    for d in range(1, NB_SEQ):
        for i in range(NB_SEQ - d):
            order.append((i, i + d))
    return order


PAIRS = _pair_order()


def slot_ij(s):
    return PAIRS[s] if s < NB_PAIRS else PAIRS[0]


# ================================================================ weights
def prep_weights(inp):
    """Pack all constants; LN affine folded into consumer weights."""
    w = {}
    f16 = lambda a: np.ascontiguousarray(a, dtype=np.float16)
    f32 = lambda a: np.ascontiguousarray(a, dtype=np.float32)

    def col(v, n=128):
        v = np.asarray(v, np.float32).reshape(-1)
        if v.size == 64:
            v = np.tile(v, 2)
        if v.size == 1:
            v = np.full(n, v[0], np.float32)
        return f32(v.reshape(n, 1))

    w_in = np.asarray(inp["w_in"])
    w["wconv"] = f16(np.concatenate([w_in.T, w_in.T], axis=1))
    w["bconv"] = col(inp["b_in"])

    # LN instances: 0 = (g0, be0); k+1 = block-k (ln_g[k], ln_b[k]).
    lns = [(np.asarray(inp["g0"], np.float32),
            np.asarray(inp["be0"], np.float32))]
    for k in range(N_BLOCKS):
        lns.append((np.asarray(inp["ln_g"][k], np.float32),
                    np.asarray(inp["ln_b"][k], np.float32)))

    def bd(m, gamma):
        # block-diag lhsT with gamma folded into input columns
        mt = (np.asarray(m, np.float32) * gamma[None, :]).T
        z = np.zeros((128, 128), np.float16)
        z[:64, :64] = mt
        z[64:, 64:] = mt
        return z

    def fold_bias(m, beta, b):
        return np.asarray(m, np.float32) @ beta + np.asarray(b, np.float32)

    # LN instance feeding each phase's input:
    #  rowA/rowB(k): instance k (ln0 for k=0, ffn-LN of k-1 else)
    #  colA/colB(k): instance k+1 (row-LN of block k)
    #  ffn(k):       instance k+1 (col-LN of block k)
    for k in range(N_BLOCKS):
        gr, br_ = lns[k]        # feeds row attention
        gc, bc_ = lns[k + 1]    # feeds col attention and ffn
        for nm, wk, bk, g_, b_ in [
                ("rq", "rqw", "rqb", gr, br_), ("rk", "rkw", "rkb", gr, br_),
                ("rv", "rvw", "rvb", gr, br_), ("cq", "cqw", "cqb", gc, bc_),
                ("ck", "ckw", "ckb", gc, bc_), ("cv", "cvw", "cvb", gc, bc_)]:
            m = np.asarray(inp[wk][k])
            w[f"{nm}{k}"] = f16(bd(m, g_))
            bb = fold_bias(m, b_, inp[bk][k])
            w[f"{nm}b{k}"] = col(bb)
            w[f"{nm}b1{k}"] = col(bb + 1.0)
            w[f"{nm}bn{k}"] = col(-bb)
        # output projections: plain weights; bias handled by bias-row MM
        for nm, wk in [("rp", "rpw"), ("cp", "cpw")]:
            m = np.asarray(inp[wk][k])
            z = np.zeros((128, 128), np.float16)
            z[:64, :64] = m.T
            z[64:, 64:] = m.T
            w[f"{nm}{k}"] = f16(z)
        # residual bias rows: rp-bias + beta of the residual LN instance
        rpb = np.asarray(inp["rpb"][k], np.float32)
        cpb = np.asarray(inp["cpb"][k], np.float32)
        w[f"rpbrow{k}"] = f16(np.tile(rpb + br_, 2).reshape(1, 128))
        w[f"cpbrow{k}"] = f16(np.tile(cpb + bc_, 2).reshape(1, 128))
        # residual gamma cols
        w[f"rgcol{k}"] = col(gr)
        w[f"cgcol{k}"] = col(gc)
        w[f"fgcol{k}"] = col(gc)

        f1w = np.asarray(inp["f1w"][k], np.float32) * gc[None, :]
        f1b = np.asarray(inp["f1w"][k], np.float32) @ bc_ \
            + np.asarray(inp["f1b"][k], np.float32)
        f2w = np.asarray(inp["f2w"][k])
        for j in range(4):
            g, hh = j // 2, (j % 2) * 128
            lt = np.zeros((128, 128), np.float16)
            lt[g * 64:(g + 1) * 64, :] = f1w[hh:hh + 128, :].T
            w[f"f1_{k}_{j}"] = f16(lt)
            lt2 = np.zeros((128, 128), np.float16)
            lt2[:, g * 64:(g + 1) * 64] = f2w[:, hh:hh + 128].T
            w[f"f2_{k}_{j}"] = f16(lt2)
            w[f"f1b_{k}_{j}"] = f32(f1b[hh:hh + 128].reshape(128, 1))
        f2b = np.asarray(inp["f2b"][k], np.float32)
        w[f"f2brow{k}"] = f16(np.tile(f2b + bc_, 2).reshape(1, 128))
        # last block's ffn has no LN after; residual gamma is col instance
        # (gc) because hp there is the col-LN output of block k.

    # conv + gather feed ln0; gather output is raw (pre-LN) so the rowA
    # weights above already fold ln0 -> nothing extra here.

    # stat slabs for GSZ-tile groups: per tau [128, 64]:
    # rows 2*tau+g get ones at partitions g*64..g*64+64
    stat = np.zeros((128, GSZ * 64), np.float16)
    for tau in range(GSZ):
        for g in range(2):
            stat[g * 64:(g + 1) * 64, tau * 64 + 2 * tau + g] = 1.0
    w["stat_lt"] = f16(stat)

    # output stage (16-tile groups)
    outw = np.zeros((128, OGSZ * 32), np.float16)
    wo = np.asarray(inp["wout"], np.float32).reshape(-1)
    for tau in range(OGSZ):
        for g in range(2):
            outw[g * 64:(g + 1) * 64, tau * 32 + 2 * tau + g] = wo
    w["outw_lt"] = f16(outw)
    w["boutc"] = f32(np.full((32, 1), np.asarray(inp["bout"]).reshape(-1)[0],
                             np.float32))
    w["epsc"] = f32(np.full((64, 1), 1e-5, np.float32))
    w["c64"] = f32(np.full((64, 1), 1.0 / 64, np.float32))
    w["onec"] = f32(np.full((32, 1), 1.0, np.float32))

    p8 = np.zeros((128, 128), np.float16)
    for blk in range(8):
        p8[blk * 16:(blk + 1) * 16, blk * 16:(blk + 1) * 16] = 1.0
    w["P8"] = f16(p8)
    h64 = np.zeros((128, 64), np.float16)
    h64[np.arange(128), np.arange(128) % 64] = 1.0
    w["H64"] = f16(h64)
    hlast = h64.copy()
    hlast[64:, :] = 0.0
    w["H64_last"] = f16(hlast)
    h64t = np.zeros((64, 128), np.float16)
    h64t[np.arange(128) % 64, np.arange(128)] = 1.0
    w["H64T"] = f16(h64t)

    # ---- pack into two tensors ----
    s16, s32 = _pack_layout()
    p16 = np.zeros((128, s16[-1][2] + s16[-1][3]), np.float16)
    for name, rows, off, cols in s16:
        p16[:rows, off:off + cols] = w[name]
    p32 = np.zeros((128, s32[-1][2] + s32[-1][3]), np.float32)
    for name, rows, off, cols in s32:
        p32[:rows, off:off + cols] = w[name]
    w["wpack16"] = p16
    w["wpack32"] = p32
    return w


def _pack_layout():
    e16, e32 = [], []
    o16 = o32 = 0

    def a16(name, rows, cols):
        nonlocal o16
        e16.append((name, rows, o16, cols))
        o16 += cols

    def a32(name, rows, cols):
        nonlocal o32
        e32.append((name, rows, o32, cols))
        o32 += cols

    a16("wconv", CIN, 128)
    a32("bconv", 128, 1)
    for k in range(N_BLOCKS):
        for nm in ["rq", "rk", "rv", "cq", "ck", "cv"]:
            a16(f"{nm}{k}", 128, 128)
            a32(f"{nm}b{k}", 128, 1)
            a32(f"{nm}b1{k}", 128, 1)
            a32(f"{nm}bn{k}", 128, 1)
        for nm in ["rp", "cp"]:
            a16(f"{nm}{k}", 128, 128)
        a16(f"rpbrow{k}", 1, 128)
        a16(f"cpbrow{k}", 1, 128)
        a32(f"rgcol{k}", 128, 1)
        a32(f"cgcol{k}", 128, 1)
        a32(f"fgcol{k}", 128, 1)
        for j in range(4):
            a16(f"f1_{k}_{j}", 128, 128)
            a16(f"f2_{k}_{j}", 128, 128)
            a32(f"f1b_{k}_{j}", 128, 1)
        a16(f"f2brow{k}", 1, 128)
    a16("stat_lt", 128, GSZ * 64)
    a16("outw_lt", 128, OGSZ * 32)
    a32("boutc", 32, 1)
    a32("epsc", 64, 1)
    a32("c64", 64, 1)
    a32("onec", 32, 1)
    a16("P8", 128, 128)
    a16("H64", 128, 64)
    a16("H64_last", 128, 64)
    a16("H64T", 64, 128)
    return e16, e32


WEIGHT_SPECS = []


def _spec():
    e16, e32 = _pack_layout()
    n16 = e16[-1][2] + e16[-1][3]
    n32 = e32[-1][2] + e32[-1][3]
    return [("wpack16", (128, n16), F16), ("wpack32", (128, n32), F32)]


# ================================================================ views
def _q(ap):
    return ap.rearrange("p (q l) -> p q l", q=NQ)


def _bq(ap_col4):
    """[128, 4] slice -> [128, 4, 128] broadcast over l."""
    a = ap_col4.copy()
    a.ap = a.ap[:-1] + [list(a.ap[-1]), [0, LSH]]
    return a


def _bl(ap_l):
    """[128, 128] -> [128, 4, 128] broadcast over quads."""
    a = ap_l.copy()
    a.ap = a.ap[:-1] + [[0, NQ], list(a.ap[-1])]
    return a


def _brows(ap_2rows):
    """[2, F] rows -> broadcast to [2, 64, F] (DMA source: each row
    repeated 64x so the dest covers 128 partitions)."""
    a = ap_2rows.copy()
    a.ap = a.ap[:1] + [[0, 64], list(a.ap[-1])]
    return a


# ================================================================ kernel IR
def build_kernel():
    _lazy_imports()
    import concourse.bacc as bacc
    import concourse.tile as tile

    global WEIGHT_SPECS
    WEIGHT_SPECS = _spec()

    nc = bacc.Bacc("TRN2", target_bir_lowering=False, debug=False,
                   num_devices=N_CORES)
    xin_d = nc.dram_tensor("xin", [CIN, NB_SEQ, LSH], F16,
                           kind="ExternalInput").ap()
    wd = {}
    for name, shape, dtype in WEIGHT_SPECS:
        wd[name] = nc.dram_tensor(name, list(shape), dtype,
                                  kind="ExternalInput").ap()
    yout_d = nc.dram_tensor("yout", [32, 4 * NOG], F32,
                            kind="ExternalOutput").ap()

    with tile.TileContext(nc) as tc:
        _body(nc, tc, xin_d, wd, yout_d)
    nc.compile()
    return nc


def _body(nc, tc, xin_d, wd, yout_d):
    ctx = contextlib.ExitStack()
    ctx.enter_context(nc.allow_low_precision(
        reason="normalized activations; f16 everywhere is plenty for 2e-2"))
    P = 128

    wpool = ctx.enter_context(tc.tile_pool(name="w", bufs=1))
    hpool = ctx.enter_context(tc.tile_pool(name="hp", bufs=1))
    spool = ctx.enter_context(tc.tile_pool(name="scr", bufs=3))
    xpool = ctx.enter_context(tc.tile_pool(name="xpre", bufs=1))
    gpool = ctx.enter_context(tc.tile_pool(name="grp", bufs=1))
    stpool = ctx.enter_context(tc.tile_pool(name="st2", bufs=2))
    bpool = ctx.enter_context(tc.tile_pool(name="bc", bufs=3))
    ppool = ctx.enter_context(tc.tile_pool(name="ps", bufs=6, space="PSUM"))
    ppers = ctx.enter_context(tc.tile_pool(name="ps2", bufs=1, space="PSUM"))
    dpool = ctx.enter_context(tc.tile_pool(name="dram", bufs=1, space="DRAM"))

    e16, e32 = _pack_layout()
    n16 = e16[-1][2] + e16[-1][3]
    n32 = e32[-1][2] + e32[-1][3]
    pk16 = wpool.tile([128, n16], F16, tag="pk16", name="pk16")
    pk32 = wpool.tile([128, n32], F32, tag="pk32", name="pk32")
    nc.sync.dma_start(pk16[:], wd["wpack16"][:])
    nc.sync.dma_start(pk32[:], wd["wpack32"][:])
    W = {}
    for name, rows, off, cols in e16:
        W[name] = pk16[:rows, off:off + cols]
    for name, rows, off, cols in e32:
        W[name] = pk32[:rows, off:off + cols]

    hp = [hpool.tile([P, FD], F16, tag=f"hp{t}", name=f"hp{t}")
          for t in range(NT)]

    ksum_pr = wpool.tile([P, NQ * NT], F32, tag="ksum_pr")
    ktv_pr = wpool.tile([P, NQ * NT], F32, tag="ktv_pr")
    ksum_h = wpool.tile([P, NQ * NT], F16, tag="ksum_h")
    ktv_h = wpool.tile([P, NQ * NT], F16, tag="ktv_h")
    kc_b = wpool.tile([P, LSH], F16, tag="kc_b")
    tc_b = wpool.tile([P, LSH], F16, tag="tc_b")
    ones_row = wpool.tile([1, FD], F16, tag="ones_row")
    nc.vector.memset(ones_row[:], 1.0)

    # ============================================================ LN
    # producer(t) -> x_pre tile (REAL post-residual values).
    # sq_dve: engine for the square (True=DVE, False=Pool)
    # t1_dve: engine for the first apply multiply
    def ln_phase(producer, sq_dve=False, t1_dve=False):
        stages = producer if isinstance(producer, (list, tuple)) \
            else [producer]
        ns = len(stages)
        pending = []
        for gi in range(NGROUP):
            t0, tend = gi * GSZ, min(NT, gi * GSZ + GSZ)
            ntl = tend - t0
            s_ps = ppers.tile([64, FD], F32, tag="acc1")
            sq_ps = ppers.tile([64, FD], F32, tag="acc2")
            xs = [None] * ntl

            def finish(tau, x_pre, ntl=ntl, s_ps=s_ps, sq_ps=sq_ps, xs=xs,
                       t0=t0):
                xs[tau] = x_pre
                sq = spool.tile([P, FD], F16, tag="sq")
                use_dve = sq_dve if isinstance(sq_dve, bool) \
                    else (tau % 3 == 0)
                if use_dve:
                    nc.vector.tensor_mul(sq[:], x_pre[:], x_pre[:])
                else:
                    nc.gpsimd.tensor_mul(sq[:], x_pre[:], x_pre[:])
                sl = W["stat_lt"][:, tau * 64:tau * 64 + 64]
                nc.tensor.matmul(s_ps[:], sl, x_pre[:],
                                 start=(tau == 0), stop=(tau == ntl - 1))
                nc.tensor.matmul(sq_ps[:], sl, sq[:],
                                 start=(tau == 0), stop=(tau == ntl - 1))

            carry = {}
            for i in range(ntl + ns - 1):
                for si, f in enumerate(stages):
                    tau = i - si
                    if 0 <= tau < ntl:
                        r = f(t0 + tau, carry)
                        if si == ns - 1:
                            finish(tau, r)
                if pending:
                    pending.pop(0)()
                if pending:
                    pending.pop(0)()
            mu = gpool.tile([64, FD], F32, tag="ln_mu")
            e2 = gpool.tile([64, FD], F32, tag="ln_e2")
            msq = gpool.tile([64, FD], F32, tag="ln_msq")
            st2 = stpool.tile([64, 2 * FD], F16, tag="ln_st2")
            nc.scalar.activation(mu[:], s_ps[:], AF.Copy, scale=1.0 / 64)
            nc.vector.tensor_mul(msq[:], mu[:], mu[:])
            nc.vector.scalar_tensor_tensor(
                e2[:], sq_ps[:], W["c64"], msq[:], ALU.mult, ALU.subtract)
            nc.scalar.activation(msq[:], e2[:], AF.Ln, bias=W["epsc"])
            nc.scalar.activation(st2[:, :FD], msq[:], AF.Exp, scale=-0.5)
            nc.gpsimd.tensor_mul(st2[:, FD:], mu[:], st2[:, :FD])
            while pending:
                pending.pop(0)()

            bds = {}

            def mk_dma(tau, st2=st2, bds=bds):
                def go():
                    bdst = bpool.tile([P, 2 * FD], F16, tag="bdst")
                    nc.sync.dma_start(bdst[:],
                                      _brows(st2[2 * tau:2 * tau + 2, :]))
                    bds[tau] = bdst
                return go

            def mk_cmp(tau, t0=t0, xs=xs, bds=bds):
                def go():
                    t = t0 + tau
                    bdst = bds.pop(tau)
                    t1 = spool.tile([P, FD], F16, tag="t1")
                    use_dve = t1_dve if isinstance(t1_dve, bool) \
                        else (tau % 2 == 0)
                    if use_dve:
                        nc.vector.tensor_mul(t1[:], xs[tau][:], bdst[:, :FD])
                    else:
                        nc.gpsimd.tensor_mul(t1[:], xs[tau][:], bdst[:, :FD])
                    nc.vector.tensor_sub(hp[t][:], t1[:], bdst[:, FD:])
                return go

            # DMA for apply j runs 2 queue slots ahead of its compute
            q = []
            for tau in range(ntl):
                q.append(mk_dma(tau))
            for tau in range(ntl):
                q.insert(2 * tau + 2 if 2 * tau + 2 < len(q) else len(q),
                         mk_cmp(tau))
            # interleave: after position-building above, q has dma j at
            # slot ~2j and cmp j at slot ~2j+2
            pending.extend(q)
        while pending:
            pending.pop(0)()

    # ============================================================ Phase 0
    h2 = wpool.tile([P, NB_SEQ * LSH], F16, tag="h2")
    xin_f = xin_d.rearrange("c s l -> c (s l)")
    for j in range(10):
        xst = spool.tile([CIN, FD], F16, tag="sq")
        nc.sync.dma_start(xst[:], xin_f[:, j * FD:(j + 1) * FD])
        cps = ppool.tile([P, FD], F32, tag="ps")
        nc.tensor.matmul(cps[:], W["wconv"], xst[:])
        nc.scalar.activation(h2[:, j * FD:(j + 1) * FD], cps[:],
                             AF.Relu, bias=W["bconv"])
    h2q = h2[:].rearrange("p (s l) -> p s l", s=NB_SEQ)

    def gather_producer(t, carry=None):
        x_pre = xpool.tile([P, FD], F16, tag=f"xp{t % GSZ}")
        xq = _q(x_pre[:])
        eng = nc.vector
        for g in range(2):
            ij = [slot_ij(8 * t + 4 * g + q) for q in range(NQ)]
            iis = [a for a, _ in ij]
            jjs = [b for _, b in ij]
            rows = slice(g * 64, g * 64 + 64)
            if (all(iis[q] == iis[0] + q for q in range(NQ)) and
                    all(jjs[q] == jjs[0] + q for q in range(NQ))):
                eng.tensor_add(xq[rows, :, :],
                               h2q[rows, iis[0]:iis[0] + NQ, :],
                               h2q[rows, jjs[0]:jjs[0] + NQ, :])
            else:
                for q in range(NQ):
                    eng.tensor_add(xq[rows, q, :],
                                   h2q[rows, iis[q], :],
                                   h2q[rows, jjs[q], :])
        return x_pre

    ln_phase(gather_producer, sq_dve="mix", t1_dve="mix")

    # ============================================================ blocks
    for k in range(N_BLOCKS):

        # ---- row attention phase A: k/v, local partials (3-stage skew) ---
        rc = {}

        def rowA_a(t, k=k):
            k_ps = ppool.tile([P, FD], F32, tag="ps")
            nc.tensor.matmul(k_ps[:], W[f"rk{k}"], hp[t][:])
            mk = spool.tile([P, FD], F16, tag="mk")
            ek = spool.tile([P, FD], F16, tag="ek")
            nc.scalar.activation(mk[:], k_ps[:], AF.Relu, scale=-1.0,
                                 bias=W[f"rkbn{k}"])
            nc.scalar.activation(ek[:], mk[:], AF.Exp, scale=-1.0)
            rc[t] = (k_ps, ek)

        def rowA_b(t, k=k):
            k_ps, ek = rc.pop(t)
            kt = spool.tile([P, FD], F16, tag="ktil")
            nc.vector.scalar_tensor_tensor(
                kt[:], k_ps[:], W[f"rkb1{k}"], ek[:], ALU.add, ALU.max)
            nc.vector.tensor_reduce(ksum_pr[:, NQ * t:NQ * t + NQ],
                                    _q(kt[:]), mybir.AxisListType.X, ALU.add)
            kb_ps = ppool.tile([P, FD], F32, tag="ps")
            nc.tensor.matmul(kb_ps[:], W["P8"], kt[:])
            kb = spool.tile([P, FD], F16, tag="kbsb")
            nc.scalar.activation(kb[:], kb_ps[:], AF.Identity)
            v_ps = ppool.tile([P, FD], F32, tag="ps")
            nc.tensor.matmul(v_ps[:], W[f"rv{k}"], hp[t][:])
            rc[("b", t)] = (kb, v_ps)

        def rowA_c(t, k=k):
            kb, v_ps = rc.pop(("b", t))
            vw = spool.tile([P, FD], F16, tag="vw")
            for q in range(NQ):
                sl = slice(q * LSH, (q + 1) * LSH)
                nc.vector.affine_mul_reduce(
                    vw[:, sl], ktv_pr[:, NQ * t + q:NQ * t + q + 1],
                    v_ps[:, sl], kb[:, sl], 1.0, W[f"rvb{k}"])

        # AllReduce in two halves: first half overlaps rowA's tail.
        TSPLIT = 64

        def ar(lo, hi, half, k=k):
            c0, c1 = NQ * lo, NQ * hi
            n = c1 - c0
            bin_ = dpool.tile([P, 2 * n], F32, tag=f"arin{k}_{half}")
            bout_ = dpool.tile([P, 2 * n], F32, tag=f"arout{k}_{half}")
            nc.sync.dma_start(bin_[:, :n], ksum_pr[:, c0:c1])
            nc.sync.dma_start(bin_[:, n:], ktv_pr[:, c0:c1])
            nc.gpsimd.collective_compute(
                "AllReduce", ALU.add,
                replica_groups=[[0, 1, 2, 3], [4, 5, 6, 7]],
                ins=[bin_.opt()], outs=[bout_.opt()])
            nc.sync.dma_start(ksum_pr[:, c0:c1], bout_[:, :n])
            nc.sync.dma_start(ktv_pr[:, c0:c1], bout_[:, n:])
            nc.scalar.activation(ksum_h[:, c0:c1], ksum_pr[:, c0:c1],
                                 AF.Identity)
            nc.scalar.activation(ktv_h[:, c0:c1], ktv_pr[:, c0:c1],
                                 AF.Identity)

        for i in range(NT + 2):
            if i < NT:
                rowA_a(i)
            if 1 <= i <= NT:
                rowA_b(i - 1)
            if i >= 2:
                rowA_c(i - 2)
            if i == TSPLIT + 2:
                ar(0, TSPLIT, 0)
        ar(TSPLIT, NT, 1)

        # ---- row attention phase B (3-stage skew producer) ---------------
        def row_b_a(t, carry, k=k):
            q_ps = ppool.tile([P, FD], F32, tag="ps")
            nc.tensor.matmul(q_ps[:], W[f"rq{k}"], hp[t][:])
            mq = spool.tile([P, FD], F16, tag="mk")
            eq = spool.tile([P, FD], F16, tag="ek")
            qb1 = spool.tile([P, FD], F16, tag="kbsb")
            nc.scalar.activation(mq[:], q_ps[:], AF.Relu, scale=-1.0,
                                 bias=W[f"rqbn{k}"])
            nc.scalar.activation(eq[:], mq[:], AF.Exp, scale=-1.0)
            nc.scalar.activation(qb1[:], q_ps[:], AF.Identity,
                                 bias=W[f"rqb1{k}"])
            carry[t] = (eq, qb1)

        def row_b_b(t, carry, k=k):
            eq, qb1 = carry.pop(t)
            qt = spool.tile([P, FD], F16, tag="ktil")
            nc.vector.tensor_max(qt[:], qb1[:], eq[:])
            prod = spool.tile([P, FD], F16, tag="vw")
            nc.vector.tensor_tensor(_q(prod[:]), _q(qt[:]),
                                    _bq(ksum_h[:, NQ * t:NQ * t + NQ]),
                                    ALU.mult)
            dn_ps = ppool.tile([P, FD], F32, tag="ps")
            nc.tensor.matmul(dn_ps[:], W["P8"], prod[:])
            carry[("b", t)] = dn_ps

        def row_b_c(t, carry, k=k):
            dn_ps = carry.pop(("b", t))
            z = spool.tile([P, FD], F16, tag="z")
            nc.vector.reciprocal(z[:], dn_ps[:])
            V = spool.tile([P, FD], F16, tag="V")
            nc.vector.tensor_tensor(_q(V[:]), _q(z[:]),
                                    _bq(ktv_h[:, NQ * t:NQ * t + NQ]),
                                    ALU.mult)
            att_ps = ppool.tile([P, FD], F32, tag="ps")
            nc.tensor.matmul(att_ps[:], W[f"rpbrow{k}"], ones_row[:],
                             start=True, stop=False)
            nc.tensor.matmul(att_ps[:], W[f"rp{k}"], V[:],
                             start=False, stop=True)
            carry[("c", t)] = att_ps

        def row_b_d(t, carry, k=k):
            att_ps = carry.pop(("c", t))
            x_pre = xpool.tile([P, FD], F16, tag=f"xp{t % GSZ}")
            nc.vector.scalar_tensor_tensor(
                x_pre[:], hp[t][:], W[f"rgcol{k}"], att_ps[:],
                ALU.mult, ALU.add)
            return x_pre

        ln_phase([row_b_a, row_b_b, row_b_c, row_b_d],
                 sq_dve="mix", t1_dve="mix")

        # ---- column attention phase A: k/v + local pair reduction --------
        kc_ps = ppers.tile([64, FD], F32, tag="acc1")
        tv_ps = ppers.tile([64, FD], F32, tag="acc2")
        cc = {}

        def colA_a(t, k=k):
            ck_ps = ppool.tile([P, FD], F32, tag="ps")
            nc.tensor.matmul(ck_ps[:], W[f"ck{k}"], hp[t][:])
            mk = spool.tile([P, FD], F16, tag="mk")
            ek = spool.tile([P, FD], F16, tag="ek")
            nc.scalar.activation(mk[:], ck_ps[:], AF.Relu, scale=-1.0,
                                 bias=W[f"ckbn{k}"])
            nc.scalar.activation(ek[:], mk[:], AF.Exp, scale=-1.0)
            cc[t] = (ck_ps, ek)

        def colA_b(t, k=k):
            ck_ps, ek = cc.pop(t)
            h64 = W["H64_last"] if t == NT - 1 else W["H64"]
            kt = spool.tile([P, FD], F16, tag="ktil")
            nc.vector.scalar_tensor_tensor(
                kt[:], ck_ps[:], W[f"ckb1{k}"], ek[:], ALU.add, ALU.max)
            kb_ps = ppool.tile([P, FD], F32, tag="ps")
            nc.tensor.matmul(kb_ps[:], W["P8"], kt[:])
            kb = spool.tile([P, FD], F16, tag="kbsb")
            if t % 2 == 0:
                nc.scalar.activation(kb[:], kb_ps[:], AF.Identity)
            else:
                nc.vector.tensor_copy(kb[:], kb_ps[:])
            nc.tensor.matmul(kc_ps[:], h64[:], kt[:],
                             start=(t == 0), stop=(t == NT - 1))
            cc[("b", t)] = kb

        def colA_c(t, k=k):
            kb = cc.pop(("b", t))
            cv_ps = ppool.tile([P, FD], F32, tag="ps")
            nc.tensor.matmul(cv_ps[:], W[f"cv{k}"], hp[t][:])
            cc[("c", t)] = (kb, cv_ps)

        def colA_d(t, k=k):
            kb, cv_ps = cc.pop(("c", t))
            h64 = W["H64_last"] if t == NT - 1 else W["H64"]
            vw = spool.tile([P, FD], F16, tag="vw")
            nc.vector.scalar_tensor_tensor(
                vw[:], cv_ps[:], W[f"cvb{k}"], kb[:], ALU.add, ALU.mult)
            nc.tensor.matmul(tv_ps[:], h64[:], vw[:],
                             start=(t == 0), stop=(t == NT - 1))

        for i in range(NT + 3):
            if i < NT:
                colA_a(i)
            if 1 <= i <= NT:
                colA_b(i - 1)
            if 2 <= i <= NT + 1:
                colA_c(i - 2)
            if i >= 3:
                colA_d(i - 3)
        kcs_sb = gpool.tile([64, FD], F32, tag="ln_mu")
        tvs_sb = gpool.tile([64, FD], F32, tag="ln_e2")
        nc.vector.tensor_copy(kcs_sb[:], kc_ps[:])
        nc.vector.tensor_copy(tvs_sb[:], tv_ps[:])
        ksc = gpool.tile([64, LSH], F16, tag="ksc")
        tvc = gpool.tile([64, LSH], F16, tag="tvc")
        fo1 = gpool.tile([64, LSH], F16, tag="fold1")
        fo2 = gpool.tile([64, LSH], F16, tag="fold2")
        kq, tq = _q(kcs_sb[:]), _q(tvs_sb[:])
        nc.vector.tensor_add(fo1[:], kq[:, 0, :], kq[:, 1, :])
        nc.vector.tensor_add(ksc[:], kq[:, 2, :], kq[:, 3, :])
        nc.vector.tensor_add(ksc[:], fo1[:], ksc[:])
        nc.gpsimd.tensor_add(fo2[:], tq[:, 0, :], tq[:, 1, :])
        nc.gpsimd.tensor_add(tvc[:], tq[:, 2, :], tq[:, 3, :])
        nc.gpsimd.tensor_add(tvc[:], fo2[:], tvc[:])
        kcb_ps = ppool.tile([P, FD], F32, tag="ps")
        nc.tensor.matmul(kcb_ps[:, :LSH], W["H64T"], ksc[:])
        nc.vector.tensor_copy(kc_b[:], kcb_ps[:, :LSH])
        tcb_ps = ppool.tile([P, FD], F32, tag="ps")
        nc.tensor.matmul(tcb_ps[:, :LSH], W["H64T"], tvc[:])
        nc.vector.tensor_copy(tc_b[:], tcb_ps[:, :LSH])

        # ---- column attention phase B (3-stage skew producer) ------------
        def col_b_a(t, carry, k=k):
            q_ps = ppool.tile([P, FD], F32, tag="ps")
            nc.tensor.matmul(q_ps[:], W[f"cq{k}"], hp[t][:])
            mq = spool.tile([P, FD], F16, tag="mk")
            eq = spool.tile([P, FD], F16, tag="ek")
            qb1 = spool.tile([P, FD], F16, tag="kbsb")
            nc.scalar.activation(mq[:], q_ps[:], AF.Relu, scale=-1.0,
                                 bias=W[f"cqbn{k}"])
            nc.scalar.activation(eq[:], mq[:], AF.Exp, scale=-1.0)
            nc.scalar.activation(qb1[:], q_ps[:], AF.Identity,
                                 bias=W[f"cqb1{k}"])
            carry[t] = (eq, qb1)

        def col_b_b(t, carry, k=k):
            eq, qb1 = carry.pop(t)
            qt = spool.tile([P, FD], F16, tag="ktil")
            nc.vector.tensor_max(qt[:], qb1[:], eq[:])
            prod = spool.tile([P, FD], F16, tag="vw")
            nc.vector.tensor_tensor(_q(prod[:]), _q(qt[:]), _bl(kc_b[:]),
                                    ALU.mult)
            dn_ps = ppool.tile([P, FD], F32, tag="ps")
            nc.tensor.matmul(dn_ps[:], W["P8"], prod[:])
            carry[("b", t)] = dn_ps

        def col_b_c(t, carry, k=k):
            dn_ps = carry.pop(("b", t))
            z = spool.tile([P, FD], F16, tag="z")
            nc.vector.reciprocal(z[:], dn_ps[:])
            V = spool.tile([P, FD], F16, tag="V")
            nc.vector.tensor_tensor(_q(V[:]), _q(z[:]), _bl(tc_b[:]),
                                    ALU.mult)
            att_ps = ppool.tile([P, FD], F32, tag="ps")
            nc.tensor.matmul(att_ps[:], W[f"cpbrow{k}"], ones_row[:],
                             start=True, stop=False)
            nc.tensor.matmul(att_ps[:], W[f"cp{k}"], V[:],
                             start=False, stop=True)
            carry[("c", t)] = att_ps

        def col_b_d(t, carry, k=k):
            att_ps = carry.pop(("c", t))
            x_pre = xpool.tile([P, FD], F16, tag=f"xp{t % GSZ}")
            nc.vector.scalar_tensor_tensor(
                x_pre[:], hp[t][:], W[f"cgcol{k}"], att_ps[:],
                ALU.mult, ALU.add)
            return x_pre

        ln_phase([col_b_a, col_b_b, col_b_c, col_b_d],
                 sq_dve="mix", t1_dve="mix")

        # ---- FFN ----------------------------------------------------------
        def ffn(t, carry=None, k=k, to_hp=False):
            o_ps = ppool.tile([P, FD], F32, tag="ps")
            nc.tensor.matmul(o_ps[:], W[f"f2brow{k}"], ones_row[:],
                             start=True, stop=False)
            for j in range(4):
                h_ps = ppool.tile([P, FD], F32, tag="ps")
                nc.tensor.matmul(h_ps[:], W[f"f1_{k}_{j}"], hp[t][:])
                hid = spool.tile([P, FD], F16, tag="V")
                nc.scalar.activation(hid[:], h_ps[:], AF.Gelu,
                                     bias=W[f"f1b_{k}_{j}"])
                nc.tensor.matmul(o_ps[:], W[f"f2_{k}_{j}"], hid[:],
                                 start=False, stop=(j == 3))
            if to_hp:
                nc.vector.scalar_tensor_tensor(
                    hp[t][:], hp[t][:], W[f"fgcol{k}"], o_ps[:],
                    ALU.mult, ALU.add)
                return None
            x_pre = xpool.tile([P, FD], F16, tag=f"xp{t % GSZ}")
            nc.vector.scalar_tensor_tensor(
                x_pre[:], hp[t][:], W[f"fgcol{k}"], o_ps[:],
                ALU.mult, ALU.add)
            return x_pre

        if k != N_BLOCKS - 1:
            ln_phase(ffn, sq_dve=True, t1_dve=True)
        else:
            for t in range(NT):
                ffn(t, to_hp=True)

    # ============================================================ output
    ystage = wpool.tile([32, 4 * NOG], F32, tag="ystage")
    for gi in range(NOG):
        t0, tend = gi * OGSZ, min(NT, gi * OGSZ + OGSZ)
        ntl = tend - t0
        o_ps = ppers.tile([64, FD], F32, tag="acc1")
        for tau in range(ntl):
            nc.tensor.matmul(o_ps[:32, :],
                             W["outw_lt"][:, tau * 32:(tau + 1) * 32],
                             hp[t0 + tau][:],
                             start=(tau == 0), stop=(tau == ntl - 1))
        ab = gpool.tile([64, FD], F32, tag="ln_mu")
        l1 = gpool.tile([64, FD], F32, tag="ln_e2")
        rl = gpool.tile([64, FD], F32, tag="ln_msq")
        nc.scalar.activation(ab[:32, :], o_ps[:32, :], AF.Abs, bias=W["boutc"])
        nc.scalar.activation(ab[:32, :], ab[:32, :], AF.Exp, scale=-1.0)
        nc.scalar.activation(l1[:32, :], ab[:32, :], AF.Ln, bias=W["onec"])
        nc.scalar.activation(rl[:32, :], o_ps[:32, :], AF.Relu, bias=W["boutc"])
        nc.vector.tensor_add(l1[:32, :], l1[:32, :], rl[:32, :])
        nc.vector.tensor_reduce(
            ystage[:, 4 * gi:4 * gi + 4],
            l1[:32, :].rearrange("p (q l) -> p q l", q=NQ),
            mybir.AxisListType.X, ALU.add)
    nc.sync.dma_start(yout_d[:], ystage[:])
    ctx.close()


# ================================================================ host API
_NC_CACHE = {}


def _get_nc():
    if "nc" not in _NC_CACHE:
        _NC_CACHE["nc"] = build_kernel()
    return _NC_CACHE["nc"]


def kernel(**inputs):
    from concourse.bass_utils import run_bass_kernel_spmd

    nc = _get_nc()
    w = prep_weights(inputs)

    x = np.asarray(inputs["x"])
    in_maps = []
    for core in range(N_CORES):
        b, lq = core // 4, core % 4
        xs = x[b, :, lq * LSH:(lq + 1) * LSH, :]
        xs = np.ascontiguousarray(np.transpose(xs, (0, 2, 1)),
                                  dtype=np.float16)
        m = {"xin": xs, "wpack16": w["wpack16"], "wpack32": w["wpack32"]}
        in_maps.append(m)

    res = run_bass_kernel_spmd(nc, in_maps, core_ids=list(range(N_CORES)))
    outs = [r["yout"] for r in res.results]

    y = np.zeros((B, NB_PAIRS), np.float64)
    for core in range(N_CORES):
        b = core // 4
        st = outs[core].astype(np.float64)
        for gi in range(NOG):
            for tau in range(min(OGSZ, NT - gi * OGSZ)):
                t = gi * OGSZ + tau
                for g in range(2):
                    for q in range(NQ):
                        s = 8 * t + 4 * g + q
                        if s < NB_PAIRS:
                            y[b, s] += st[2 * tau + g, 4 * gi + q]
    y /= SEQ_LEN

    out = np.zeros((B, NB_PAIRS), np.float32)
    ii, jj = np.triu_indices(NB_SEQ, 1)
    tri = {(a, c): p for p, (a, c) in enumerate(zip(ii, jj))}
    for s, (a, c) in enumerate(PAIRS):
        out[:, tri[(a, c)]] = y[:, s]
    return out
